# revision 1
# baseline (speedup 1.0000x reference)
"""Multi-head self-attention (RoPE, causal) on 8 trn2 NeuronCores.

Sharding: batch (4) x head-group (2x8 heads) = 8 shards, one per core.
Each core: QKV projection for its 8 heads -> RoPE -> causal flash
attention (scores kept transposed [k, q] so PV needs no transposes;
softmax denominators accumulated on the PE via ones-column matmuls) ->
partial o_proj over its 512 head-dims. Host sums the two partial
o_proj outputs of each batch pair (the tensor-parallel all-reduce) and
concatenates batches.

All matmuls run in float32r (FP22 multiplies, fp32 accumulate) at full
PE rate. Head-dim contraction (64) is packed two heads at a time with
tile_position row/col tiling so the 128x128 array stays full.
"""
import sys
import math

sys.path.insert(0, "/opt/trn_rl_repo")

import numpy as np
from contextlib import ExitStack

import concourse.bacc as bacc
import concourse.tile as tile
from concourse import mybir
from concourse.bass_utils import run_bass_kernel_spmd

B, S, D, H, DK = 4, 2048, 1024, 16, 64
NCORES = 8
ND = D // 128          # 8 d-tiles of the model dim
NT = S // 512          # 4 token super-blocks
NKT = S // 128         # 16 key/token 128-blocks
HPC = H // 2           # heads per core = 8
NHP = HPC // 2         # head-pairs per core = 4
F32 = mybir.dt.float32
F32R = mybir.dt.float32r
BF16 = mybir.dt.bfloat16
NEG = -30000.0

_CACHE = {}


def _build():
    nc = bacc.Bacc("TRN2", target_bir_lowering=False, num_devices=NCORES)

    xT_d = nc.dram_tensor("xT", [D, S], F32, kind="ExternalInput")
    wq_d = nc.dram_tensor("wq", [D, HPC * DK], F32, kind="ExternalInput")
    wk_d = nc.dram_tensor("wk", [D, HPC * DK], F32, kind="ExternalInput")
    wv_d = nc.dram_tensor("wv", [D, HPC * DK], F32, kind="ExternalInput")
    wo_d = nc.dram_tensor("wo", [HPC * DK, D], F32, kind="ExternalInput")
    ropeC_d = nc.dram_tensor("ropeC", [128, S], F32, kind="ExternalInput")
    ropeS_d = nc.dram_tensor("ropeS", [128, S], F32, kind="ExternalInput")
    mask_d = nc.dram_tensor("mask", [128, 128], F32, kind="ExternalInput")
    yT_d = nc.dram_tensor("yT", [D, S], F32, kind="ExternalOutput")

    aT_d = nc.dram_tensor("aT_scratch", [HPC * DK, S], F32R)

    with ExitStack() as ctx:
        tc = ctx.enter_context(tile.TileContext(nc))

        const = ctx.enter_context(tc.tile_pool(name="const", bufs=1))
        ps = ctx.enter_context(tc.tile_pool(name="ps", bufs=2, space="PSUM"))

        # ---- constants -------------------------------------------------
        ropeC = const.tile([128, S], F32)
        nc.sync.dma_start(out=ropeC, in_=ropeC_d[:, :])
        ropeS = const.tile([128, S], F32)
        nc.sync.dma_start(out=ropeS, in_=ropeS_d[:, :])
        maskt = const.tile([128, 128], mybir.dt.bfloat16)
        nc.gpsimd.dma_start(out=maskt[:, :], in_=mask_d[:, :])
        ones_f = const.tile([128, 1], F32)
        nc.vector.memset(ones_f, 1.0)
        ones_col = const.tile([128, 1], BF16)
        nc.vector.tensor_copy(ones_col, ones_f)
        ones_row_f = const.tile([33, 64], F32)
        nc.vector.memset(ones_row_f, 1.0)
        ones_row = const.tile([33, 64], BF16)
        nc.vector.tensor_copy(ones_row, ones_row_f)

        with ExitStack() as phase_a:
            xpool = phase_a.enter_context(tc.tile_pool(name="x", bufs=1))
            vpool = phase_a.enter_context(tc.tile_pool(name="v", bufs=1))
            qkpool = phase_a.enter_context(tc.tile_pool(name="qk", bufs=2))
            wpool = phase_a.enter_context(tc.tile_pool(name="w", bufs=2))
            tmp = phase_a.enter_context(tc.tile_pool(name="tmp", bufs=2))
            es = phase_a.enter_context(tc.tile_pool(name="es", bufs=3))
            apool = phase_a.enter_context(tc.tile_pool(name="a", bufs=2))
            pv = phase_a.enter_context(tc.tile_pool(name="pv", bufs=1, space="PSUM"))
            pd = phase_a.enter_context(tc.tile_pool(name="pd", bufs=2, space="PSUM"))
            pb = phase_a.enter_context(tc.tile_pool(name="pb", bufs=1, space="PSUM"))

            # ---- x^T resident ------------------------------------------
            xT = xpool.tile([128, ND, S], F32R)
            for d in range(ND):
                nc.sync.dma_start(
                    out=xT[:, d, :],
                    in_=xT_d[128 * d : 128 * (d + 1), :].bitcast(F32R),
                )

            # ---- V projection (all 8 heads): V[t, e_v] -----------------
            with ExitStack() as vphase:
                wvpool = vphase.enter_context(tc.tile_pool(name="wv", bufs=1))
                wv_sb = wvpool.tile([128, ND, HPC * DK], F32R)
                for d in range(ND):
                    nc.sync.dma_start(
                        out=wv_sb[:, d, :],
                        in_=wv_d[128 * d : 128 * (d + 1), :].bitcast(F32R),
                    )
                V = vpool.tile([128, NKT, HPC * DK], BF16)
                for t in range(NKT):
                    psv_t = ps.tile([128, 1024], F32, tag="ps")
                    psv = psv_t[:, 0:512]
                    for d in range(ND):
                        nc.tensor.matmul(
                            psv[:, :],
                            xT[:, d, 128 * t : 128 * (t + 1)],
                            wv_sb[:, d, :],
                            start=(d == 0),
                            stop=(d == ND - 1),
                        )
                    nc.vector.tensor_copy(V[:, t, :], psv[:, :])

            # ---- per head-pair: Q^T/K^T projection + rope + attention --
            # Projection of head-pair hp+1 is emitted interleaved into the
            # attention loop of hp so the PE always has independent matmuls
            # while the scalar engine computes exps (keeps HAM warm).
            def proj_units(hp, QT, KT):
                """List of emit-closures for one head-pair's Q/K projection."""
                units = []
                state = {}

                def dma_unit(w_d, wtag):
                    def emit():
                        wt = wpool.tile([128, ND, 128], F32R, tag=wtag)
                        for d in range(ND):
                            nc.sync.dma_start(
                                out=wt[:, d, :],
                                in_=w_d[
                                    128 * d : 128 * (d + 1),
                                    128 * hp : 128 * (hp + 1),
                                ].bitcast(F32R),
                            )
                        state[wtag] = wt
                    return emit

                def tb_unit(wtag, OUT, tb):
                    def emit():
                        wt = state[wtag]
                        psq_t = ps.tile([128, 1024], F32, tag="ps")
                        psq = psq_t[:, 0:512]
                        for d in range(ND):
                            nc.tensor.matmul(
                                psq[:, :],
                                wt[:, d, :],
                                xT[:, d, 512 * tb : 512 * (tb + 1)],
                                start=(d == 0),
                                stop=(d == ND - 1),
                            )
                        # rope: out = psq*C + swap32(psq)*S
                        t2 = tmp.tile([128, 512], F32, tag="t2")
                        cs = slice(512 * tb, 512 * (tb + 1))
                        for h2 in range(2):
                            b0 = 64 * h2
                            nc.vector.tensor_mul(
                                t2[b0 : b0 + 32, :],
                                psq[b0 + 32 : b0 + 64, :],
                                ropeS[b0 : b0 + 32, cs],
                            )
                            nc.vector.tensor_mul(
                                t2[b0 + 32 : b0 + 64, :],
                                psq[b0 : b0 + 32, :],
                                ropeS[b0 + 32 : b0 + 64, cs],
                            )
                        t1 = tmp.tile([128, 512], F32, tag="t1")
                        nc.vector.tensor_mul(t1[:, :], psq[:, :], ropeC[:, cs])
                        nc.vector.tensor_add(OUT[:, cs], t1[:, :], t2[:, :])
                    return emit

                for w_d, OUT, wtag in ((wq_d, QT, "wq"), (wk_d, KT, "wk")):
                    units.append(dma_unit(w_d, wtag))
                    for tb in range(NT):
                        units.append(tb_unit(wtag, OUT, tb))
                return units

            qk_tiles = []
            for hp in range(NHP):
                qt_tile = qkpool.tile([128, S], F32R, tag="qt")
                kt_tile = qkpool.tile([128, S], F32R, tag="kt")
                qk_tiles.append((qt_tile, kt_tile))

            # head-pair 0's projection up front
            for emit in proj_units(0, *qk_tiles[0]):
                emit()

            for hp in range(NHP):
                QT, KT = qk_tiles[hp]
                pending = (
                    list(proj_units(hp + 1, *qk_tiles[hp + 1]))
                    if hp + 1 < NHP
                    else []
                )
                pending.reverse()  # pop() from the front
                slot = 0

                # attention for this head pair
                for qb in range(NT):
                    po = pv.tile([128, 512], F32, tag="pv")
                    pde = pd.tile([33, 512], F32, tag="pd")
                    nkb = 4 * qb + 4
                    qslice = slice(512 * qb, 512 * (qb + 1))

                    def emit_scores(kb):
                        pss = ps.tile([128, 2, 512], F32, tag="ps")
                        for h2 in range(2):
                            b0 = 64 * h2
                            nc.tensor.matmul(
                                pss[:, h2, :],
                                KT[b0 : b0 + 64, 128 * kb : 128 * (kb + 1)],
                                QT[b0 : b0 + 64, qslice],
                                start=True,
                                stop=True,
                                tile_position=(b0, 0),
                                skip_group_check=True,
                            )
                        return pss

                    pss_cur = emit_scores(0)
                    for kb in range(nkb):
                        pss = pss_cur
                        if kb + 1 < nkb:
                            pss_cur = emit_scores(kb + 1)
                        # interleave one unit of the next head-pair's
                        # projection every few iterations
                        slot += 1
                        if pending and slot % 4 == 0:
                            pending.pop()()
                        r = kb - 4 * qb
                        q0 = 128 * r if r >= 0 else 0
                        if r >= 0:
                            # mask the diagonal 128x128 triangle of both heads
                            for h2 in range(2):
                                nc.vector.tensor_add(
                                    pss[:, h2, q0 : q0 + 128],
                                    pss[:, h2, q0 : q0 + 128],
                                    maskt[:, :],
                                )
                        es_t = es.tile([128, 2, 512], BF16, tag="es")
                        nc.scalar.activation(
                            es_t[:, :, q0:512],
                            pss[:, :, q0:512],
                            mybir.ActivationFunctionType.Exp,
                        )
                        first = kb == 0
                        last = kb == nkb - 1
                        for h2 in range(2):
                            b0 = 64 * h2
                            h_global = 2 * hp + h2
                            nc.tensor.matmul(
                                po[b0 : b0 + 64, q0:512],
                                V[:, kb, 64 * h_global : 64 * (h_global + 1)],
                                es_t[:, h2, q0:512],
                                start=first,
                                stop=last,
                                tile_position=(0, b0),
                                skip_group_check=True,
                            )
                            nc.tensor.matmul(
                                pde[32 * h2 : 32 * h2 + 1, q0:512],
                                ones_col[:, :],
                                es_t[:, h2, q0:512],
                                start=first,
                                stop=last,
                                tile_position=(0, 32 * h2),
                                skip_group_check=True,
                            )
                    # normalize: aT = po * (1/denom); one batched reciprocal
                    # over the whole denom tile (rows 1-31,33+ are unused
                    # garbage but reciprocal cost is free-dim bound anyway)
                    den_sb = tmp.tile([33, 512], BF16, tag="den")
                    with nc.allow_low_precision(reason="bf16 softmax recip"):
                        nc.vector.reciprocal(den_sb[:, :], pde[:, :])
                    psb = pb.tile([128, 512], F32, tag="pb")
                    nc.tensor.matmul(
                        psb[0:64, :],
                        ones_row[0:1, :],
                        den_sb[0:1, :],
                        start=True,
                        stop=True,
                        tile_position=(0, 0),
                        skip_group_check=True,
                    )
                    nc.tensor.matmul(
                        psb[64:128, :],
                        ones_row[32:33, :],
                        den_sb[32:33, :],
                        start=True,
                        stop=True,
                        tile_position=(32, 64),
                        skip_group_check=True,
                    )
                    recbc = tmp.tile([128, 512], F32, tag="recbc")
                    nc.vector.tensor_copy(recbc[:, :], psb[:, :])
                    aT_t = apool.tile([128, 512], F32R, tag="at")
                    nc.vector.tensor_mul(aT_t[:, :], po[:, :], recbc[:, :])
                    nc.sync.dma_start(
                        out=aT_d[
                            128 * hp : 128 * (hp + 1),
                            512 * qb : 512 * (qb + 1),
                        ],
                        in_=aT_t[:, :],
                    )

        # ---- o_proj (partial over this core's 512 head dims) -----------
        with ExitStack() as phase_b:
            wopool = phase_b.enter_context(tc.tile_pool(name="wo", bufs=1))
            a2pool = phase_b.enter_context(tc.tile_pool(name="a2", bufs=1))
            ypool = phase_b.enter_context(tc.tile_pool(name="y", bufs=2))

            wo_sb = wopool.tile([128, 4, D], F32R)
            for dd in range(4):
                nc.sync.dma_start(
                    out=wo_sb[:, dd, :],
                    in_=wo_d[128 * dd : 128 * (dd + 1), :].bitcast(F32R),
                )
            aT2 = a2pool.tile([128, 4, S], F32R)
            for dd in range(4):
                nc.sync.dma_start(
                    out=aT2[:, dd, :], in_=aT_d[128 * dd : 128 * (dd + 1), :]
                )
            for et in range(ND):
                for tb in range(NT):
                    psy_t = ps.tile([128, 1024], F32, tag="ps")
                    psy = psy_t[:, 0:512]
                    for dd in range(4):
                        nc.tensor.matmul(
                            psy[:, :],
                            wo_sb[:, dd, 128 * et : 128 * (et + 1)],
                            aT2[:, dd, 512 * tb : 512 * (tb + 1)],
                            start=(dd == 0),
                            stop=(dd == 3),
                        )
                    y_t = ypool.tile([128, 512], F32, tag="y")
                    nc.vector.tensor_copy(y_t[:, :], psy[:, :])
                    nc.sync.dma_start(
                        out=yT_d[
                            128 * et : 128 * (et + 1),
                            512 * tb : 512 * (tb + 1),
                        ],
                        in_=y_t[:, :],
                    )

    nc.compile()
    return nc


_PERM = np.concatenate([np.arange(0, DK, 2), np.arange(1, DK, 2)])


def _prep_core_inputs(x, token_positions, w_qkv, w_o, core):
    b = core // 2
    h0 = HPC * (core % 2)

    xT = np.ascontiguousarray(x[b].T.astype(np.float32))

    w_q = w_qkv[0 * D : 1 * D]
    w_k = w_qkv[1 * D : 2 * D]
    w_v = w_qkv[2 * D : 3 * D]

    def gather(w, permute, scale):
        rows = []
        for j in range(HPC):
            g = h0 + j
            blk = w[DK * g : DK * (g + 1)]
            if permute:
                blk = blk[_PERM]
            rows.append(blk)
        out = np.concatenate(rows, axis=0).astype(np.float32) * scale
        return np.ascontiguousarray(out.T)  # [D, HPC*DK]

    wq = gather(w_q, True, 1.0 / math.sqrt(DK))
    wk = gather(w_k, True, 1.0)
    wv = gather(w_v, False, 1.0)

    # w_o: [e_out, d_in]; take the d rows of this core's heads -> [512, D]
    rows = []
    for j in range(HPC):
        g = h0 + j
        rows.append(w_o[:, DK * g : DK * (g + 1)].T)
    wo = np.ascontiguousarray(np.concatenate(rows, axis=0).astype(np.float32))

    pos = token_positions.astype(np.float32)
    inv = (10000.0 ** (-(np.arange(0, DK, 2, dtype=np.float32)) / DK)).astype(
        np.float32
    )
    ang = pos[:, None] * inv[None, :]  # [S, 32]
    c = np.cos(ang).T.astype(np.float32)  # [32, S]
    s = np.sin(ang).T.astype(np.float32)
    C64 = np.concatenate([c, c], axis=0)
    S64 = np.concatenate([-s, s], axis=0)
    ropeC = np.ascontiguousarray(np.concatenate([C64, C64], axis=0))
    ropeS = np.ascontiguousarray(np.concatenate([S64, S64], axis=0))

    ki = np.arange(128)[:, None]
    qi = np.arange(128)[None, :]
    mask = np.where(ki <= qi, 0.0, NEG).astype(np.float32)

    return {
        "xT": xT,
        "wq": wq,
        "wk": wk,
        "wv": wv,
        "wo": wo,
        "ropeC": ropeC,
        "ropeS": ropeS,
        "mask": mask,
    }


def kernel(x, token_positions, w_qkv, w_o):
    x = np.asarray(x, dtype=np.float32)
    token_positions = np.asarray(token_positions)
    w_qkv = np.asarray(w_qkv, dtype=np.float32)
    w_o = np.asarray(w_o, dtype=np.float32)

    if "nc" not in _CACHE:
        _CACHE["nc"] = _build()
    nc = _CACHE["nc"]

    in_maps = [
        _prep_core_inputs(x, token_positions, w_qkv, w_o, c)
        for c in range(NCORES)
    ]
    res = run_bass_kernel_spmd(nc, in_maps, core_ids=list(range(NCORES)))
    _CACHE["last_results"] = res

    out = np.empty((B, S, D), dtype=np.float32)
    for b in range(B):
        yT = res.results[2 * b]["yT"] + res.results[2 * b + 1]["yT"]
        out[b] = yT.T
    return out



# revision 4
# speedup vs baseline: 1.1501x; 1.1501x over previous
"""Multi-head self-attention (RoPE, causal) on 8 trn2 NeuronCores.

Sharding: batch (4) x head-group (2x8 heads) = 8 shards, one per core.
Each core: QKV projection for its 8 heads -> RoPE -> causal flash
attention (scores kept transposed [k, q] so PV needs no transposes;
softmax denominators accumulated on the PE via ones-column matmuls) ->
partial o_proj over its 512 head-dims. Host sums the two partial
o_proj outputs of each batch pair (the tensor-parallel all-reduce) and
concatenates batches.

v2 changes vs baseline:
- attention output aT stays in SBUF (bf16) -- no DRAM round trip; o_proj
  (bf16 weights) is interleaved into the last head-pair's attention loop.
- softmax reciprocal via reciprocal_approx_fast (5x cheaper on DVE).
- V-projection PSUM->SBUF casts moved to the scalar engine (idle then).
- head-pair-0 projection interleaved into the V-projection phase; xT
  DMA'd in token-major chunks so the first matmuls start sooner.
- pv PSUM pool double-buffered so consecutive qb blocks overlap.
"""
import sys
import math

sys.path.insert(0, "/opt/trn_rl_repo")

import numpy as np
import ml_dtypes
from contextlib import ExitStack
from collections import deque

import concourse.bacc as bacc
import concourse.tile as tile
from concourse import mybir
from concourse.bass_utils import run_bass_kernel_spmd

B, S, D, H, DK = 4, 2048, 1024, 16, 64
NCORES = 8
ND = D // 128          # 8 d-tiles of the model dim
NT = S // 512          # 4 token super-blocks
NKT = S // 128         # 16 key/token 128-blocks
HPC = H // 2           # heads per core = 8
NHP = HPC // 2         # head-pairs per core = 4
F32 = mybir.dt.float32
F32R = mybir.dt.float32r
BF16 = mybir.dt.bfloat16
NEG = -30000.0

_CACHE = {}


def _build():
    nc = bacc.Bacc("TRN2", target_bir_lowering=False, num_devices=NCORES)

    xT_d = nc.dram_tensor("xT", [D, S], F32, kind="ExternalInput")
    wq_d = nc.dram_tensor("wq", [D, HPC * DK], F32, kind="ExternalInput")
    wk_d = nc.dram_tensor("wk", [D, HPC * DK], F32, kind="ExternalInput")
    wv_d = nc.dram_tensor("wv", [D, HPC * DK], F32, kind="ExternalInput")
    wo_d = nc.dram_tensor("wo", [HPC * DK, D], BF16, kind="ExternalInput")
    ropeC_d = nc.dram_tensor("ropeC", [128, S], F32, kind="ExternalInput")
    ropeS_d = nc.dram_tensor("ropeS", [128, S], F32, kind="ExternalInput")
    mask_d = nc.dram_tensor("mask", [128, 128], F32, kind="ExternalInput")
    yT_d = nc.dram_tensor("yT", [D, S], F32, kind="ExternalOutput")

    with ExitStack() as ctx:
        tc = ctx.enter_context(tile.TileContext(nc))

        const = ctx.enter_context(tc.tile_pool(name="const", bufs=1))
        ps = ctx.enter_context(tc.tile_pool(name="ps", bufs=2, space="PSUM"))
        pv = ctx.enter_context(tc.tile_pool(name="pv", bufs=2, space="PSUM"))
        pdb = ctx.enter_context(tc.tile_pool(name="pdb", bufs=1, space="PSUM"))
        xpool = ctx.enter_context(tc.tile_pool(name="x", bufs=1))
        vpool = ctx.enter_context(tc.tile_pool(name="v", bufs=1))
        wvpool = ctx.enter_context(tc.tile_pool(name="wv", bufs=1))
        qkpool = ctx.enter_context(tc.tile_pool(name="qk", bufs=2))
        wpool = ctx.enter_context(tc.tile_pool(name="w", bufs=2))
        atpool = ctx.enter_context(tc.tile_pool(name="at", bufs=1))
        wopool = ctx.enter_context(tc.tile_pool(name="wo", bufs=1))
        es = ctx.enter_context(tc.tile_pool(name="es", bufs=3))
        tmp = ctx.enter_context(tc.tile_pool(name="tmp", bufs=1))
        ypool = ctx.enter_context(tc.tile_pool(name="y", bufs=2))

        # ---- constants -------------------------------------------------
        ropeC = const.tile([128, S], F32)
        nc.sync.dma_start(out=ropeC, in_=ropeC_d[:, :])
        ropeS = const.tile([128, S], F32)
        nc.sync.dma_start(out=ropeS, in_=ropeS_d[:, :])
        maskt = const.tile([128, 128], BF16)
        nc.gpsimd.dma_start(out=maskt[:, :], in_=mask_d[:, :])
        ones_f = const.tile([128, 1], F32)
        nc.vector.memset(ones_f, 1.0)
        ones_col = const.tile([128, 1], BF16)
        nc.vector.tensor_copy(ones_col, ones_f)
        ones_row_f = const.tile([33, 64], F32)
        nc.vector.memset(ones_row_f, 1.0)
        ones_row = const.tile([33, 64], BF16)
        nc.vector.tensor_copy(ones_row, ones_row_f)
        # warm the ACT exp table set before any copies run on it
        warm = const.tile([128, 8], F32)
        nc.vector.memset(warm, 0.0)
        warm_out = const.tile([128, 8], BF16)
        nc.scalar.activation(
            warm_out, warm, mybir.ActivationFunctionType.Exp
        )

        # ---- weight + x DMAs (token-major x chunks) --------------------
        wv_sb = wvpool.tile([128, ND, HPC * DK], F32R)
        for d in range(ND):
            nc.sync.dma_start(
                out=wv_sb[:, d, :],
                in_=wv_d[128 * d : 128 * (d + 1), :].bitcast(F32R),
            )
        wo_sb = wopool.tile([128, 4, D], BF16)
        for dd in range(4):
            nc.sync.dma_start(
                out=wo_sb[:, dd, :],
                in_=wo_d[128 * dd : 128 * (dd + 1), :],
            )
        xT = xpool.tile([128, ND, S], F32R)
        for tb in range(NT):
            for d in range(ND):
                nc.sync.dma_start(
                    out=xT[:, d, 512 * tb : 512 * (tb + 1)],
                    in_=xT_d[
                        128 * d : 128 * (d + 1), 512 * tb : 512 * (tb + 1)
                    ].bitcast(F32R),
                )

        V = vpool.tile([128, NKT, HPC * DK], BF16)
        aT = atpool.tile([128, NHP, NT, 512], BF16)

        # ---- V projection units (PSUM->SBUF cast on scalar engine) -----
        def v_unit(t):
            def emit():
                psv_t = ps.tile([128, 1024], F32, tag="ps")
                psv = psv_t[:, 0:512]
                for d in range(ND):
                    nc.tensor.matmul(
                        psv[:, :],
                        xT[:, d, 128 * t : 128 * (t + 1)],
                        wv_sb[:, d, :],
                        start=(d == 0),
                        stop=(d == ND - 1),
                    )
                nc.scalar.copy(V[:, t, :], psv[:, :])
            return emit

        # ---- per head-pair Q^T/K^T projection + rope units -------------
        qk_tiles = {}

        def proj_units(hp):
            qt_tile = qkpool.tile([128, S], F32R, tag="qt", name=f"qt{hp}")
            kt_tile = qkpool.tile([128, S], F32R, tag="kt", name=f"kt{hp}")
            qk_tiles[hp] = (qt_tile, kt_tile)
            units = []
            state = {}

            def dma_unit(w_d, wtag):
                def emit():
                    wt = wpool.tile(
                        [128, ND, 128], F32R, tag=wtag, name=f"{wtag}{hp}"
                    )
                    for d in range(ND):
                        nc.sync.dma_start(
                            out=wt[:, d, :],
                            in_=w_d[
                                128 * d : 128 * (d + 1),
                                128 * hp : 128 * (hp + 1),
                            ].bitcast(F32R),
                        )
                    state[wtag] = wt
                return emit

            def tb_unit(wtag, OUT, tb):
                def emit():
                    wt = state[wtag]
                    psq_t = ps.tile([128, 1024], F32, tag="ps")
                    psq = psq_t[:, 0:512]
                    for d in range(ND):
                        nc.tensor.matmul(
                            psq[:, :],
                            wt[:, d, :],
                            xT[:, d, 512 * tb : 512 * (tb + 1)],
                            start=(d == 0),
                            stop=(d == ND - 1),
                        )
                    # rope: out = psq*C + swap32(psq)*S
                    t2 = tmp.tile([128, 512], F32, tag="t2")
                    cs = slice(512 * tb, 512 * (tb + 1))
                    for h2 in range(2):
                        b0 = 64 * h2
                        nc.vector.tensor_mul(
                            t2[b0 : b0 + 32, :],
                            psq[b0 + 32 : b0 + 64, :],
                            ropeS[b0 : b0 + 32, cs],
                        )
                        nc.vector.tensor_mul(
                            t2[b0 + 32 : b0 + 64, :],
                            psq[b0 : b0 + 32, :],
                            ropeS[b0 + 32 : b0 + 64, cs],
                        )
                    t1 = tmp.tile([128, 512], F32, tag="t1")
                    nc.vector.tensor_mul(t1[:, :], psq[:, :], ropeC[:, cs])
                    nc.vector.tensor_add(OUT[:, cs], t1[:, :], t2[:, :])
                return emit

            for w_d, outi, wtag in ((wq_d, 0, "wq"), (wk_d, 1, "wk")):
                units.append(dma_unit(w_d, wtag))
                for tb in range(NT):
                    units.append(
                        tb_unit(
                            wtag,
                            qt_tile if outi == 0 else kt_tile,
                            tb,
                        )
                    )
            return units

        # ---- o_proj units (aT in SBUF, bf16) ---------------------------
        def oproj_unit(tb, et):
            def emit():
                psy_t = ps.tile([128, 1024], F32, tag="ps")
                psy = psy_t[:, 0:512]
                for dd in range(4):
                    nc.tensor.matmul(
                        psy[:, :],
                        wo_sb[:, dd, 128 * et : 128 * (et + 1)],
                        aT[:, dd, tb, :],
                        start=(dd == 0),
                        stop=(dd == 3),
                    )
                y_t = ypool.tile([128, 512], F32, tag="y")
                nc.vector.tensor_copy(y_t[:, :], psy[:, :])
                nc.sync.dma_start(
                    out=yT_d[
                        128 * et : 128 * (et + 1),
                        512 * tb : 512 * (tb + 1),
                    ],
                    in_=y_t[:, :],
                )
            return emit

        # ---- phase 0: V projection + head-pair-0 projection ------------
        v_units = [v_unit(t) for t in range(NKT)]
        p0_units = proj_units(0)
        merged = list(v_units[:4])
        i = j = 0
        rest_v = v_units[4:]
        while i < len(p0_units) or j < len(rest_v):
            if j < len(rest_v):
                merged.append(rest_v[j])
                j += 1
            if i < len(p0_units):
                merged.append(p0_units[i])
                i += 1
        for u in merged:
            u()

        # ---- attention (o_proj interleaved into last head-pair) --------
        for hp in range(NHP):
            QT, KT = qk_tiles[hp]
            pending = deque(proj_units(hp + 1)) if hp + 1 < NHP else deque()
            every = 4 if hp + 1 < NHP else 1
            slot = 0

            for qb in range(NT):
                po = pv.tile([128, 512], F32, tag="pv")
                pde = pdb.tile([33, 512], F32, tag="pd")
                nkb = 4 * qb + 4
                qslice = slice(512 * qb, 512 * (qb + 1))

                def emit_scores(kb):
                    pss = ps.tile([128, 2, 512], F32, tag="ps")
                    for h2 in range(2):
                        b0 = 64 * h2
                        nc.tensor.matmul(
                            pss[:, h2, :],
                            KT[b0 : b0 + 64, 128 * kb : 128 * (kb + 1)],
                            QT[b0 : b0 + 64, qslice],
                            start=True,
                            stop=True,
                            tile_position=(b0, 0),
                            skip_group_check=True,
                        )
                    return pss

                pss_cur = emit_scores(0)
                for kb in range(nkb):
                    pss = pss_cur
                    if kb + 1 < nkb:
                        pss_cur = emit_scores(kb + 1)
                    slot += 1
                    if pending and slot % every == 0:
                        pending.popleft()()
                    r = kb - 4 * qb
                    q0 = 128 * r if r >= 0 else 0
                    if r >= 0:
                        # mask the diagonal 128x128 triangle of both heads
                        for h2 in range(2):
                            nc.vector.tensor_add(
                                pss[:, h2, q0 : q0 + 128],
                                pss[:, h2, q0 : q0 + 128],
                                maskt[:, :],
                            )
                    es_t = es.tile([128, 2, 512], BF16, tag="es")
                    nc.scalar.activation(
                        es_t[:, :, q0:512],
                        pss[:, :, q0:512],
                        mybir.ActivationFunctionType.Exp,
                    )
                    first = kb == 0
                    last = kb == nkb - 1
                    for h2 in range(2):
                        b0 = 64 * h2
                        h_global = 2 * hp + h2
                        nc.tensor.matmul(
                            po[b0 : b0 + 64, q0:512],
                            V[:, kb, 64 * h_global : 64 * (h_global + 1)],
                            es_t[:, h2, q0:512],
                            start=first,
                            stop=last,
                            tile_position=(0, b0),
                            skip_group_check=True,
                        )
                        nc.tensor.matmul(
                            pde[32 * h2 : 32 * h2 + 1, q0:512],
                            ones_col[:, :],
                            es_t[:, h2, q0:512],
                            start=first,
                            stop=last,
                            tile_position=(0, 32 * h2),
                            skip_group_check=True,
                        )
                # normalize: aT = po * (1/denom); fast approx reciprocal
                # (rows 1-31,33+ of pde are unused garbage)
                den_f = tmp.tile([33, 512], F32, tag="denf")
                nc.vector.reciprocal_approx_fast(den_f[:, :], pde[:, :])
                den = tmp.tile([33, 512], BF16, tag="den")
                nc.vector.tensor_copy(den[:, :], den_f[:, :])
                psb = pdb.tile([128, 512], F32, tag="pb")
                nc.tensor.matmul(
                    psb[0:64, :],
                    ones_row[0:1, :],
                    den[0:1, :],
                    start=True,
                    stop=True,
                    tile_position=(0, 0),
                    skip_group_check=True,
                )
                nc.tensor.matmul(
                    psb[64:128, :],
                    ones_row[32:33, :],
                    den[32:33, :],
                    start=True,
                    stop=True,
                    tile_position=(32, 64),
                    skip_group_check=True,
                )
                recbc = tmp.tile([128, 512], F32, tag="recbc")
                nc.vector.tensor_copy(recbc[:, :], psb[:, :])
                nc.vector.tensor_mul(
                    aT[:, hp, qb, :], po[:, :], recbc[:, :]
                )
                if hp == NHP - 1:
                    for et in range(ND):
                        pending.append(oproj_unit(qb, et))

            while pending:
                pending.popleft()()

    nc.compile()
    return nc


_PERM = np.concatenate([np.arange(0, DK, 2), np.arange(1, DK, 2)])


def _prep_core_inputs(x, token_positions, w_qkv, w_o, core):
    b = core // 2
    h0 = HPC * (core % 2)

    xT = np.ascontiguousarray(x[b].T.astype(np.float32))

    w_q = w_qkv[0 * D : 1 * D]
    w_k = w_qkv[1 * D : 2 * D]
    w_v = w_qkv[2 * D : 3 * D]

    def gather(w, permute, scale):
        rows = []
        for j in range(HPC):
            g = h0 + j
            blk = w[DK * g : DK * (g + 1)]
            if permute:
                blk = blk[_PERM]
            rows.append(blk)
        out = np.concatenate(rows, axis=0).astype(np.float32) * scale
        return np.ascontiguousarray(out.T)  # [D, HPC*DK]

    wq = gather(w_q, True, 1.0 / math.sqrt(DK))
    wk = gather(w_k, True, 1.0)
    wv = gather(w_v, False, 1.0)

    # w_o: [e_out, d_in]; take the d rows of this core's heads -> [512, D]
    rows = []
    for j in range(HPC):
        g = h0 + j
        rows.append(w_o[:, DK * g : DK * (g + 1)].T)
    wo = np.ascontiguousarray(
        np.concatenate(rows, axis=0).astype(ml_dtypes.bfloat16)
    )

    pos = token_positions.astype(np.float32)
    inv = (10000.0 ** (-(np.arange(0, DK, 2, dtype=np.float32)) / DK)).astype(
        np.float32
    )
    ang = pos[:, None] * inv[None, :]  # [S, 32]
    c = np.cos(ang).T.astype(np.float32)  # [32, S]
    s = np.sin(ang).T.astype(np.float32)
    C64 = np.concatenate([c, c], axis=0)
    S64 = np.concatenate([-s, s], axis=0)
    ropeC = np.ascontiguousarray(np.concatenate([C64, C64], axis=0))
    ropeS = np.ascontiguousarray(np.concatenate([S64, S64], axis=0))

    ki = np.arange(128)[:, None]
    qi = np.arange(128)[None, :]
    mask = np.where(ki <= qi, 0.0, NEG).astype(np.float32)

    return {
        "xT": xT,
        "wq": wq,
        "wk": wk,
        "wv": wv,
        "wo": wo,
        "ropeC": ropeC,
        "ropeS": ropeS,
        "mask": mask,
    }


def kernel(x, token_positions, w_qkv, w_o):
    x = np.asarray(x, dtype=np.float32)
    token_positions = np.asarray(token_positions)
    w_qkv = np.asarray(w_qkv, dtype=np.float32)
    w_o = np.asarray(w_o, dtype=np.float32)

    if "nc" not in _CACHE:
        _CACHE["nc"] = _build()
    nc = _CACHE["nc"]

    in_maps = [
        _prep_core_inputs(x, token_positions, w_qkv, w_o, c)
        for c in range(NCORES)
    ]
    res = run_bass_kernel_spmd(nc, in_maps, core_ids=list(range(NCORES)))
    _CACHE["last_results"] = res

    out = np.empty((B, S, D), dtype=np.float32)
    for b in range(B):
        yT = res.results[2 * b]["yT"] + res.results[2 * b + 1]["yT"]
        out[b] = yT.T
    return out


# revision 6
# speedup vs baseline: 1.4047x; 1.2213x over previous
"""Multi-head self-attention (RoPE, causal) on 8 trn2 NeuronCores.

Sharding: batch (4) x head-group (2x8 heads) = 8 shards, one per core.
Each core: QKV projection for its 8 heads -> RoPE -> causal flash
attention (scores kept transposed [k, q] so PV needs no transposes;
softmax denominators accumulated on the PE via ones-column matmuls) ->
partial o_proj over its 512 head-dims. Host sums the two partial
o_proj outputs of each batch pair (the tensor-parallel all-reduce) and
concatenates batches.

v3: all matmuls bf16 (f32r streams at 1.5 cyc/row on HW); po/pde pairs
emitted pair-wise so the PE column-tiles run concurrently; softmax
normalization emission deferred two iterations so the in-order PE queue
never waits on the DVE reciprocal; startup DMAs spread across idle
engine queues; aT kept in SBUF; o_proj interleaved into the last
head-pair's attention loop.
"""
import sys
import math

sys.path.insert(0, "/opt/trn_rl_repo")

import numpy as np
import ml_dtypes
from contextlib import ExitStack
from collections import deque

import concourse.bacc as bacc
import concourse.tile as tile
from concourse import mybir
from concourse.bass_utils import run_bass_kernel_spmd

B, S, D, H, DK = 4, 2048, 1024, 16, 64
NCORES = 8
ND = D // 128          # 8 d-tiles of the model dim
NT = S // 512          # 4 token super-blocks
NKT = S // 128         # 16 key/token 128-blocks
HPC = H // 2           # heads per core = 8
NHP = HPC // 2         # head-pairs per core = 4
F32 = mybir.dt.float32
F32R = mybir.dt.float32r
BF16 = mybir.dt.bfloat16
NEG = -30000.0

USE_BF16 = True        # bf16 x/w/q/k (1 cyc/row on PE) vs f32r (1.5)

_CACHE = {}


def _build():
    nc = bacc.Bacc("TRN2", target_bir_lowering=False, num_devices=NCORES)

    IDT = BF16 if USE_BF16 else F32
    ILD = BF16 if USE_BF16 else F32R

    xT_d = nc.dram_tensor("xT", [D, S], IDT, kind="ExternalInput")
    wq_d = nc.dram_tensor("wq", [D, HPC * DK], IDT, kind="ExternalInput")
    wk_d = nc.dram_tensor("wk", [D, HPC * DK], IDT, kind="ExternalInput")
    wv_d = nc.dram_tensor("wv", [D, HPC * DK], IDT, kind="ExternalInput")
    wo_d = nc.dram_tensor("wo", [HPC * DK, D], BF16, kind="ExternalInput")
    ropeC_d = nc.dram_tensor("ropeC", [128, S], F32, kind="ExternalInput")
    ropeS_d = nc.dram_tensor("ropeS", [128, S], F32, kind="ExternalInput")
    mask_d = nc.dram_tensor("mask", [128, 128], F32, kind="ExternalInput")
    yT_d = nc.dram_tensor("yT", [D, S], F32, kind="ExternalOutput")

    with ExitStack() as ctx:
        tc = ctx.enter_context(tile.TileContext(nc))

        const = ctx.enter_context(tc.tile_pool(name="const", bufs=1))
        ps = ctx.enter_context(tc.tile_pool(name="ps", bufs=2, space="PSUM"))
        pv = ctx.enter_context(tc.tile_pool(name="pv", bufs=2, space="PSUM"))
        pdb = ctx.enter_context(tc.tile_pool(name="pdb", bufs=2, space="PSUM"))
        xpool = ctx.enter_context(tc.tile_pool(name="x", bufs=1))
        vpool = ctx.enter_context(tc.tile_pool(name="v", bufs=1))
        wvpool = ctx.enter_context(tc.tile_pool(name="wv", bufs=1))
        qkpool = ctx.enter_context(tc.tile_pool(name="qk", bufs=2))
        wpool = ctx.enter_context(tc.tile_pool(name="w", bufs=2))
        atpool = ctx.enter_context(tc.tile_pool(name="at", bufs=1))
        wopool = ctx.enter_context(tc.tile_pool(name="wo", bufs=1))
        es = ctx.enter_context(tc.tile_pool(name="es", bufs=3))
        tmp = ctx.enter_context(tc.tile_pool(name="tmp", bufs=1))
        ypool = ctx.enter_context(tc.tile_pool(name="y", bufs=2))

        # ---- high-priority input DMAs, spread across idle engine queues ----
        wv_sb = wvpool.tile([128, ND, HPC * DK], ILD)
        xT = xpool.tile([128, ND, S], ILD)
        for d in range(ND):
            nc.sync.dma_start(
                out=wv_sb[:, d, :],
                in_=wv_d[128 * d : 128 * (d + 1), :],
            )
            nc.gpsimd.dma_start(
                out=xT[:, d, 0:512],
                in_=xT_d[128 * d : 128 * (d + 1), 0:512],
            )
        ropeC = const.tile([128, S], F32)
        nc.scalar.dma_start(out=ropeC, in_=ropeC_d[:, :])
        ropeS = const.tile([128, S], F32)
        nc.scalar.dma_start(out=ropeS, in_=ropeS_d[:, :])
        for tb in range(1, NT):
            for d in range(ND):
                eng = nc.sync if (d % 2 == 0) else nc.gpsimd
                eng.dma_start(
                    out=xT[:, d, 512 * tb : 512 * (tb + 1)],
                    in_=xT_d[
                        128 * d : 128 * (d + 1), 512 * tb : 512 * (tb + 1)
                    ],
                )
        wo_sb = wopool.tile([128, 4, D], BF16)
        for dd in range(4):
            nc.scalar.dma_start(
                out=wo_sb[:, dd, :],
                in_=wo_d[128 * dd : 128 * (dd + 1), :],
            )
        maskt = const.tile([128, 128], BF16)
        nc.gpsimd.dma_start(out=maskt[:, :], in_=mask_d[:, :])

        # ---- constants -------------------------------------------------
        ones_f = const.tile([128, 1], F32)
        nc.vector.memset(ones_f, 1.0)
        ones_col = const.tile([128, 1], BF16)
        nc.vector.tensor_copy(ones_col, ones_f)
        ones_row_f = const.tile([33, 64], F32)
        nc.vector.memset(ones_row_f, 1.0)
        ones_row = const.tile([33, 64], BF16)
        nc.vector.tensor_copy(ones_row, ones_row_f)
        # warm the ACT exp table set before any copies run on it
        warm = const.tile([128, 8], F32)
        nc.vector.memset(warm, 0.0)
        warm_out = const.tile([128, 8], BF16)
        nc.scalar.activation(
            warm_out, warm, mybir.ActivationFunctionType.Exp
        )

        V = vpool.tile([128, NKT, HPC * DK], BF16)
        aT = atpool.tile([128, NHP, NT, 512], BF16)

        # ---- V projection units (PSUM->SBUF cast on scalar engine) -----
        def v_unit(t):
            def emit():
                psv_t = ps.tile([128, 1024], F32, tag="ps")
                psv = psv_t[:, 0:512]
                for d in range(ND):
                    nc.tensor.matmul(
                        psv[:, :],
                        xT[:, d, 128 * t : 128 * (t + 1)],
                        wv_sb[:, d, :],
                        start=(d == 0),
                        stop=(d == ND - 1),
                    )
                nc.scalar.copy(V[:, t, :], psv[:, :])
            return emit

        # ---- per head-pair Q^T/K^T projection + rope units -------------
        qk_tiles = {}

        def proj_units(hp):
            qt_tile = qkpool.tile([128, S], IDT, tag="qt", name=f"qt{hp}")
            kt_tile = qkpool.tile([128, S], IDT, tag="kt", name=f"kt{hp}")
            qk_tiles[hp] = (qt_tile, kt_tile)
            units = []
            state = {}

            def dma_unit(w_d, wtag):
                def emit():
                    wt = wpool.tile(
                        [128, ND, 128], ILD, tag=wtag, name=f"{wtag}{hp}"
                    )
                    for d in range(ND):
                        nc.sync.dma_start(
                            out=wt[:, d, :],
                            in_=w_d[
                                128 * d : 128 * (d + 1),
                                128 * hp : 128 * (hp + 1),
                            ],
                        )
                    state[wtag] = wt
                return emit

            def tb_unit(wtag, OUT, tb):
                def emit():
                    wt = state[wtag]
                    psq_t = ps.tile([128, 1024], F32, tag="ps")
                    psq = psq_t[:, 0:512]
                    for d in range(ND):
                        nc.tensor.matmul(
                            psq[:, :],
                            wt[:, d, :],
                            xT[:, d, 512 * tb : 512 * (tb + 1)],
                            start=(d == 0),
                            stop=(d == ND - 1),
                        )
                    # rope: out = psq*C + swap32(psq)*S
                    t2 = tmp.tile([128, 512], F32, tag="t2")
                    cs = slice(512 * tb, 512 * (tb + 1))
                    for h2 in range(2):
                        b0 = 64 * h2
                        nc.vector.tensor_mul(
                            t2[b0 : b0 + 32, :],
                            psq[b0 + 32 : b0 + 64, :],
                            ropeS[b0 : b0 + 32, cs],
                        )
                        nc.vector.tensor_mul(
                            t2[b0 + 32 : b0 + 64, :],
                            psq[b0 : b0 + 32, :],
                            ropeS[b0 + 32 : b0 + 64, cs],
                        )
                    t1 = tmp.tile([128, 512], F32, tag="t1")
                    nc.vector.tensor_mul(t1[:, :], psq[:, :], ropeC[:, cs])
                    nc.vector.tensor_add(OUT[:, cs], t1[:, :], t2[:, :])
                return emit

            for w_d, outi, wtag in ((wq_d, 0, "wq"), (wk_d, 1, "wk")):
                units.append(dma_unit(w_d, wtag))
                for tb in range(NT):
                    units.append(
                        tb_unit(
                            wtag,
                            qt_tile if outi == 0 else kt_tile,
                            tb,
                        )
                    )
            return units

        # ---- o_proj units (aT in SBUF, bf16) ---------------------------
        def oproj_unit(tb, et):
            def emit():
                psy_t = ps.tile([128, 1024], F32, tag="ps")
                psy = psy_t[:, 0:512]
                for dd in range(4):
                    nc.tensor.matmul(
                        psy[:, :],
                        wo_sb[:, dd, 128 * et : 128 * (et + 1)],
                        aT[:, dd, tb, :],
                        start=(dd == 0),
                        stop=(dd == 3),
                    )
                y_t = ypool.tile([128, 512], F32, tag="y")
                nc.vector.tensor_copy(y_t[:, :], psy[:, :])
                nc.sync.dma_start(
                    out=yT_d[
                        128 * et : 128 * (et + 1),
                        512 * tb : 512 * (tb + 1),
                    ],
                    in_=y_t[:, :],
                )
            return emit

        # ---- phase 0: V projection + head-pair-0 projection ------------
        v_units = [v_unit(t) for t in range(NKT)]
        p0_units = proj_units(0)
        merged = list(v_units[:4])
        i = j = 0
        rest_v = v_units[4:]
        while i < len(p0_units) or j < len(rest_v):
            if j < len(rest_v):
                merged.append(rest_v[j])
                j += 1
            if i < len(p0_units):
                merged.append(p0_units[i])
                i += 1
        for u in merged:
            u()

        # ---- attention (o_proj interleaved into last head-pair) --------
        for hp in range(NHP):
            QT, KT = qk_tiles[hp]
            pending = deque(proj_units(hp + 1)) if hp + 1 < NHP else deque()
            every = 4 if hp + 1 < NHP else 1
            slot = 0
            norm_q = deque()

            def make_norm(hp_, qb_, po_, pde_, pend_):
                def emit():
                    # normalize: aT = po * (1/denom); fast approx recip
                    # (rows 1-31,33+ of pde are unused garbage)
                    den_f = tmp.tile([33, 512], F32, tag="denf")
                    nc.vector.reciprocal_approx_fast(den_f[:, :], pde_[:, :])
                    den = tmp.tile([33, 512], BF16, tag="den")
                    nc.vector.tensor_copy(den[:, :], den_f[:, :])
                    psb_t = ps.tile([128, 1024], F32, tag="ps")
                    psb = psb_t[:, 0:512]
                    nc.tensor.matmul(
                        psb[0:64, :],
                        ones_row[0:1, :],
                        den[0:1, :],
                        start=True,
                        stop=True,
                        tile_position=(0, 0),
                        skip_group_check=True,
                    )
                    nc.tensor.matmul(
                        psb[64:128, :],
                        ones_row[32:33, :],
                        den[32:33, :],
                        start=True,
                        stop=True,
                        tile_position=(32, 64),
                        skip_group_check=True,
                    )
                    recbc = tmp.tile([128, 512], F32, tag="recbc")
                    nc.vector.tensor_copy(recbc[:, :], psb[:, :])
                    nc.vector.tensor_mul(
                        aT[:, hp_, qb_, :], po_[:, :], recbc[:, :]
                    )
                    if hp_ == NHP - 1:
                        for et in range(ND):
                            pend_.append(oproj_unit(qb_, et))
                return emit

            for qb in range(NT):
                po = pv.tile([128, 512], F32, tag="pv")
                pde = pdb.tile([33, 512], F32, tag="pd")
                nkb = 4 * qb + 4
                qslice = slice(512 * qb, 512 * (qb + 1))

                def emit_scores(kb):
                    pss = ps.tile([128, 2, 512], F32, tag="ps")
                    for h2 in range(2):
                        b0 = 64 * h2
                        nc.tensor.matmul(
                            pss[:, h2, :],
                            KT[b0 : b0 + 64, 128 * kb : 128 * (kb + 1)],
                            QT[b0 : b0 + 64, qslice],
                            start=True,
                            stop=True,
                            tile_position=(b0, 0),
                            skip_group_check=True,
                        )
                    return pss

                pss_cur = emit_scores(0)
                for kb in range(nkb):
                    pss = pss_cur
                    if kb + 1 < nkb:
                        pss_cur = emit_scores(kb + 1)
                    slot += 1
                    if pending and slot % every == 0:
                        pending.popleft()()
                    if norm_q and kb == 2:
                        norm_q.popleft()()
                    r = kb - 4 * qb
                    q0 = 128 * r if r >= 0 else 0
                    if r >= 0:
                        # mask the diagonal 128x128 triangle of both heads
                        for h2 in range(2):
                            nc.vector.tensor_add(
                                pss[:, h2, q0 : q0 + 128],
                                pss[:, h2, q0 : q0 + 128],
                                maskt[:, :],
                            )
                    es_t = es.tile([128, 2, 512], BF16, tag="es")
                    nc.scalar.activation(
                        es_t[:, :, q0:512],
                        pss[:, :, q0:512],
                        mybir.ActivationFunctionType.Exp,
                    )
                    first = kb == 0
                    last = kb == nkb - 1
                    for h2 in range(2):
                        b0 = 64 * h2
                        h_global = 2 * hp + h2
                        nc.tensor.matmul(
                            po[b0 : b0 + 64, q0:512],
                            V[:, kb, 64 * h_global : 64 * (h_global + 1)],
                            es_t[:, h2, q0:512],
                            start=first,
                            stop=last,
                            tile_position=(0, b0),
                            skip_group_check=True,
                        )
                    for h2 in range(2):
                        nc.tensor.matmul(
                            pde[32 * h2 : 32 * h2 + 1, q0:512],
                            ones_col[:, :],
                            es_t[:, h2, q0:512],
                            start=first,
                            stop=last,
                            tile_position=(0, 32 * h2),
                            skip_group_check=True,
                        )
                norm_q.append(make_norm(hp, qb, po, pde, pending))

            while norm_q:
                norm_q.popleft()()
            while pending:
                pending.popleft()()

    nc.compile()
    return nc


_PERM = np.concatenate([np.arange(0, DK, 2), np.arange(1, DK, 2)])
_IN_NP = ml_dtypes.bfloat16 if USE_BF16 else np.float32


def _prep_core_inputs(x, token_positions, w_qkv, w_o, core):
    b = core // 2
    h0 = HPC * (core % 2)

    xT = np.ascontiguousarray(x[b].T.astype(_IN_NP))

    w_q = w_qkv[0 * D : 1 * D]
    w_k = w_qkv[1 * D : 2 * D]
    w_v = w_qkv[2 * D : 3 * D]

    def gather(w, permute, scale):
        rows = []
        for j in range(HPC):
            g = h0 + j
            blk = w[DK * g : DK * (g + 1)]
            if permute:
                blk = blk[_PERM]
            rows.append(blk)
        out = np.concatenate(rows, axis=0).astype(np.float32) * scale
        return np.ascontiguousarray(out.T.astype(_IN_NP))  # [D, HPC*DK]

    wq = gather(w_q, True, 1.0 / math.sqrt(DK))
    wk = gather(w_k, True, 1.0)
    wv = gather(w_v, False, 1.0)

    # w_o: [e_out, d_in]; take the d rows of this core's heads -> [512, D]
    rows = []
    for j in range(HPC):
        g = h0 + j
        rows.append(w_o[:, DK * g : DK * (g + 1)].T)
    wo = np.ascontiguousarray(
        np.concatenate(rows, axis=0).astype(ml_dtypes.bfloat16)
    )

    pos = token_positions.astype(np.float32)
    inv = (10000.0 ** (-(np.arange(0, DK, 2, dtype=np.float32)) / DK)).astype(
        np.float32
    )
    ang = pos[:, None] * inv[None, :]  # [S, 32]
    c = np.cos(ang).T.astype(np.float32)  # [32, S]
    s = np.sin(ang).T.astype(np.float32)
    C64 = np.concatenate([c, c], axis=0)
    S64 = np.concatenate([-s, s], axis=0)
    ropeC = np.ascontiguousarray(np.concatenate([C64, C64], axis=0))
    ropeS = np.ascontiguousarray(np.concatenate([S64, S64], axis=0))

    ki = np.arange(128)[:, None]
    qi = np.arange(128)[None, :]
    mask = np.where(ki <= qi, 0.0, NEG).astype(np.float32)

    return {
        "xT": xT,
        "wq": wq,
        "wk": wk,
        "wv": wv,
        "wo": wo,
        "ropeC": ropeC,
        "ropeS": ropeS,
        "mask": mask,
    }


def kernel(x, token_positions, w_qkv, w_o):
    x = np.asarray(x, dtype=np.float32)
    token_positions = np.asarray(token_positions)
    w_qkv = np.asarray(w_qkv, dtype=np.float32)
    w_o = np.asarray(w_o, dtype=np.float32)

    if "nc" not in _CACHE:
        _CACHE["nc"] = _build()
    nc = _CACHE["nc"]

    in_maps = [
        _prep_core_inputs(x, token_positions, w_qkv, w_o, c)
        for c in range(NCORES)
    ]
    res = run_bass_kernel_spmd(nc, in_maps, core_ids=list(range(NCORES)))
    _CACHE["last_results"] = res

    out = np.empty((B, S, D), dtype=np.float32)
    for b in range(B):
        yT = res.results[2 * b]["yT"] + res.results[2 * b + 1]["yT"]
        out[b] = yT.T
    return out


# revision 15
# speedup vs baseline: 1.5434x; 1.0988x over previous
"""Multi-head self-attention (RoPE, causal) on 8 trn2 NeuronCores.

Sharding: batch (4) x head-group (2x8 heads) = 8 shards, one per core.
Each core: QKV projection for its 8 heads -> RoPE -> causal flash
attention (scores kept transposed [k, q] so PV needs no transposes;
softmax denominators accumulated on the PE via ones-column matmuls) ->
partial o_proj over its 512 head-dims. Host sums the two partial
o_proj outputs of each batch pair (the tensor-parallel all-reduce) and
concatenates batches.

v3: all matmuls bf16 (f32r streams at 1.5 cyc/row on HW); po/pde pairs
emitted pair-wise so the PE column-tiles run concurrently; softmax
normalization emission deferred two iterations so the in-order PE queue
never waits on the DVE reciprocal; startup DMAs spread across idle
engine queues; aT kept in SBUF; o_proj interleaved into the last
head-pair's attention loop.
"""
import sys
import math

sys.path.insert(0, "/opt/trn_rl_repo")

import numpy as np
import ml_dtypes
from contextlib import ExitStack
from collections import deque

import concourse.bacc as bacc
import concourse.tile as tile
from concourse import mybir
from concourse.bass_utils import run_bass_kernel_spmd

B, S, D, H, DK = 4, 2048, 1024, 16, 64
NCORES = 8
ND = D // 128          # 8 d-tiles of the model dim
NT = S // 512          # 4 token super-blocks
NKT = S // 128         # 16 key/token 128-blocks
HPC = H // 2           # heads per core = 8
NHP = HPC // 2         # head-pairs per core = 4
F32 = mybir.dt.float32
F32R = mybir.dt.float32r
BF16 = mybir.dt.bfloat16
NEG = -30000.0

USE_BF16 = True        # bf16 x/w/q/k (1 cyc/row on PE) vs f32r (1.5)

_CACHE = {}


def _build():
    nc = bacc.Bacc("TRN2", target_bir_lowering=False, num_devices=NCORES)

    IDT = BF16 if USE_BF16 else F32
    ILD = BF16 if USE_BF16 else F32R

    xT_d = nc.dram_tensor("xT", [D, S], IDT, kind="ExternalInput")
    wq_d = nc.dram_tensor("wq", [D, HPC * DK], IDT, kind="ExternalInput")
    wk_d = nc.dram_tensor("wk", [D, HPC * DK], IDT, kind="ExternalInput")
    wv_d = nc.dram_tensor("wv", [D, HPC * DK], IDT, kind="ExternalInput")
    wo_d = nc.dram_tensor("wo", [HPC * DK, D], BF16, kind="ExternalInput")
    ropeC_d = nc.dram_tensor("ropeC", [128, S], F32, kind="ExternalInput")
    ropeS_d = nc.dram_tensor("ropeS", [128, S], F32, kind="ExternalInput")
    maskT_d = nc.dram_tensor("maskT", [128, 128], BF16, kind="ExternalInput")
    ident_d = nc.dram_tensor("ident", [128, 128], BF16, kind="ExternalInput")
    yT_d = nc.dram_tensor("yT", [D, S], F32, kind="ExternalOutput")

    with ExitStack() as ctx:
        tc = ctx.enter_context(tile.TileContext(nc))

        const = ctx.enter_context(tc.tile_pool(name="const", bufs=1))
        ps = ctx.enter_context(tc.tile_pool(name="ps", bufs=2, space="PSUM"))
        pv = ctx.enter_context(tc.tile_pool(name="pv", bufs=2, space="PSUM"))
        pdb = ctx.enter_context(tc.tile_pool(name="pdb", bufs=2, space="PSUM"))
        xpool = ctx.enter_context(tc.tile_pool(name="x", bufs=1))
        vpool = ctx.enter_context(tc.tile_pool(name="v", bufs=1))
        wvpool = ctx.enter_context(tc.tile_pool(name="wv", bufs=1))
        qkpool = ctx.enter_context(tc.tile_pool(name="qk", bufs=2))
        wpool = ctx.enter_context(tc.tile_pool(name="w", bufs=2))
        atpool = ctx.enter_context(tc.tile_pool(name="at", bufs=1))
        wopool = ctx.enter_context(tc.tile_pool(name="wo", bufs=1))
        es = ctx.enter_context(tc.tile_pool(name="es", bufs=3))
        tmp = ctx.enter_context(tc.tile_pool(name="tmp", bufs=1))
        ypool = ctx.enter_context(tc.tile_pool(name="y", bufs=2))

        # ---- high-priority input DMAs, spread across idle engine queues ----
        wv_sb = wvpool.tile([128, ND, HPC * DK], ILD)
        xT = xpool.tile([128, ND, S], ILD)
        for d in range(ND):
            nc.sync.dma_start(
                out=wv_sb[:, d, :],
                in_=wv_d[128 * d : 128 * (d + 1), :],
            )
            nc.gpsimd.dma_start(
                out=xT[:, d, 0:512],
                in_=xT_d[128 * d : 128 * (d + 1), 0:512],
            )
        ropeC = const.tile([128, S], F32)
        nc.scalar.dma_start(out=ropeC, in_=ropeC_d[:, :])
        ropeS = const.tile([128, S], F32)
        nc.scalar.dma_start(out=ropeS, in_=ropeS_d[:, :])
        for tb in range(1, NT):
            for d in range(ND):
                eng = nc.sync if (d % 2 == 0) else nc.scalar
                eng.dma_start(
                    out=xT[:, d, 512 * tb : 512 * (tb + 1)],
                    in_=xT_d[
                        128 * d : 128 * (d + 1), 512 * tb : 512 * (tb + 1)
                    ],
                )
        maskT_sb = const.tile([128, 128], BF16)
        nc.scalar.dma_start(out=maskT_sb[:, :], in_=maskT_d[:, :])
        ident_sb = const.tile([128, 128], BF16)
        nc.scalar.dma_start(out=ident_sb[:, :], in_=ident_d[:, :])
        wo_sb = wopool.tile([128, 4, D], BF16)
        for dd in range(4):
            nc.sync.dma_start(
                out=wo_sb[:, dd, :],
                in_=wo_d[128 * dd : 128 * (dd + 1), :],
            )

        # ---- constants -------------------------------------------------
        ones_f = const.tile([128, 1], F32)
        nc.vector.memset(ones_f, 1.0)
        ones_col = const.tile([128, 1], BF16)
        nc.vector.tensor_copy(ones_col, ones_f)
        ones_row_f = const.tile([33, 64], F32)
        nc.vector.memset(ones_row_f, 1.0)
        ones_row = const.tile([33, 64], BF16)
        nc.vector.tensor_copy(ones_row, ones_row_f)
        # warm the ACT exp table set before any copies run on it
        warm = const.tile([128, 8], F32)
        nc.vector.memset(warm, 0.0)
        warm_out = const.tile([128, 8], BF16)
        nc.scalar.activation(
            warm_out, warm, mybir.ActivationFunctionType.Exp
        )

        V = vpool.tile([128, NKT, HPC * DK], BF16)
        aT = atpool.tile([128, NHP, NT, 512], BF16)

        # ---- V projection units (PSUM->SBUF cast on scalar engine) -----
        def v_unit(t):
            def emit():
                psv_t = ps.tile([128, 1024], F32, tag="ps")
                psv = psv_t[:, 0:512]
                for d in range(ND):
                    nc.tensor.matmul(
                        psv[:, :],
                        xT[:, d, 128 * t : 128 * (t + 1)],
                        wv_sb[:, d, :],
                        start=(d == 0),
                        stop=(d == ND - 1),
                    )
                nc.scalar.copy(V[:, t, :], psv[:, :])
            return emit

        # ---- per head-pair Q^T/K^T projection + rope units -------------
        qk_tiles = {}

        def proj_units(hp):
            qt_tile = qkpool.tile([128, S], IDT, tag="qt", name=f"qt{hp}")
            kt_tile = qkpool.tile([128, S], IDT, tag="kt", name=f"kt{hp}")
            qk_tiles[hp] = (qt_tile, kt_tile)
            units = []
            state = {}

            def dma_unit(w_d, wtag):
                def emit():
                    wt = wpool.tile(
                        [128, ND, 128], ILD, tag=wtag, name=f"{wtag}{hp}"
                    )
                    for d in range(ND):
                        nc.gpsimd.dma_start(
                            out=wt[:, d, :],
                            in_=w_d[
                                128 * d : 128 * (d + 1),
                                128 * hp : 128 * (hp + 1),
                            ],
                        )
                    state[wtag] = wt
                return emit

            def tb_unit(wtag, OUT, tb):
                def emit():
                    wt = state[wtag]
                    psq_t = ps.tile([128, 1024], F32, tag="ps")
                    psq = psq_t[:, 0:512]
                    for d in range(ND):
                        nc.tensor.matmul(
                            psq[:, :],
                            wt[:, d, :],
                            xT[:, d, 512 * tb : 512 * (tb + 1)],
                            start=(d == 0),
                            stop=(d == ND - 1),
                        )
                    # rope: out = psq*C + swap32(psq)*S
                    t2 = tmp.tile([128, 512], F32, tag="t2")
                    cs = slice(512 * tb, 512 * (tb + 1))
                    for h2 in range(2):
                        b0 = 64 * h2
                        nc.vector.tensor_mul(
                            t2[b0 : b0 + 32, :],
                            psq[b0 + 32 : b0 + 64, :],
                            ropeS[b0 : b0 + 32, cs],
                        )
                        nc.vector.tensor_mul(
                            t2[b0 + 32 : b0 + 64, :],
                            psq[b0 : b0 + 32, :],
                            ropeS[b0 + 32 : b0 + 64, cs],
                        )
                    t1 = tmp.tile([128, 512], F32, tag="t1")
                    nc.vector.tensor_mul(t1[:, :], psq[:, :], ropeC[:, cs])
                    nc.vector.tensor_add(OUT[:, cs], t1[:, :], t2[:, :])
                return emit

            for w_d, outi, wtag in ((wq_d, 0, "wq"), (wk_d, 1, "wk")):
                units.append(dma_unit(w_d, wtag))
                for tb in range(NT):
                    units.append(
                        tb_unit(
                            wtag,
                            qt_tile if outi == 0 else kt_tile,
                            tb,
                        )
                    )
            return units

        # ---- o_proj units (aT in SBUF, bf16) ---------------------------
        def oproj_unit(tb, et):
            def emit():
                psy_t = ps.tile([128, 1024], F32, tag="ps")
                psy = psy_t[:, 0:512]
                for dd in range(4):
                    nc.tensor.matmul(
                        psy[:, :],
                        wo_sb[:, dd, 128 * et : 128 * (et + 1)],
                        aT[:, dd, tb, :],
                        start=(dd == 0),
                        stop=(dd == 3),
                    )
                y_t = ypool.tile([128, 512], F32, tag="y")
                nc.vector.tensor_copy(y_t[:, :], psy[:, :])
                nc.sync.dma_start(
                    out=yT_d[
                        128 * et : 128 * (et + 1),
                        512 * tb : 512 * (tb + 1),
                    ],
                    in_=y_t[:, :],
                )
            return emit

        # ---- phase 0: V projection + head-pair-0 projection ------------
        v_units = [v_unit(t) for t in range(NKT)]
        p0_units = proj_units(0)
        merged = list(v_units[:4])
        i = j = 0
        rest_v = v_units[4:]
        while i < len(p0_units) or j < len(rest_v):
            if j < len(rest_v):
                merged.append(rest_v[j])
                j += 1
            if i < len(p0_units):
                merged.append(p0_units[i])
                i += 1
        for u in merged:
            u()

        # ---- attention (o_proj interleaved into last head-pair) --------
        norm_a = deque()   # recip + bf16 convert (DVE), popped early
        norm_b = deque()   # broadcast matmuls + aT mul, popped later
        norm_state = {}

        def make_norm_a(key, pde_):
            def emit():
                # fast approx reciprocal of the softmax denominators
                # (rows 1-31,33+ of pde are unused garbage)
                den_f = tmp.tile([33, 512], F32, tag="denf")
                nc.vector.reciprocal_approx_fast(den_f[:, :], pde_[:, :])
                den = tmp.tile([33, 512], BF16, tag="den")
                nc.vector.tensor_copy(den[:, :], den_f[:, :])
                norm_state[key] = den
            return emit

        def make_norm_b(key, hp_, qb_, po_, pend_):
            def emit():
                den = norm_state.pop(key)
                psb_t = ps.tile([128, 1024], F32, tag="ps")
                psb = psb_t[:, 0:512]
                nc.tensor.matmul(
                    psb[0:64, :],
                    ones_row[0:1, :],
                    den[0:1, :],
                    start=True,
                    stop=True,
                    tile_position=(0, 0),
                    skip_group_check=True,
                )
                nc.tensor.matmul(
                    psb[64:128, :],
                    ones_row[32:33, :],
                    den[32:33, :],
                    start=True,
                    stop=True,
                    tile_position=(32, 64),
                    skip_group_check=True,
                )
                recbc = tmp.tile([128, 512], F32, tag="recbc")
                nc.vector.tensor_copy(recbc[:, :], psb[:, :])
                nc.vector.tensor_mul(
                    aT[:, hp_, qb_, :], po_[:, :], recbc[:, :]
                )
                if hp_ == NHP - 1:
                    for et in range(ND):
                        pend_.append(oproj_unit(qb_, et))
            return emit

        for hp in range(NHP):
            QT, KT = qk_tiles[hp]
            pending = deque(proj_units(hp + 1)) if hp + 1 < NHP else deque()
            every = 4 if hp + 1 < NHP else 1
            slot = 0

            for qb in range(NT):
                po = pv.tile([128, 512], F32, tag="pv")
                pde = pdb.tile([33, 512], F32, tag="pd")
                nkb = 4 * qb + 4
                qslice = slice(512 * qb, 512 * (qb + 1))

                def emit_scores(kb):
                    pss = ps.tile([128, 2, 512], F32, tag="ps")
                    rr = kb - 4 * qb
                    qq0 = 128 * rr if rr >= 0 else 0
                    for h2 in range(2):
                        b0 = 64 * h2
                        nc.tensor.matmul(
                            pss[:, h2, :],
                            KT[b0 : b0 + 64, 128 * kb : 128 * (kb + 1)],
                            QT[b0 : b0 + 64, qslice],
                            start=True,
                            stop=True,
                            tile_position=(b0, 0),
                            skip_group_check=True,
                        )
                    if rr >= 0:
                        # accumulate the causal -inf triangle onto the
                        # diagonal 128x128 block of both heads on the PE
                        for h2 in range(2):
                            nc.tensor.matmul(
                                pss[:, h2, qq0 : qq0 + 128],
                                maskT_sb[:, :],
                                ident_sb[:, :],
                                start=False,
                                stop=True,
                                skip_group_check=True,
                            )
                    return pss

                pss_cur = emit_scores(0)
                for kb in range(nkb):
                    pss = pss_cur
                    if kb + 1 < nkb:
                        pss_cur = emit_scores(kb + 1)
                    slot += 1
                    if pending and slot % every == 0:
                        pending.popleft()()
                    if norm_a and kb == 0:
                        norm_a.popleft()()
                    if norm_b and kb == 3:
                        norm_b.popleft()()
                    r = kb - 4 * qb
                    q0 = 128 * r if r >= 0 else 0
                    es_t = es.tile([128, 2, 512], BF16, tag="es")
                    nc.scalar.activation(
                        es_t[:, :, q0:512],
                        pss[:, :, q0:512],
                        mybir.ActivationFunctionType.Exp,
                    )
                    first = kb == 0
                    last = kb == nkb - 1
                    for h2 in range(2):
                        b0 = 64 * h2
                        h_global = 2 * hp + h2
                        nc.tensor.matmul(
                            po[b0 : b0 + 64, q0:512],
                            V[:, kb, 64 * h_global : 64 * (h_global + 1)],
                            es_t[:, h2, q0:512],
                            start=first,
                            stop=last,
                            tile_position=(0, b0),
                            skip_group_check=True,
                        )
                    for h2 in range(2):
                        nc.tensor.matmul(
                            pde[32 * h2 : 32 * h2 + 1, q0:512],
                            ones_col[:, :],
                            es_t[:, h2, q0:512],
                            start=first,
                            stop=last,
                            tile_position=(0, 32 * h2),
                            skip_group_check=True,
                        )
                key = (hp, qb)
                norm_a.append(make_norm_a(key, pde))
                norm_b.append(make_norm_b(key, hp, qb, po, pending))

            while pending:
                pending.popleft()()
            if hp == NHP - 1:
                while norm_a:
                    norm_a.popleft()()
                while norm_b:
                    norm_b.popleft()()
                while pending:
                    pending.popleft()()

    nc.compile()
    return nc


_PERM = np.concatenate([np.arange(0, DK, 2), np.arange(1, DK, 2)])
_IN_NP = ml_dtypes.bfloat16 if USE_BF16 else np.float32


def _prep_core_inputs(x, token_positions, w_qkv, w_o, core):
    b = core // 2
    h0 = HPC * (core % 2)

    xT = np.ascontiguousarray(x[b].T.astype(_IN_NP))

    w_q = w_qkv[0 * D : 1 * D]
    w_k = w_qkv[1 * D : 2 * D]
    w_v = w_qkv[2 * D : 3 * D]

    def gather(w, permute, scale):
        rows = []
        for j in range(HPC):
            g = h0 + j
            blk = w[DK * g : DK * (g + 1)]
            if permute:
                blk = blk[_PERM]
            rows.append(blk)
        out = np.concatenate(rows, axis=0).astype(np.float32) * scale
        return np.ascontiguousarray(out.T.astype(_IN_NP))  # [D, HPC*DK]

    wq = gather(w_q, True, 1.0 / math.sqrt(DK))
    wk = gather(w_k, True, 1.0)
    wv = gather(w_v, False, 1.0)

    # w_o: [e_out, d_in]; take the d rows of this core's heads -> [512, D]
    rows = []
    for j in range(HPC):
        g = h0 + j
        rows.append(w_o[:, DK * g : DK * (g + 1)].T)
    wo = np.ascontiguousarray(
        np.concatenate(rows, axis=0).astype(ml_dtypes.bfloat16)
    )

    pos = token_positions.astype(np.float32)
    inv = (10000.0 ** (-(np.arange(0, DK, 2, dtype=np.float32)) / DK)).astype(
        np.float32
    )
    ang = pos[:, None] * inv[None, :]  # [S, 32]
    c = np.cos(ang).T.astype(np.float32)  # [32, S]
    s = np.sin(ang).T.astype(np.float32)
    C64 = np.concatenate([c, c], axis=0)
    S64 = np.concatenate([-s, s], axis=0)
    ropeC = np.ascontiguousarray(np.concatenate([C64, C64], axis=0))
    ropeS = np.ascontiguousarray(np.concatenate([S64, S64], axis=0))

    ki = np.arange(128)[:, None]
    qi = np.arange(128)[None, :]
    mask = np.where(ki <= qi, 0.0, NEG).astype(np.float32)
    maskT = np.ascontiguousarray(mask.T.astype(ml_dtypes.bfloat16))
    ident = np.eye(128, dtype=np.float32).astype(ml_dtypes.bfloat16)

    return {
        "xT": xT,
        "wq": wq,
        "wk": wk,
        "wv": wv,
        "wo": wo,
        "ropeC": ropeC,
        "ropeS": ropeS,
        "maskT": maskT,
        "ident": ident,
    }


def kernel(x, token_positions, w_qkv, w_o):
    x = np.asarray(x, dtype=np.float32)
    token_positions = np.asarray(token_positions)
    w_qkv = np.asarray(w_qkv, dtype=np.float32)
    w_o = np.asarray(w_o, dtype=np.float32)

    if "nc" not in _CACHE:
        _CACHE["nc"] = _build()
    nc = _CACHE["nc"]

    in_maps = [
        _prep_core_inputs(x, token_positions, w_qkv, w_o, c)
        for c in range(NCORES)
    ]
    res = run_bass_kernel_spmd(nc, in_maps, core_ids=list(range(NCORES)))
    _CACHE["last_results"] = res

    out = np.empty((B, S, D), dtype=np.float32)
    for b in range(B):
        yT = res.results[2 * b]["yT"] + res.results[2 * b + 1]["yT"]
        out[b] = yT.T
    return out


# revision 21
# speedup vs baseline: 1.8087x; 1.1719x over previous
"""Multi-head self-attention (RoPE, causal) on 8 trn2 NeuronCores.

Sharding: batch (4) x head-group (2x8 heads) = 8 shards, one per core.
Each core: QKV projection for its 8 heads -> RoPE -> causal flash
attention (scores kept transposed [k, q] so PV needs no transposes;
softmax denominators accumulated on the PE via ones-column matmuls) ->
partial o_proj over its 512 head-dims. Host sums the two partial
o_proj outputs of each batch pair (the tensor-parallel all-reduce) and
concatenates batches.

v3: all matmuls bf16 (f32r streams at 1.5 cyc/row on HW); po/pde pairs
emitted pair-wise so the PE column-tiles run concurrently; softmax
normalization emission deferred two iterations so the in-order PE queue
never waits on the DVE reciprocal; startup DMAs spread across idle
engine queues; aT kept in SBUF; o_proj interleaved into the last
head-pair's attention loop.
"""
import sys
import math

sys.path.insert(0, "/opt/trn_rl_repo")

import numpy as np
import ml_dtypes
from contextlib import ExitStack
from collections import deque

import concourse.bacc as bacc
import concourse.tile as tile
from concourse import mybir
from concourse.bass_utils import run_bass_kernel_spmd

B, S, D, H, DK = 4, 2048, 1024, 16, 64
NCORES = 8
ND = D // 128          # 8 d-tiles of the model dim
NT = S // 512          # 4 token super-blocks
NKT = S // 128         # 16 key/token 128-blocks
HPC = H // 2           # heads per core = 8
NHP = HPC // 2         # head-pairs per core = 4
F32 = mybir.dt.float32
F32R = mybir.dt.float32r
BF16 = mybir.dt.bfloat16
NEG = -30000.0

USE_BF16 = True        # bf16 x/w/q/k (1 cyc/row on PE) vs f32r (1.5)

_CACHE = {}


def _build():
    nc = bacc.Bacc("TRN2", target_bir_lowering=False, num_devices=NCORES)

    IDT = BF16 if USE_BF16 else F32
    ILD = BF16 if USE_BF16 else F32R

    xT_d = nc.dram_tensor("xT", [D, S], IDT, kind="ExternalInput")
    wq_d = nc.dram_tensor("wq", [D, HPC * DK], IDT, kind="ExternalInput")
    wk_d = nc.dram_tensor("wk", [D, HPC * DK], IDT, kind="ExternalInput")
    wv_d = nc.dram_tensor("wv", [D, HPC * DK], IDT, kind="ExternalInput")
    wo_d = nc.dram_tensor("wo", [HPC * DK, D], BF16, kind="ExternalInput")
    ropeC_d = nc.dram_tensor("ropeC", [128, S], BF16, kind="ExternalInput")
    ropeS_d = nc.dram_tensor("ropeS", [128, S], BF16, kind="ExternalInput")
    maskT_d = nc.dram_tensor("maskT", [128, 128], BF16, kind="ExternalInput")
    ident_d = nc.dram_tensor("ident", [128, 128], BF16, kind="ExternalInput")
    yT_d = nc.dram_tensor("yT", [D, S], F32, kind="ExternalOutput")

    with ExitStack() as ctx:
        tc = ctx.enter_context(tile.TileContext(nc))

        const = ctx.enter_context(tc.tile_pool(name="const", bufs=1))
        ps = ctx.enter_context(tc.tile_pool(name="ps", bufs=2, space="PSUM"))
        pv = ctx.enter_context(tc.tile_pool(name="pv", bufs=2, space="PSUM"))
        pdb = ctx.enter_context(tc.tile_pool(name="pdb", bufs=2, space="PSUM"))
        xpool = ctx.enter_context(tc.tile_pool(name="x", bufs=1))
        vpool = ctx.enter_context(tc.tile_pool(name="v", bufs=1))
        wvpool = ctx.enter_context(tc.tile_pool(name="wv", bufs=1))
        qkpool = ctx.enter_context(tc.tile_pool(name="qk", bufs=2))
        wpool = ctx.enter_context(tc.tile_pool(name="w", bufs=2))
        atpool = ctx.enter_context(tc.tile_pool(name="at", bufs=1))
        wopool = ctx.enter_context(tc.tile_pool(name="wo", bufs=1))
        es = ctx.enter_context(tc.tile_pool(name="es", bufs=3))
        tmp = ctx.enter_context(tc.tile_pool(name="tmp", bufs=1))
        ypool = ctx.enter_context(tc.tile_pool(name="y", bufs=2))

        # ---- high-priority input DMAs, spread across idle engine queues ----
        wv_sb = wvpool.tile([128, ND, HPC * DK], ILD)
        xT = xpool.tile([128, ND, S], ILD)
        for d in range(ND):
            nc.sync.dma_start(
                out=wv_sb[:, d, :],
                in_=wv_d[128 * d : 128 * (d + 1), :],
            )
            nc.gpsimd.dma_start(
                out=xT[:, d, 0:512],
                in_=xT_d[128 * d : 128 * (d + 1), 0:512],
            )
        ropeC = const.tile([128, S], BF16)
        nc.scalar.dma_start(out=ropeC, in_=ropeC_d[:, :])
        ropeS = const.tile([128, S], BF16)
        nc.scalar.dma_start(out=ropeS, in_=ropeS_d[:, :])
        for tb in range(1, NT):
            for d in range(ND):
                eng = nc.sync if (d % 2 == 0) else nc.gpsimd
                eng.dma_start(
                    out=xT[:, d, 512 * tb : 512 * (tb + 1)],
                    in_=xT_d[
                        128 * d : 128 * (d + 1), 512 * tb : 512 * (tb + 1)
                    ],
                )
        maskT_sb = const.tile([128, 128], BF16)
        nc.scalar.dma_start(out=maskT_sb[:, :], in_=maskT_d[:, :])
        ident_sb = const.tile([128, 128], BF16)
        nc.scalar.dma_start(out=ident_sb[:, :], in_=ident_d[:, :])
        wo_sb = wopool.tile([128, 4, D], BF16)
        for dd in range(4):
            nc.sync.dma_start(
                out=wo_sb[:, dd, :],
                in_=wo_d[128 * dd : 128 * (dd + 1), :],
            )

        # ---- constants -------------------------------------------------
        ones_f = const.tile([128, 1], F32)
        nc.vector.memset(ones_f, 1.0)
        ones_col = const.tile([128, 1], BF16)
        nc.vector.tensor_copy(ones_col, ones_f)
        ones_row_f = const.tile([33, 64], F32)
        nc.vector.memset(ones_row_f, 1.0)
        ones_row = const.tile([33, 64], BF16)
        nc.vector.tensor_copy(ones_row, ones_row_f)
        # warm the ACT exp table set before any copies run on it
        warm = const.tile([128, 8], F32)
        nc.vector.memset(warm, 0.0)
        warm_out = const.tile([128, 8], BF16)
        nc.scalar.activation(
            warm_out, warm, mybir.ActivationFunctionType.Exp
        )

        V = vpool.tile([128, NKT, HPC * DK], BF16)
        aT = atpool.tile([128, NHP, NT, 512], BF16)

        # ---- V projection units (PSUM->SBUF cast on scalar engine) -----
        def v_unit(t):
            def emit():
                psv_t = ps.tile([128, 1024], F32, tag="ps")
                psv = psv_t[:, 0:512]
                for d in range(ND):
                    nc.tensor.matmul(
                        psv[:, :],
                        xT[:, d, 128 * t : 128 * (t + 1)],
                        wv_sb[:, d, :],
                        start=(d == 0),
                        stop=(d == ND - 1),
                    )
                nc.scalar.copy(V[:, t, :], psv[:, :])
            return emit

        # ---- per head-pair Q^T/K^T projection + rope units -------------
        qk_tiles = {}

        def proj_units(hp):
            qt_tile = qkpool.tile([128, S], IDT, tag="qt", name=f"qt{hp}")
            kt_tile = qkpool.tile([128, S], IDT, tag="kt", name=f"kt{hp}")
            qk_tiles[hp] = (qt_tile, kt_tile)
            units = []
            state = {}

            def dma_unit(w_d, wtag):
                def emit():
                    wt = wpool.tile(
                        [128, ND, 128], ILD, tag=wtag, name=f"{wtag}{hp}"
                    )
                    for d in range(ND):
                        nc.gpsimd.dma_start(
                            out=wt[:, d, :],
                            in_=w_d[
                                128 * d : 128 * (d + 1),
                                128 * hp : 128 * (hp + 1),
                            ],
                        )
                    state[wtag] = wt
                return emit

            def tb_unit(wtag, OUT, tb):
                def emit():
                    wt = state[wtag]
                    psq_t = ps.tile([128, 1024], F32, tag="ps")
                    psq = psq_t[:, 0:512]
                    for d in range(ND):
                        nc.tensor.matmul(
                            psq[:, :],
                            wt[:, d, :],
                            xT[:, d, 512 * tb : 512 * (tb + 1)],
                            start=(d == 0),
                            stop=(d == ND - 1),
                        )
                    # single fast PSUM read frees the psum ring slot; the
                    # rope math then runs from SBUF bf16 at 2x DVE rate
                    psq_bf = tmp.tile([128, 512], BF16, tag="psqbf")
                    nc.vector.tensor_copy(psq_bf[:, :], psq[:, :])
                    # rope: out = psq*C + swap32(psq)*S
                    t2 = tmp.tile([128, 512], BF16, tag="t2")
                    cs = slice(512 * tb, 512 * (tb + 1))
                    # ropeS rows are pre-swapped host-side so each mul's two
                    # SBUF inputs share a base partition (walrus requirement)
                    for h2 in range(2):
                        b0 = 64 * h2
                        nc.vector.tensor_mul(
                            t2[b0 : b0 + 32, :],
                            psq_bf[b0 + 32 : b0 + 64, :],
                            ropeS[b0 + 32 : b0 + 64, cs],
                        )
                        nc.vector.tensor_mul(
                            t2[b0 + 32 : b0 + 64, :],
                            psq_bf[b0 : b0 + 32, :],
                            ropeS[b0 : b0 + 32, cs],
                        )
                    t1 = tmp.tile([128, 512], BF16, tag="t1")
                    nc.vector.tensor_mul(t1[:, :], psq_bf[:, :], ropeC[:, cs])
                    nc.vector.tensor_add(OUT[:, cs], t1[:, :], t2[:, :])
                return emit

            for w_d, outi, wtag in ((wq_d, 0, "wq"), (wk_d, 1, "wk")):
                units.append(dma_unit(w_d, wtag))
                for tb in range(NT):
                    units.append(
                        tb_unit(
                            wtag,
                            qt_tile if outi == 0 else kt_tile,
                            tb,
                        )
                    )
            return units

        # ---- o_proj units (aT in SBUF, bf16) ---------------------------
        def oproj_unit(tb, et):
            def emit():
                psy_t = ps.tile([128, 1024], F32, tag="ps")
                psy = psy_t[:, 0:512]
                for dd in range(4):
                    nc.tensor.matmul(
                        psy[:, :],
                        wo_sb[:, dd, 128 * et : 128 * (et + 1)],
                        aT[:, dd, tb, :],
                        start=(dd == 0),
                        stop=(dd == 3),
                    )
                y_t = ypool.tile([128, 512], F32, tag="y")
                nc.vector.tensor_copy(y_t[:, :], psy[:, :])
                nc.sync.dma_start(
                    out=yT_d[
                        128 * et : 128 * (et + 1),
                        512 * tb : 512 * (tb + 1),
                    ],
                    in_=y_t[:, :],
                )
            return emit

        # ---- phase 0: V projection + head-pair-0 projection ------------
        v_units = [v_unit(t) for t in range(NKT)]
        p0_units = proj_units(0)
        merged = list(v_units[:4])
        i = j = 0
        rest_v = v_units[4:]
        while i < len(p0_units) or j < len(rest_v):
            if j < len(rest_v):
                merged.append(rest_v[j])
                j += 1
            if i < len(p0_units):
                merged.append(p0_units[i])
                i += 1
        for u in merged:
            u()

        # ---- attention (o_proj interleaved into last head-pair) --------
        norm_a = deque()   # recip + bf16 convert (DVE), popped early
        norm_b = deque()   # broadcast matmuls + aT mul, popped later
        norm_state = {}

        def make_norm_a(key, pde_):
            def emit():
                # fast approx reciprocal of the softmax denominators
                # (rows 1-31,33+ of pde are unused garbage)
                den_f = tmp.tile([33, 512], F32, tag="denf")
                nc.vector.reciprocal_approx_fast(den_f[:, :], pde_[:, :])
                den = tmp.tile([33, 512], BF16, tag="den")
                nc.vector.tensor_copy(den[:, :], den_f[:, :])
                norm_state[key] = den
            return emit

        def make_norm_b(key, hp_, qb_, po_, pend_):
            def emit():
                den = norm_state.pop(key)
                psb_t = ps.tile([128, 1024], F32, tag="ps")
                psb = psb_t[:, 0:512]
                nc.tensor.matmul(
                    psb[0:64, :],
                    ones_row[0:1, :],
                    den[0:1, :],
                    start=True,
                    stop=True,
                    tile_position=(0, 0),
                    skip_group_check=True,
                )
                nc.tensor.matmul(
                    psb[64:128, :],
                    ones_row[32:33, :],
                    den[32:33, :],
                    start=True,
                    stop=True,
                    tile_position=(32, 64),
                    skip_group_check=True,
                )
                recbc = tmp.tile([128, 512], F32, tag="recbc")
                nc.vector.tensor_copy(recbc[:, :], psb[:, :])
                nc.vector.tensor_mul(
                    aT[:, hp_, qb_, :], po_[:, :], recbc[:, :]
                )
                if hp_ == NHP - 1:
                    for et in range(ND):
                        pend_.append(oproj_unit(qb_, et))
            return emit

        for hp in range(NHP):
            QT, KT = qk_tiles[hp]
            pending = deque(proj_units(hp + 1)) if hp + 1 < NHP else deque()
            every = 4 if hp + 1 < NHP else 1
            slot = 0

            for qb in range(NT):
                po = pv.tile([128, 512], F32, tag="pv")
                pde = pdb.tile([33, 512], F32, tag="pd")
                nkb = 4 * qb + 4
                qslice = slice(512 * qb, 512 * (qb + 1))

                def emit_scores(kb):
                    pss = ps.tile([128, 2, 512], F32, tag="ps")
                    rr = kb - 4 * qb
                    qq0 = 128 * rr if rr >= 0 else 0
                    for h2 in range(2):
                        b0 = 64 * h2
                        nc.tensor.matmul(
                            pss[:, h2, :],
                            KT[b0 : b0 + 64, 128 * kb : 128 * (kb + 1)],
                            QT[b0 : b0 + 64, qslice],
                            start=True,
                            stop=True,
                            tile_position=(b0, 0),
                            skip_group_check=True,
                        )
                    if rr >= 0:
                        # accumulate the causal -inf triangle onto the
                        # diagonal 128x128 block of both heads on the PE
                        for h2 in range(2):
                            nc.tensor.matmul(
                                pss[:, h2, qq0 : qq0 + 128],
                                maskT_sb[:, :],
                                ident_sb[:, :],
                                start=False,
                                stop=True,
                                skip_group_check=True,
                            )
                    return pss

                pss_cur = emit_scores(0)
                for kb in range(nkb):
                    pss = pss_cur
                    if kb + 1 < nkb:
                        pss_cur = emit_scores(kb + 1)
                    slot += 1
                    if pending and slot % every == 0:
                        pending.popleft()()
                    if norm_a and kb == 0:
                        norm_a.popleft()()
                    if norm_b and kb == 3:
                        norm_b.popleft()()
                    r = kb - 4 * qb
                    q0 = 128 * r if r >= 0 else 0
                    es_t = es.tile([128, 2, 512], BF16, tag="es")
                    nc.scalar.activation(
                        es_t[:, :, q0:512],
                        pss[:, :, q0:512],
                        mybir.ActivationFunctionType.Exp,
                    )
                    first = kb == 0
                    last = kb == nkb - 1
                    for h2 in range(2):
                        b0 = 64 * h2
                        h_global = 2 * hp + h2
                        nc.tensor.matmul(
                            po[b0 : b0 + 64, q0:512],
                            V[:, kb, 64 * h_global : 64 * (h_global + 1)],
                            es_t[:, h2, q0:512],
                            start=first,
                            stop=last,
                            tile_position=(0, b0),
                            skip_group_check=True,
                        )
                    for h2 in range(2):
                        nc.tensor.matmul(
                            pde[32 * h2 : 32 * h2 + 1, q0:512],
                            ones_col[:, :],
                            es_t[:, h2, q0:512],
                            start=first,
                            stop=last,
                            tile_position=(0, 32 * h2),
                            skip_group_check=True,
                        )
                key = (hp, qb)
                norm_a.append(make_norm_a(key, pde))
                norm_b.append(make_norm_b(key, hp, qb, po, pending))

            while pending:
                pending.popleft()()
            if hp == NHP - 1:
                while norm_a:
                    norm_a.popleft()()
                while norm_b:
                    norm_b.popleft()()
                while pending:
                    pending.popleft()()

    nc.compile()
    return nc


_PERM = np.concatenate([np.arange(0, DK, 2), np.arange(1, DK, 2)])
_IN_NP = ml_dtypes.bfloat16 if USE_BF16 else np.float32


def _prep_core_inputs(x, token_positions, w_qkv, w_o, core):
    b = core // 2
    h0 = HPC * (core % 2)

    xT = np.ascontiguousarray(x[b].T.astype(_IN_NP))

    w_q = w_qkv[0 * D : 1 * D]
    w_k = w_qkv[1 * D : 2 * D]
    w_v = w_qkv[2 * D : 3 * D]

    def gather(w, permute, scale):
        rows = []
        for j in range(HPC):
            g = h0 + j
            blk = w[DK * g : DK * (g + 1)]
            if permute:
                blk = blk[_PERM]
            rows.append(blk)
        out = np.concatenate(rows, axis=0).astype(np.float32) * scale
        return np.ascontiguousarray(out.T.astype(_IN_NP))  # [D, HPC*DK]

    wq = gather(w_q, True, 1.0 / math.sqrt(DK))
    wk = gather(w_k, True, 1.0)
    wv = gather(w_v, False, 1.0)

    # w_o: [e_out, d_in]; take the d rows of this core's heads -> [512, D]
    rows = []
    for j in range(HPC):
        g = h0 + j
        rows.append(w_o[:, DK * g : DK * (g + 1)].T)
    wo = np.ascontiguousarray(
        np.concatenate(rows, axis=0).astype(ml_dtypes.bfloat16)
    )

    pos = token_positions.astype(np.float32)
    inv = (10000.0 ** (-(np.arange(0, DK, 2, dtype=np.float32)) / DK)).astype(
        np.float32
    )
    ang = pos[:, None] * inv[None, :]  # [S, 32]
    c = np.cos(ang).T.astype(np.float32)  # [32, S]
    s = np.sin(ang).T.astype(np.float32)
    C64 = np.concatenate([c, c], axis=0)
    # rows pre-swapped: row block [0:32] holds +s (multiplies x1 into the
    # x2 output slot), [32:64] holds -s (multiplies x2 into the x1 slot)
    S64 = np.concatenate([s, -s], axis=0)
    ropeC = np.ascontiguousarray(
        np.concatenate([C64, C64], axis=0).astype(ml_dtypes.bfloat16)
    )
    ropeS = np.ascontiguousarray(
        np.concatenate([S64, S64], axis=0).astype(ml_dtypes.bfloat16)
    )

    ki = np.arange(128)[:, None]
    qi = np.arange(128)[None, :]
    mask = np.where(ki <= qi, 0.0, NEG).astype(np.float32)
    maskT = np.ascontiguousarray(mask.T.astype(ml_dtypes.bfloat16))
    ident = np.eye(128, dtype=np.float32).astype(ml_dtypes.bfloat16)

    return {
        "xT": xT,
        "wq": wq,
        "wk": wk,
        "wv": wv,
        "wo": wo,
        "ropeC": ropeC,
        "ropeS": ropeS,
        "maskT": maskT,
        "ident": ident,
    }


def kernel(x, token_positions, w_qkv, w_o):
    x = np.asarray(x, dtype=np.float32)
    token_positions = np.asarray(token_positions)
    w_qkv = np.asarray(w_qkv, dtype=np.float32)
    w_o = np.asarray(w_o, dtype=np.float32)

    if "nc" not in _CACHE:
        _CACHE["nc"] = _build()
    nc = _CACHE["nc"]

    in_maps = [
        _prep_core_inputs(x, token_positions, w_qkv, w_o, c)
        for c in range(NCORES)
    ]
    res = run_bass_kernel_spmd(nc, in_maps, core_ids=list(range(NCORES)))
    _CACHE["last_results"] = res

    out = np.empty((B, S, D), dtype=np.float32)
    for b in range(B):
        yT = res.results[2 * b]["yT"] + res.results[2 * b + 1]["yT"]
        out[b] = yT.T
    return out


# revision 23
# speedup vs baseline: 1.8874x; 1.0435x over previous
"""Multi-head self-attention (RoPE, causal) on 8 trn2 NeuronCores.

Sharding: batch (4) x head-group (2x8 heads) = 8 shards, one per core.
Each core: QKV projection for its 8 heads -> RoPE -> causal flash
attention (scores kept transposed [k, q] so PV needs no transposes;
softmax denominators accumulated on the PE via ones-column matmuls) ->
partial o_proj over its 512 head-dims. Host sums the two partial
o_proj outputs of each batch pair (the tensor-parallel all-reduce) and
concatenates batches.

v3: all matmuls bf16 (f32r streams at 1.5 cyc/row on HW); po/pde pairs
emitted pair-wise so the PE column-tiles run concurrently; softmax
normalization emission deferred two iterations so the in-order PE queue
never waits on the DVE reciprocal; startup DMAs spread across idle
engine queues; aT kept in SBUF; o_proj interleaved into the last
head-pair's attention loop.
"""
import sys
import math

sys.path.insert(0, "/opt/trn_rl_repo")

import numpy as np
import ml_dtypes
from contextlib import ExitStack
from collections import deque

import concourse.bacc as bacc
import concourse.tile as tile
from concourse import mybir
from concourse.bass_utils import run_bass_kernel_spmd

B, S, D, H, DK = 4, 2048, 1024, 16, 64
NCORES = 8
ND = D // 128          # 8 d-tiles of the model dim
NT = S // 512          # 4 token super-blocks
NKT = S // 128         # 16 key/token 128-blocks
HPC = H // 2           # heads per core = 8
NHP = HPC // 2         # head-pairs per core = 4
F32 = mybir.dt.float32
F32R = mybir.dt.float32r
BF16 = mybir.dt.bfloat16
NEG = -30000.0

USE_BF16 = True        # bf16 x/w/q/k (1 cyc/row on PE) vs f32r (1.5)

_CACHE = {}


def _build():
    nc = bacc.Bacc("TRN2", target_bir_lowering=False, num_devices=NCORES)

    IDT = BF16 if USE_BF16 else F32
    ILD = BF16 if USE_BF16 else F32R

    xT_d = nc.dram_tensor("xT", [D, S], IDT, kind="ExternalInput")
    wq_d = nc.dram_tensor("wq", [D, HPC * DK], IDT, kind="ExternalInput")
    wk_d = nc.dram_tensor("wk", [D, HPC * DK], IDT, kind="ExternalInput")
    wv_d = nc.dram_tensor("wv", [D, HPC * DK], IDT, kind="ExternalInput")
    wo_d = nc.dram_tensor("wo", [HPC * DK, D], BF16, kind="ExternalInput")
    ropeC_d = nc.dram_tensor("ropeC", [128, S], BF16, kind="ExternalInput")
    ropeS_d = nc.dram_tensor("ropeS", [128, S], BF16, kind="ExternalInput")
    maskT_d = nc.dram_tensor("maskT", [128, 128], BF16, kind="ExternalInput")
    ident_d = nc.dram_tensor("ident", [128, 128], BF16, kind="ExternalInput")
    yT_d = nc.dram_tensor("yT", [D, S], F32, kind="ExternalOutput")

    with ExitStack() as ctx:
        tc = ctx.enter_context(tile.TileContext(nc))

        const = ctx.enter_context(tc.tile_pool(name="const", bufs=1))
        ps = ctx.enter_context(tc.tile_pool(name="ps", bufs=2, space="PSUM"))
        pv = ctx.enter_context(tc.tile_pool(name="pv", bufs=2, space="PSUM"))
        pdb = ctx.enter_context(tc.tile_pool(name="pdb", bufs=2, space="PSUM"))
        xpool = ctx.enter_context(tc.tile_pool(name="x", bufs=1))
        vpool = ctx.enter_context(tc.tile_pool(name="v", bufs=1))
        wvpool = ctx.enter_context(tc.tile_pool(name="wv", bufs=1))
        qkpool = ctx.enter_context(tc.tile_pool(name="qk", bufs=2))
        wpool = ctx.enter_context(tc.tile_pool(name="w", bufs=2))
        atpool = ctx.enter_context(tc.tile_pool(name="at", bufs=1))
        wopool = ctx.enter_context(tc.tile_pool(name="wo", bufs=1))
        es = ctx.enter_context(tc.tile_pool(name="es", bufs=3))
        tmp = ctx.enter_context(tc.tile_pool(name="tmp", bufs=1))
        ypool = ctx.enter_context(tc.tile_pool(name="y", bufs=2))

        # ---- high-priority input DMAs, spread across idle engine queues ----
        wv_sb = wvpool.tile([128, ND, HPC * DK], ILD)
        xT = xpool.tile([128, ND, S], ILD)
        for d in range(ND):
            nc.sync.dma_start(
                out=wv_sb[:, d, :],
                in_=wv_d[128 * d : 128 * (d + 1), :],
            )
            nc.gpsimd.dma_start(
                out=xT[:, d, 0:512],
                in_=xT_d[128 * d : 128 * (d + 1), 0:512],
            )
        ropeC = const.tile([128, S], BF16)
        nc.scalar.dma_start(out=ropeC, in_=ropeC_d[:, :])
        ropeS = const.tile([128, S], BF16)
        nc.scalar.dma_start(out=ropeS, in_=ropeS_d[:, :])
        for tb in range(1, NT):
            for d in range(ND):
                eng = nc.sync if (d % 2 == 0) else nc.gpsimd
                eng.dma_start(
                    out=xT[:, d, 512 * tb : 512 * (tb + 1)],
                    in_=xT_d[
                        128 * d : 128 * (d + 1), 512 * tb : 512 * (tb + 1)
                    ],
                )
        maskT_sb = const.tile([128, 128], BF16)
        nc.scalar.dma_start(out=maskT_sb[:, :], in_=maskT_d[:, :])
        ident_sb = const.tile([128, 128], BF16)
        nc.scalar.dma_start(out=ident_sb[:, :], in_=ident_d[:, :])
        wo_sb = wopool.tile([128, 4, D], BF16)
        for dd in range(4):
            nc.sync.dma_start(
                out=wo_sb[:, dd, :],
                in_=wo_d[128 * dd : 128 * (dd + 1), :],
            )

        # ---- constants -------------------------------------------------
        ones_f = const.tile([128, 1], F32)
        nc.vector.memset(ones_f, 1.0)
        ones_col = const.tile([128, 1], BF16)
        nc.vector.tensor_copy(ones_col, ones_f)
        ones_row_f = const.tile([33, 64], F32)
        nc.vector.memset(ones_row_f, 1.0)
        ones_row = const.tile([33, 64], BF16)
        nc.vector.tensor_copy(ones_row, ones_row_f)
        # warm the ACT exp table set before any copies run on it
        warm = const.tile([128, 8], F32)
        nc.vector.memset(warm, 0.0)
        warm_out = const.tile([128, 8], BF16)
        nc.scalar.activation(
            warm_out, warm, mybir.ActivationFunctionType.Exp
        )

        V = vpool.tile([128, NKT, HPC * DK], BF16)
        aT = atpool.tile([128, NHP, NT, 512], BF16)

        # ---- V projection units (PSUM->SBUF cast on scalar engine) -----
        def v_unit(t):
            def emit():
                psv_t = ps.tile([128, 1024], F32, tag="ps")
                psv = psv_t[:, 0:512]
                for d in range(ND):
                    nc.tensor.matmul(
                        psv[:, :],
                        xT[:, d, 128 * t : 128 * (t + 1)],
                        wv_sb[:, d, :],
                        start=(d == 0),
                        stop=(d == ND - 1),
                    )
                nc.scalar.copy(V[:, t, :], psv[:, :])
            return emit

        # ---- per head-pair Q^T/K^T projection + rope units -------------
        qk_tiles = {}

        def proj_units(hp):
            qt_tile = qkpool.tile([128, S], IDT, tag="qt", name=f"qt{hp}")
            kt_tile = qkpool.tile([128, S], IDT, tag="kt", name=f"kt{hp}")
            qk_tiles[hp] = (qt_tile, kt_tile)
            units = []
            state = {}

            def dma_unit(w_d, wtag):
                def emit():
                    wt = wpool.tile(
                        [128, ND, 128], ILD, tag=wtag, name=f"{wtag}{hp}"
                    )
                    for d in range(ND):
                        nc.gpsimd.dma_start(
                            out=wt[:, d, :],
                            in_=w_d[
                                128 * d : 128 * (d + 1),
                                128 * hp : 128 * (hp + 1),
                            ],
                        )
                    state[wtag] = wt
                return emit

            def mm_unit(wtag, tb):
                def emit():
                    wt = state[wtag]
                    psq_t = ps.tile([128, 1024], F32, tag="ps")
                    psq = psq_t[:, 0:512]
                    for d in range(ND):
                        nc.tensor.matmul(
                            psq[:, :],
                            wt[:, d, :],
                            xT[:, d, 512 * tb : 512 * (tb + 1)],
                            start=(d == 0),
                            stop=(d == ND - 1),
                        )
                    # single fast PSUM read frees the psum ring slot; the
                    # rope math then runs from SBUF bf16 at 2x DVE rate
                    psq_bf = tmp.tile(
                        [128, 512], BF16, tag="psqbf", bufs=2
                    )
                    nc.vector.tensor_copy(psq_bf[:, :], psq[:, :])
                    state[(wtag, tb)] = psq_bf
                return emit

            def rope_unit(wtag, OUT, tb):
                def emit():
                    psq_bf = state.pop((wtag, tb))
                    # rope: out = psq*C + swap32(psq)*S
                    t2 = tmp.tile([128, 512], BF16, tag="t2")
                    cs = slice(512 * tb, 512 * (tb + 1))
                    # ropeS rows are pre-swapped host-side so each mul's two
                    # SBUF inputs share a base partition (walrus requirement)
                    for h2 in range(2):
                        b0 = 64 * h2
                        nc.vector.tensor_mul(
                            t2[b0 : b0 + 32, :],
                            psq_bf[b0 + 32 : b0 + 64, :],
                            ropeS[b0 + 32 : b0 + 64, cs],
                        )
                        nc.vector.tensor_mul(
                            t2[b0 + 32 : b0 + 64, :],
                            psq_bf[b0 : b0 + 32, :],
                            ropeS[b0 : b0 + 32, cs],
                        )
                    t1 = tmp.tile([128, 512], BF16, tag="t1")
                    nc.vector.tensor_mul(t1[:, :], psq_bf[:, :], ropeC[:, cs])
                    nc.vector.tensor_add(OUT[:, cs], t1[:, :], t2[:, :])
                return emit

            for w_d, outi, wtag in ((wq_d, 0, "wq"), (wk_d, 1, "wk")):
                units.append(dma_unit(w_d, wtag))
                for tb in range(NT):
                    units.append(mm_unit(wtag, tb))
                    units.append(
                        rope_unit(
                            wtag,
                            qt_tile if outi == 0 else kt_tile,
                            tb,
                        )
                    )
            return units

        # ---- o_proj units (aT in SBUF, bf16) ---------------------------
        def oproj_unit(tb, et):
            def emit():
                psy_t = ps.tile([128, 1024], F32, tag="ps")
                psy = psy_t[:, 0:512]
                for dd in range(4):
                    nc.tensor.matmul(
                        psy[:, :],
                        wo_sb[:, dd, 128 * et : 128 * (et + 1)],
                        aT[:, dd, tb, :],
                        start=(dd == 0),
                        stop=(dd == 3),
                    )
                y_t = ypool.tile([128, 512], F32, tag="y")
                nc.vector.tensor_copy(y_t[:, :], psy[:, :])
                nc.sync.dma_start(
                    out=yT_d[
                        128 * et : 128 * (et + 1),
                        512 * tb : 512 * (tb + 1),
                    ],
                    in_=y_t[:, :],
                )
            return emit

        # ---- phase 0: V projection + head-pair-0 projection ------------
        v_units = [v_unit(t) for t in range(NKT)]
        p0_units = proj_units(0)
        merged = list(v_units[:4])
        i = j = 0
        rest_v = v_units[4:]
        while i < len(p0_units) or j < len(rest_v):
            if j < len(rest_v):
                merged.append(rest_v[j])
                j += 1
            if i < len(p0_units):
                merged.append(p0_units[i])
                i += 1
        for u in merged:
            u()

        # ---- attention (o_proj interleaved into last head-pair) --------
        norm_a = deque()   # recip + bf16 convert (DVE), popped early
        norm_b = deque()   # broadcast matmuls + aT mul, popped later
        norm_state = {}

        def make_norm_a(key, pde_):
            def emit():
                # fast approx reciprocal of the softmax denominators
                # (rows 1-31,33+ of pde are unused garbage)
                den_f = tmp.tile([33, 512], F32, tag="denf")
                nc.vector.reciprocal_approx_fast(den_f[:, :], pde_[:, :])
                den = tmp.tile([33, 512], BF16, tag="den")
                nc.vector.tensor_copy(den[:, :], den_f[:, :])
                norm_state[key] = den
            return emit

        def make_norm_b(key, hp_, qb_, po_, pend_):
            def emit():
                den = norm_state.pop(key)
                psb_t = ps.tile([128, 1024], F32, tag="ps")
                psb = psb_t[:, 0:512]
                nc.tensor.matmul(
                    psb[0:64, :],
                    ones_row[0:1, :],
                    den[0:1, :],
                    start=True,
                    stop=True,
                    tile_position=(0, 0),
                    skip_group_check=True,
                )
                nc.tensor.matmul(
                    psb[64:128, :],
                    ones_row[32:33, :],
                    den[32:33, :],
                    start=True,
                    stop=True,
                    tile_position=(32, 64),
                    skip_group_check=True,
                )
                recbc = tmp.tile([128, 512], F32, tag="recbc")
                nc.vector.tensor_copy(recbc[:, :], psb[:, :])
                nc.vector.tensor_mul(
                    aT[:, hp_, qb_, :], po_[:, :], recbc[:, :]
                )
                if hp_ == NHP - 1:
                    for et in range(ND):
                        pend_.append(oproj_unit(qb_, et))
            return emit

        def emit_scores(hp, qb, kb):
            QT, KT = qk_tiles[hp]
            qslice0 = 512 * qb
            pss = ps.tile([128, 2, 512], F32, tag="ps")
            rr = kb - 4 * qb
            qq0 = 128 * rr if rr >= 0 else 0
            for h2 in range(2):
                b0 = 64 * h2
                nc.tensor.matmul(
                    pss[:, h2, qq0:512],
                    KT[b0 : b0 + 64, 128 * kb : 128 * (kb + 1)],
                    QT[b0 : b0 + 64, qslice0 + qq0 : qslice0 + 512],
                    start=True,
                    stop=True,
                    tile_position=(b0, 0),
                    skip_group_check=True,
                )
            if rr >= 0:
                # accumulate the causal -inf triangle onto the
                # diagonal 128x128 block of both heads on the PE
                for h2 in range(2):
                    nc.tensor.matmul(
                        pss[:, h2, qq0 : qq0 + 128],
                        maskT_sb[:, :],
                        ident_sb[:, :],
                        start=False,
                        stop=True,
                        skip_group_check=True,
                    )
            return pss

        iters = [
            (hp, qb, kb)
            for hp in range(NHP)
            for qb in range(NT)
            for kb in range(4 * qb + 4)
        ]
        pending = deque()
        every = 4
        slot = 0
        po = pde = None
        pss_next = emit_scores(*iters[0])
        for idx, (hp, qb, kb) in enumerate(iters):
            if kb == 0:
                if qb == 0:
                    pending = (
                        deque(proj_units(hp + 1))
                        if hp + 1 < NHP
                        else pending
                    )
                    every = 2 if hp + 1 < NHP else 1
                    slot = 0
                po = pv.tile([128, 512], F32, tag="pv")
                pde = pdb.tile([33, 512], F32, tag="pd")
            nkb = 4 * qb + 4
            pss = pss_next
            if idx + 1 < len(iters):
                pss_next = emit_scores(*iters[idx + 1])
            slot += 1
            if pending and slot % every == 0:
                pending.popleft()()
            if norm_a and kb == 0:
                norm_a.popleft()()
            if norm_b and kb == 3:
                norm_b.popleft()()
            r = kb - 4 * qb
            q0 = 128 * r if r >= 0 else 0
            es_t = es.tile([128, 2, 512], BF16, tag="es")
            nc.scalar.activation(
                es_t[:, :, q0:512],
                pss[:, :, q0:512],
                mybir.ActivationFunctionType.Exp,
            )
            first = kb == 0
            last = kb == nkb - 1
            for h2 in range(2):
                b0 = 64 * h2
                h_global = 2 * hp + h2
                nc.tensor.matmul(
                    po[b0 : b0 + 64, q0:512],
                    V[:, kb, 64 * h_global : 64 * (h_global + 1)],
                    es_t[:, h2, q0:512],
                    start=first,
                    stop=last,
                    tile_position=(0, b0),
                    skip_group_check=True,
                )
            for h2 in range(2):
                nc.tensor.matmul(
                    pde[32 * h2 : 32 * h2 + 1, q0:512],
                    ones_col[:, :],
                    es_t[:, h2, q0:512],
                    start=first,
                    stop=last,
                    tile_position=(0, 32 * h2),
                    skip_group_check=True,
                )
            if last:
                norm_a.append(make_norm_a((hp, qb), pde))
                norm_b.append(make_norm_b((hp, qb), hp, qb, po, pending))

        while pending:
            pending.popleft()()
        while norm_a:
            norm_a.popleft()()
        while norm_b:
            norm_b.popleft()()
        while pending:
            pending.popleft()()

    nc.compile()
    return nc


_PERM = np.concatenate([np.arange(0, DK, 2), np.arange(1, DK, 2)])
_IN_NP = ml_dtypes.bfloat16 if USE_BF16 else np.float32


def _prep_core_inputs(x, token_positions, w_qkv, w_o, core):
    b = core // 2
    h0 = HPC * (core % 2)

    xT = np.ascontiguousarray(x[b].T.astype(_IN_NP))

    w_q = w_qkv[0 * D : 1 * D]
    w_k = w_qkv[1 * D : 2 * D]
    w_v = w_qkv[2 * D : 3 * D]

    def gather(w, permute, scale):
        rows = []
        for j in range(HPC):
            g = h0 + j
            blk = w[DK * g : DK * (g + 1)]
            if permute:
                blk = blk[_PERM]
            rows.append(blk)
        out = np.concatenate(rows, axis=0).astype(np.float32) * scale
        return np.ascontiguousarray(out.T.astype(_IN_NP))  # [D, HPC*DK]

    wq = gather(w_q, True, 1.0 / math.sqrt(DK))
    wk = gather(w_k, True, 1.0)
    wv = gather(w_v, False, 1.0)

    # w_o: [e_out, d_in]; take the d rows of this core's heads -> [512, D]
    rows = []
    for j in range(HPC):
        g = h0 + j
        rows.append(w_o[:, DK * g : DK * (g + 1)].T)
    wo = np.ascontiguousarray(
        np.concatenate(rows, axis=0).astype(ml_dtypes.bfloat16)
    )

    pos = token_positions.astype(np.float32)
    inv = (10000.0 ** (-(np.arange(0, DK, 2, dtype=np.float32)) / DK)).astype(
        np.float32
    )
    ang = pos[:, None] * inv[None, :]  # [S, 32]
    c = np.cos(ang).T.astype(np.float32)  # [32, S]
    s = np.sin(ang).T.astype(np.float32)
    C64 = np.concatenate([c, c], axis=0)
    # rows pre-swapped: row block [0:32] holds +s (multiplies x1 into the
    # x2 output slot), [32:64] holds -s (multiplies x2 into the x1 slot)
    S64 = np.concatenate([s, -s], axis=0)
    ropeC = np.ascontiguousarray(
        np.concatenate([C64, C64], axis=0).astype(ml_dtypes.bfloat16)
    )
    ropeS = np.ascontiguousarray(
        np.concatenate([S64, S64], axis=0).astype(ml_dtypes.bfloat16)
    )

    ki = np.arange(128)[:, None]
    qi = np.arange(128)[None, :]
    mask = np.where(ki <= qi, 0.0, NEG).astype(np.float32)
    maskT = np.ascontiguousarray(mask.T.astype(ml_dtypes.bfloat16))
    ident = np.eye(128, dtype=np.float32).astype(ml_dtypes.bfloat16)

    return {
        "xT": xT,
        "wq": wq,
        "wk": wk,
        "wv": wv,
        "wo": wo,
        "ropeC": ropeC,
        "ropeS": ropeS,
        "maskT": maskT,
        "ident": ident,
    }


def kernel(x, token_positions, w_qkv, w_o):
    x = np.asarray(x, dtype=np.float32)
    token_positions = np.asarray(token_positions)
    w_qkv = np.asarray(w_qkv, dtype=np.float32)
    w_o = np.asarray(w_o, dtype=np.float32)

    if "nc" not in _CACHE:
        _CACHE["nc"] = _build()
    nc = _CACHE["nc"]

    in_maps = [
        _prep_core_inputs(x, token_positions, w_qkv, w_o, c)
        for c in range(NCORES)
    ]
    res = run_bass_kernel_spmd(nc, in_maps, core_ids=list(range(NCORES)))
    _CACHE["last_results"] = res

    out = np.empty((B, S, D), dtype=np.float32)
    for b in range(B):
        yT = res.results[2 * b]["yT"] + res.results[2 * b + 1]["yT"]
        out[b] = yT.T
    return out


# revision 29
# speedup vs baseline: 1.9041x; 1.0088x over previous
"""Multi-head self-attention (RoPE, causal) on 8 trn2 NeuronCores.

Sharding: batch (4) x head-group (2x8 heads) = 8 shards, one per core.
Each core: QKV projection for its 8 heads -> RoPE -> causal flash
attention (scores kept transposed [k, q] so PV needs no transposes;
softmax denominators accumulated on the PE via ones-column matmuls) ->
partial o_proj over its 512 head-dims. Host sums the two partial
o_proj outputs of each batch pair (the tensor-parallel all-reduce) and
concatenates batches.

v3: all matmuls bf16 (f32r streams at 1.5 cyc/row on HW); po/pde pairs
emitted pair-wise so the PE column-tiles run concurrently; softmax
normalization emission deferred two iterations so the in-order PE queue
never waits on the DVE reciprocal; startup DMAs spread across idle
engine queues; aT kept in SBUF; o_proj interleaved into the last
head-pair's attention loop.
"""
import sys
import math

sys.path.insert(0, "/opt/trn_rl_repo")

import numpy as np
import ml_dtypes
from contextlib import ExitStack
from collections import deque

import concourse.bacc as bacc
import concourse.tile as tile
from concourse import mybir
from concourse.bass_utils import run_bass_kernel_spmd

B, S, D, H, DK = 4, 2048, 1024, 16, 64
NCORES = 8
ND = D // 128          # 8 d-tiles of the model dim
NT = S // 512          # 4 token super-blocks
NKT = S // 128         # 16 key/token 128-blocks
HPC = H // 2           # heads per core = 8
NHP = HPC // 2         # head-pairs per core = 4
F32 = mybir.dt.float32
F32R = mybir.dt.float32r
BF16 = mybir.dt.bfloat16
NEG = -30000.0

USE_BF16 = True        # bf16 x/w/q/k (1 cyc/row on PE) vs f32r (1.5)

_CACHE = {}


def _build():
    nc = bacc.Bacc("TRN2", target_bir_lowering=False, num_devices=NCORES)

    IDT = BF16 if USE_BF16 else F32
    ILD = BF16 if USE_BF16 else F32R

    xT_d = nc.dram_tensor("xT", [D, S], IDT, kind="ExternalInput")
    wq_d = nc.dram_tensor("wq", [D, HPC * DK], IDT, kind="ExternalInput")
    wk_d = nc.dram_tensor("wk", [D, HPC * DK], IDT, kind="ExternalInput")
    wv_d = nc.dram_tensor("wv", [D, HPC * DK], IDT, kind="ExternalInput")
    wo_d = nc.dram_tensor("wo", [HPC * DK, D], BF16, kind="ExternalInput")
    ropeC_d = nc.dram_tensor("ropeC", [128, S], BF16, kind="ExternalInput")
    ropeS_d = nc.dram_tensor("ropeS", [128, S], BF16, kind="ExternalInput")
    maskT_d = nc.dram_tensor("maskT", [128, 128], BF16, kind="ExternalInput")
    ident_d = nc.dram_tensor("ident", [128, 128], BF16, kind="ExternalInput")
    yT_d = nc.dram_tensor("yT", [D, S], F32, kind="ExternalOutput")

    with ExitStack() as ctx:
        tc = ctx.enter_context(tile.TileContext(nc))

        const = ctx.enter_context(tc.tile_pool(name="const", bufs=1))
        ps = ctx.enter_context(tc.tile_pool(name="ps", bufs=2, space="PSUM"))
        pv = ctx.enter_context(tc.tile_pool(name="pv", bufs=2, space="PSUM"))
        pdb = ctx.enter_context(tc.tile_pool(name="pdb", bufs=1, space="PSUM"))
        pq = ctx.enter_context(tc.tile_pool(name="pq", bufs=1, space="PSUM"))
        xpool = ctx.enter_context(tc.tile_pool(name="x", bufs=1))
        vpool = ctx.enter_context(tc.tile_pool(name="v", bufs=1))
        wvpool = ctx.enter_context(tc.tile_pool(name="wv", bufs=1))
        qkpool = ctx.enter_context(tc.tile_pool(name="qk", bufs=2))
        wpool = ctx.enter_context(tc.tile_pool(name="w", bufs=2))
        atpool = ctx.enter_context(tc.tile_pool(name="at", bufs=1))
        wopool = ctx.enter_context(tc.tile_pool(name="wo", bufs=1))
        es = ctx.enter_context(tc.tile_pool(name="es", bufs=3))
        tmp = ctx.enter_context(tc.tile_pool(name="tmp", bufs=1))
        ypool = ctx.enter_context(tc.tile_pool(name="y", bufs=2))

        # ---- high-priority input DMAs, spread across idle engine queues ----
        wv_sb = wvpool.tile([128, ND, HPC * DK], ILD)
        xT = xpool.tile([128, ND, S], ILD)
        for d in range(ND):
            nc.sync.dma_start(
                out=wv_sb[:, d, :],
                in_=wv_d[128 * d : 128 * (d + 1), :],
            )
            nc.gpsimd.dma_start(
                out=xT[:, d, 0:512],
                in_=xT_d[128 * d : 128 * (d + 1), 0:512],
            )
        ropeC = const.tile([128, S], BF16)
        nc.scalar.dma_start(out=ropeC, in_=ropeC_d[:, :])
        ropeS = const.tile([128, S], BF16)
        nc.scalar.dma_start(out=ropeS, in_=ropeS_d[:, :])
        for tb in range(1, NT):
            for d in range(ND):
                eng = nc.sync if (d % 2 == 0) else nc.gpsimd
                eng.dma_start(
                    out=xT[:, d, 512 * tb : 512 * (tb + 1)],
                    in_=xT_d[
                        128 * d : 128 * (d + 1), 512 * tb : 512 * (tb + 1)
                    ],
                )
        maskT_sb = const.tile([128, 128], BF16)
        nc.scalar.dma_start(out=maskT_sb[:, :], in_=maskT_d[:, :])
        ident_sb = const.tile([128, 128], BF16)
        nc.scalar.dma_start(out=ident_sb[:, :], in_=ident_d[:, :])
        wo_sb = wopool.tile([128, 4, D], BF16)
        for dd in range(4):
            nc.sync.dma_start(
                out=wo_sb[:, dd, :],
                in_=wo_d[128 * dd : 128 * (dd + 1), :],
            )

        # ---- constants -------------------------------------------------
        ones_f = const.tile([128, 1], F32)
        nc.vector.memset(ones_f, 1.0)
        ones_col = const.tile([128, 1], BF16)
        nc.vector.tensor_copy(ones_col, ones_f)
        ones_row_f = const.tile([33, 64], F32)
        nc.vector.memset(ones_row_f, 1.0)
        ones_row = const.tile([33, 64], BF16)
        nc.vector.tensor_copy(ones_row, ones_row_f)
        # warm the ACT exp table set before any copies run on it
        warm = const.tile([128, 8], F32)
        nc.vector.memset(warm, 0.0)
        warm_out = const.tile([128, 8], BF16)
        nc.scalar.activation(
            warm_out, warm, mybir.ActivationFunctionType.Exp
        )

        V = vpool.tile([128, NKT, HPC * DK], BF16)
        aT = atpool.tile([128, NHP, NT, 512], BF16)

        # ---- V projection units (PSUM->SBUF cast on scalar engine) -----
        def v_unit(t):
            def emit():
                psv_t = ps.tile([128, 1024], F32, tag="ps")
                psv = psv_t[:, 0:512]
                for d in range(ND):
                    nc.tensor.matmul(
                        psv[:, :],
                        xT[:, d, 128 * t : 128 * (t + 1)],
                        wv_sb[:, d, :],
                        start=(d == 0),
                        stop=(d == ND - 1),
                    )
                nc.scalar.copy(V[:, t, :], psv[:, :])
            return emit

        # ---- per head-pair Q^T/K^T projection + rope units -------------
        qk_tiles = {}

        def proj_units(hp):
            qt_tile = qkpool.tile([128, S], IDT, tag="qt", name=f"qt{hp}")
            kt_tile = qkpool.tile([128, S], IDT, tag="kt", name=f"kt{hp}")
            qk_tiles[hp] = (qt_tile, kt_tile)
            units = []
            state = {}

            def dma_unit(w_d, wtag):
                def emit():
                    wt = wpool.tile(
                        [128, ND, 128], ILD, tag=wtag, name=f"{wtag}{hp}"
                    )
                    for d in range(ND):
                        nc.gpsimd.dma_start(
                            out=wt[:, d, :],
                            in_=w_d[
                                128 * d : 128 * (d + 1),
                                128 * hp : 128 * (hp + 1),
                            ],
                        )
                    state[wtag] = wt
                return emit

            def mm_unit(wtag, tb):
                def emit():
                    wt = state[wtag]
                    psq = pq.tile([128, 512], F32, tag="pq")
                    for d in range(ND):
                        nc.tensor.matmul(
                            psq[:, :],
                            wt[:, d, :],
                            xT[:, d, 512 * tb : 512 * (tb + 1)],
                            start=(d == 0),
                            stop=(d == ND - 1),
                        )
                    # single fast PSUM read frees the psum ring slot; the
                    # rope math then runs from SBUF bf16 at 2x DVE rate
                    psq_bf = tmp.tile(
                        [128, 512], BF16, tag="psqbf", bufs=2
                    )
                    nc.vector.tensor_copy(psq_bf[:, :], psq[:, :])
                    state[(wtag, tb)] = psq_bf
                return emit

            def rope_unit(wtag, OUT, tb):
                def emit():
                    psq_bf = state.pop((wtag, tb))
                    # rope: out = psq*C + swap32(psq)*S
                    t2 = tmp.tile([128, 512], BF16, tag="t2")
                    cs = slice(512 * tb, 512 * (tb + 1))
                    # ropeS rows are pre-swapped host-side so each mul's two
                    # SBUF inputs share a base partition (walrus requirement)
                    for h2 in range(2):
                        b0 = 64 * h2
                        nc.vector.tensor_mul(
                            t2[b0 : b0 + 32, :],
                            psq_bf[b0 + 32 : b0 + 64, :],
                            ropeS[b0 + 32 : b0 + 64, cs],
                        )
                        nc.vector.tensor_mul(
                            t2[b0 + 32 : b0 + 64, :],
                            psq_bf[b0 : b0 + 32, :],
                            ropeS[b0 : b0 + 32, cs],
                        )
                    t1 = tmp.tile([128, 512], BF16, tag="t1")
                    nc.vector.tensor_mul(t1[:, :], psq_bf[:, :], ropeC[:, cs])
                    nc.vector.tensor_add(OUT[:, cs], t1[:, :], t2[:, :])
                return emit

            for w_d, outi, wtag in ((wq_d, 0, "wq"), (wk_d, 1, "wk")):
                units.append(dma_unit(w_d, wtag))
                for tb in range(NT):
                    units.append(mm_unit(wtag, tb))
                    units.append(
                        rope_unit(
                            wtag,
                            qt_tile if outi == 0 else kt_tile,
                            tb,
                        )
                    )
            return units

        # ---- o_proj units (aT in SBUF, bf16) ---------------------------
        def oproj_unit(tb, et):
            def emit():
                psy = pq.tile([128, 512], F32, tag="pq")
                for dd in range(4):
                    nc.tensor.matmul(
                        psy[:, :],
                        wo_sb[:, dd, 128 * et : 128 * (et + 1)],
                        aT[:, dd, tb, :],
                        start=(dd == 0),
                        stop=(dd == 3),
                    )
                y_t = ypool.tile([128, 512], F32, tag="y")
                nc.vector.tensor_copy(y_t[:, :], psy[:, :])
                nc.sync.dma_start(
                    out=yT_d[
                        128 * et : 128 * (et + 1),
                        512 * tb : 512 * (tb + 1),
                    ],
                    in_=y_t[:, :],
                )
            return emit

        # ---- phase 0: V projection + head-pair-0 projection ------------
        v_units = [v_unit(t) for t in range(NKT)]
        p0_units = proj_units(0)
        merged = list(v_units[:4])
        i = j = 0
        rest_v = v_units[4:]
        while i < len(p0_units) or j < len(rest_v):
            if j < len(rest_v):
                merged.append(rest_v[j])
                j += 1
            if i < len(p0_units):
                merged.append(p0_units[i])
                i += 1
        for u in merged:
            u()

        # ---- attention (o_proj interleaved into last head-pair) --------
        norm_a = deque()   # recip + bf16 convert (DVE), popped early
        norm_b = deque()   # broadcast matmuls + aT mul, popped later
        norm_state = {}

        def make_norm_a(key, pde_):
            def emit():
                # fast approx reciprocal of the softmax denominators
                # (rows 1-31,33+ of pde are unused garbage)
                den_f = tmp.tile([33, 512], F32, tag="denf")
                nc.vector.reciprocal_approx_fast(den_f[:, :], pde_[:, :])
                den = tmp.tile([33, 512], BF16, tag="den")
                nc.vector.tensor_copy(den[:, :], den_f[:, :])
                norm_state[key] = den
            return emit

        def make_norm_b(key, hp_, qb_, po_, pend_):
            def emit():
                den = norm_state.pop(key)
                psb = pq.tile([128, 512], F32, tag="pq")
                nc.tensor.matmul(
                    psb[0:64, :],
                    ones_row[0:1, :],
                    den[0:1, :],
                    start=True,
                    stop=True,
                    tile_position=(0, 0),
                    skip_group_check=True,
                )
                nc.tensor.matmul(
                    psb[64:128, :],
                    ones_row[32:33, :],
                    den[32:33, :],
                    start=True,
                    stop=True,
                    tile_position=(32, 64),
                    skip_group_check=True,
                )
                recbc = tmp.tile([128, 512], F32, tag="recbc")
                nc.vector.tensor_copy(recbc[:, :], psb[:, :])
                nc.vector.tensor_mul(
                    aT[:, hp_, qb_, :], po_[:, :], recbc[:, :]
                )
                if hp_ == NHP - 1:
                    for et in range(ND):
                        pend_.append(oproj_unit(qb_, et))
            return emit

        def emit_scores(hp, qb, kb):
            QT, KT = qk_tiles[hp]
            qslice0 = 512 * qb
            pss = ps.tile([128, 2, 512], F32, tag="ps")
            rr = kb - 4 * qb
            qq0 = 128 * rr if rr >= 0 else 0
            for h2 in range(2):
                b0 = 64 * h2
                nc.tensor.matmul(
                    pss[:, h2, qq0:512],
                    KT[b0 : b0 + 64, 128 * kb : 128 * (kb + 1)],
                    QT[b0 : b0 + 64, qslice0 + qq0 : qslice0 + 512],
                    start=True,
                    stop=True,
                    tile_position=(b0, 0),
                    skip_group_check=True,
                )
            if rr >= 0:
                # accumulate the causal -inf triangle onto the
                # diagonal 128x128 block of both heads on the PE
                for h2 in range(2):
                    nc.tensor.matmul(
                        pss[:, h2, qq0 : qq0 + 128],
                        maskT_sb[:, :],
                        ident_sb[:, :],
                        start=False,
                        stop=True,
                        skip_group_check=True,
                    )
            return pss

        iters = [
            (hp, qb, kb)
            for hp in range(NHP)
            for qb in range(NT)
            for kb in range(4 * qb + 4)
        ]
        pending = deque()
        every = 4
        slot = 0
        po = pde = None
        pss_next = emit_scores(*iters[0])
        for idx, (hp, qb, kb) in enumerate(iters):
            if kb == 0:
                if qb == 0:
                    pending = (
                        deque(proj_units(hp + 1))
                        if hp + 1 < NHP
                        else pending
                    )
                    every = 2 if hp + 1 < NHP else 1
                    slot = 0
                po = pv.tile([128, 512], F32, tag="pv")
                pde = pdb.tile([33, 512], F32, tag="pd")
            nkb = 4 * qb + 4
            pss = pss_next
            if idx + 1 < len(iters):
                pss_next = emit_scores(*iters[idx + 1])
            slot += 1
            if pending and slot % every == 0:
                pending.popleft()()
            if norm_b and kb == 3:
                norm_b.popleft()()
            r = kb - 4 * qb
            q0 = 128 * r if r >= 0 else 0
            es_t = es.tile([128, 2, 512], BF16, tag="es")
            nc.scalar.activation(
                es_t[:, :, q0:512],
                pss[:, :, q0:512],
                mybir.ActivationFunctionType.Exp,
            )
            first = kb == 0
            last = kb == nkb - 1
            for h2 in range(2):
                b0 = 64 * h2
                h_global = 2 * hp + h2
                nc.tensor.matmul(
                    po[b0 : b0 + 64, q0:512],
                    V[:, kb, 64 * h_global : 64 * (h_global + 1)],
                    es_t[:, h2, q0:512],
                    start=first,
                    stop=last,
                    tile_position=(0, b0),
                    skip_group_check=True,
                )
            for h2 in range(2):
                nc.tensor.matmul(
                    pde[32 * h2 : 32 * h2 + 1, q0:512],
                    ones_col[:, :],
                    es_t[:, h2, q0:512],
                    start=first,
                    stop=last,
                    tile_position=(0, 32 * h2),
                    skip_group_check=True,
                )
            if last:
                # reciprocal emitted right away (frees the pd slot early);
                # the broadcast+aT-mul half is deferred into the next qb
                make_norm_a((hp, qb), pde)()
                norm_b.append(make_norm_b((hp, qb), hp, qb, po, pending))

        while pending:
            pending.popleft()()
        while norm_b:
            norm_b.popleft()()
        while pending:
            pending.popleft()()

    nc.compile()
    return nc


_PERM = np.concatenate([np.arange(0, DK, 2), np.arange(1, DK, 2)])
_IN_NP = ml_dtypes.bfloat16 if USE_BF16 else np.float32


def _prep_core_inputs(x, token_positions, w_qkv, w_o, core):
    b = core // 2
    h0 = HPC * (core % 2)

    xT = np.ascontiguousarray(x[b].T.astype(_IN_NP))

    w_q = w_qkv[0 * D : 1 * D]
    w_k = w_qkv[1 * D : 2 * D]
    w_v = w_qkv[2 * D : 3 * D]

    def gather(w, permute, scale):
        rows = []
        for j in range(HPC):
            g = h0 + j
            blk = w[DK * g : DK * (g + 1)]
            if permute:
                blk = blk[_PERM]
            rows.append(blk)
        out = np.concatenate(rows, axis=0).astype(np.float32) * scale
        return np.ascontiguousarray(out.T.astype(_IN_NP))  # [D, HPC*DK]

    wq = gather(w_q, True, 1.0 / math.sqrt(DK))
    wk = gather(w_k, True, 1.0)
    wv = gather(w_v, False, 1.0)

    # w_o: [e_out, d_in]; take the d rows of this core's heads -> [512, D]
    rows = []
    for j in range(HPC):
        g = h0 + j
        rows.append(w_o[:, DK * g : DK * (g + 1)].T)
    wo = np.ascontiguousarray(
        np.concatenate(rows, axis=0).astype(ml_dtypes.bfloat16)
    )

    pos = token_positions.astype(np.float32)
    inv = (10000.0 ** (-(np.arange(0, DK, 2, dtype=np.float32)) / DK)).astype(
        np.float32
    )
    ang = pos[:, None] * inv[None, :]  # [S, 32]
    c = np.cos(ang).T.astype(np.float32)  # [32, S]
    s = np.sin(ang).T.astype(np.float32)
    C64 = np.concatenate([c, c], axis=0)
    # rows pre-swapped: row block [0:32] holds +s (multiplies x1 into the
    # x2 output slot), [32:64] holds -s (multiplies x2 into the x1 slot)
    S64 = np.concatenate([s, -s], axis=0)
    ropeC = np.ascontiguousarray(
        np.concatenate([C64, C64], axis=0).astype(ml_dtypes.bfloat16)
    )
    ropeS = np.ascontiguousarray(
        np.concatenate([S64, S64], axis=0).astype(ml_dtypes.bfloat16)
    )

    ki = np.arange(128)[:, None]
    qi = np.arange(128)[None, :]
    mask = np.where(ki <= qi, 0.0, NEG).astype(np.float32)
    maskT = np.ascontiguousarray(mask.T.astype(ml_dtypes.bfloat16))
    ident = np.eye(128, dtype=np.float32).astype(ml_dtypes.bfloat16)

    return {
        "xT": xT,
        "wq": wq,
        "wk": wk,
        "wv": wv,
        "wo": wo,
        "ropeC": ropeC,
        "ropeS": ropeS,
        "maskT": maskT,
        "ident": ident,
    }


def kernel(x, token_positions, w_qkv, w_o):
    x = np.asarray(x, dtype=np.float32)
    token_positions = np.asarray(token_positions)
    w_qkv = np.asarray(w_qkv, dtype=np.float32)
    w_o = np.asarray(w_o, dtype=np.float32)

    if "nc" not in _CACHE:
        _CACHE["nc"] = _build()
    nc = _CACHE["nc"]

    in_maps = [
        _prep_core_inputs(x, token_positions, w_qkv, w_o, c)
        for c in range(NCORES)
    ]
    res = run_bass_kernel_spmd(nc, in_maps, core_ids=list(range(NCORES)))
    _CACHE["last_results"] = res

    out = np.empty((B, S, D), dtype=np.float32)
    for b in range(B):
        yT = res.results[2 * b]["yT"] + res.results[2 * b + 1]["yT"]
        out[b] = yT.T
    return out


# revision 33
# speedup vs baseline: 1.9047x; 1.0003x over previous
"""Multi-head self-attention (RoPE, causal) on 8 trn2 NeuronCores.

Sharding: batch (4) x head-group (2x8 heads) = 8 shards, one per core.
Each core: QKV projection for its 8 heads -> RoPE -> causal flash
attention (scores kept transposed [k, q] so PV needs no transposes;
softmax denominators accumulated on the PE via ones-column matmuls) ->
partial o_proj over its 512 head-dims. Host sums the two partial
o_proj outputs of each batch pair (the tensor-parallel all-reduce) and
concatenates batches.

v3: all matmuls bf16 (f32r streams at 1.5 cyc/row on HW); po/pde pairs
emitted pair-wise so the PE column-tiles run concurrently; softmax
normalization emission deferred two iterations so the in-order PE queue
never waits on the DVE reciprocal; startup DMAs spread across idle
engine queues; aT kept in SBUF; o_proj interleaved into the last
head-pair's attention loop.
"""
import sys
import math

sys.path.insert(0, "/opt/trn_rl_repo")

import numpy as np
import ml_dtypes
from contextlib import ExitStack
from collections import deque

import concourse.bacc as bacc
import concourse.tile as tile
from concourse import mybir
from concourse.bass_utils import run_bass_kernel_spmd

B, S, D, H, DK = 4, 2048, 1024, 16, 64
NCORES = 8
ND = D // 128          # 8 d-tiles of the model dim
NT = S // 512          # 4 token super-blocks
NKT = S // 128         # 16 key/token 128-blocks
HPC = H // 2           # heads per core = 8
NHP = HPC // 2         # head-pairs per core = 4
F32 = mybir.dt.float32
F32R = mybir.dt.float32r
BF16 = mybir.dt.bfloat16
NEG = -30000.0

USE_BF16 = True        # bf16 x/w/q/k (1 cyc/row on PE) vs f32r (1.5)

_CACHE = {}


def _build():
    nc = bacc.Bacc("TRN2", target_bir_lowering=False, num_devices=NCORES)

    IDT = BF16 if USE_BF16 else F32
    ILD = BF16 if USE_BF16 else F32R

    xT_d = nc.dram_tensor("xT", [D, S], IDT, kind="ExternalInput")
    wq_d = nc.dram_tensor("wq", [D, HPC * DK], IDT, kind="ExternalInput")
    wk_d = nc.dram_tensor("wk", [D, HPC * DK], IDT, kind="ExternalInput")
    wv_d = nc.dram_tensor("wv", [D, HPC * DK], IDT, kind="ExternalInput")
    wo_d = nc.dram_tensor("wo", [HPC * DK, D], BF16, kind="ExternalInput")
    ropeC_d = nc.dram_tensor("ropeC", [128, S], BF16, kind="ExternalInput")
    ropeS_d = nc.dram_tensor("ropeS", [128, S], BF16, kind="ExternalInput")
    maskT_d = nc.dram_tensor("maskT", [128, 128], BF16, kind="ExternalInput")
    ident_d = nc.dram_tensor("ident", [128, 128], BF16, kind="ExternalInput")
    yT_d = nc.dram_tensor("yT", [D, S], F32, kind="ExternalOutput")

    with ExitStack() as ctx:
        tc = ctx.enter_context(tile.TileContext(nc))

        const = ctx.enter_context(tc.tile_pool(name="const", bufs=1))
        ps = ctx.enter_context(tc.tile_pool(name="ps", bufs=2, space="PSUM"))
        pv = ctx.enter_context(tc.tile_pool(name="pv", bufs=2, space="PSUM"))
        pdb = ctx.enter_context(tc.tile_pool(name="pdb", bufs=1, space="PSUM"))
        pq = ctx.enter_context(tc.tile_pool(name="pq", bufs=1, space="PSUM"))
        xpool = ctx.enter_context(tc.tile_pool(name="x", bufs=1))
        vpool = ctx.enter_context(tc.tile_pool(name="v", bufs=1))
        wvpool = ctx.enter_context(tc.tile_pool(name="wv", bufs=1))
        qkpool = ctx.enter_context(tc.tile_pool(name="qk", bufs=2))
        wpool = ctx.enter_context(tc.tile_pool(name="w", bufs=2))
        atpool = ctx.enter_context(tc.tile_pool(name="at", bufs=1))
        wopool = ctx.enter_context(tc.tile_pool(name="wo", bufs=1))
        es = ctx.enter_context(tc.tile_pool(name="es", bufs=3))
        tmp = ctx.enter_context(tc.tile_pool(name="tmp", bufs=1))
        ypool = ctx.enter_context(tc.tile_pool(name="y", bufs=2))

        # ---- high-priority input DMAs, spread across idle engine queues ----
        wv_sb = wvpool.tile([128, ND, HPC * DK], ILD)
        xT = xpool.tile([128, ND, S], ILD)
        for d in range(ND):
            nc.sync.dma_start(
                out=wv_sb[:, d, :],
                in_=wv_d[128 * d : 128 * (d + 1), :],
            )
            nc.gpsimd.dma_start(
                out=xT[:, d, 0:512],
                in_=xT_d[128 * d : 128 * (d + 1), 0:512],
            )
        ropeC = const.tile([128, S], BF16)
        nc.scalar.dma_start(out=ropeC, in_=ropeC_d[:, :])
        ropeS = const.tile([128, S], BF16)
        nc.scalar.dma_start(out=ropeS, in_=ropeS_d[:, :])
        for tb in range(1, NT):
            for d in range(ND):
                eng = nc.sync if (d % 2 == 0) else nc.gpsimd
                eng.dma_start(
                    out=xT[:, d, 512 * tb : 512 * (tb + 1)],
                    in_=xT_d[
                        128 * d : 128 * (d + 1), 512 * tb : 512 * (tb + 1)
                    ],
                )
        maskT_sb = const.tile([128, 128], BF16)
        nc.scalar.dma_start(out=maskT_sb[:, :], in_=maskT_d[:, :])
        ident_sb = const.tile([128, 128], BF16)
        nc.scalar.dma_start(out=ident_sb[:, :], in_=ident_d[:, :])
        wo_sb = wopool.tile([128, 4, D], BF16)
        for dd in range(4):
            nc.sync.dma_start(
                out=wo_sb[:, dd, :],
                in_=wo_d[128 * dd : 128 * (dd + 1), :],
            )

        # ---- constants -------------------------------------------------
        ones_f = const.tile([128, 1], F32)
        nc.vector.memset(ones_f, 1.0)
        ones_col = const.tile([128, 1], BF16)
        nc.vector.tensor_copy(ones_col, ones_f)
        ones_row_f = const.tile([33, 64], F32)
        nc.vector.memset(ones_row_f, 1.0)
        ones_row = const.tile([33, 64], BF16)
        nc.vector.tensor_copy(ones_row, ones_row_f)
        # warm the ACT exp table set before any copies run on it
        warm = const.tile([128, 8], F32)
        nc.vector.memset(warm, 0.0)
        warm_out = const.tile([128, 8], BF16)
        nc.scalar.activation(
            warm_out, warm, mybir.ActivationFunctionType.Exp
        )
        # keep the PE busy through the startup DMA wait so the HAM clock
        # gate is at full rate (K=8/8) when the real matmuls arrive
        ones128 = const.tile([128, 64], BF16)
        nc.vector.memset(ones128, 1.0)
        pwarm = pq.tile([128, 512], F32, tag="pq", name="pwarm")
        for _ in range(64):
            nc.tensor.matmul(
                pwarm[0:64, 0:64],
                ones128[:, :],
                ones128[:, :],
                start=True,
                stop=True,
                skip_group_check=True,
            )

        V = vpool.tile([128, NKT, HPC * DK], BF16)
        aT = atpool.tile([128, NHP, NT, 512], BF16)

        # ---- V projection units (PSUM->SBUF cast on scalar engine) -----
        def v_unit(t):
            def emit():
                psv_t = ps.tile([128, 1024], F32, tag="ps")
                psv = psv_t[:, 0:512]
                for d in range(ND):
                    nc.tensor.matmul(
                        psv[:, :],
                        xT[:, d, 128 * t : 128 * (t + 1)],
                        wv_sb[:, d, :],
                        start=(d == 0),
                        stop=(d == ND - 1),
                    )
                nc.scalar.copy(V[:, t, :], psv[:, :])
            return emit

        # ---- per head-pair Q^T/K^T projection + rope units -------------
        qk_tiles = {}

        def proj_units(hp):
            qt_tile = qkpool.tile([128, S], IDT, tag="qt", name=f"qt{hp}")
            kt_tile = qkpool.tile([128, S], IDT, tag="kt", name=f"kt{hp}")
            qk_tiles[hp] = (qt_tile, kt_tile)
            units = []
            state = {}

            def dma_unit(w_d, wtag):
                def emit():
                    wt = wpool.tile(
                        [128, ND, 128], ILD, tag=wtag, name=f"{wtag}{hp}"
                    )
                    for d in range(ND):
                        nc.gpsimd.dma_start(
                            out=wt[:, d, :],
                            in_=w_d[
                                128 * d : 128 * (d + 1),
                                128 * hp : 128 * (hp + 1),
                            ],
                        )
                    state[wtag] = wt
                return emit

            def mm_unit(wtag, tb):
                def emit():
                    wt = state[wtag]
                    psq = pq.tile([128, 512], F32, tag="pq")
                    for d in range(ND):
                        nc.tensor.matmul(
                            psq[:, :],
                            wt[:, d, :],
                            xT[:, d, 512 * tb : 512 * (tb + 1)],
                            start=(d == 0),
                            stop=(d == ND - 1),
                        )
                    # single fast PSUM read frees the psum ring slot; the
                    # rope math then runs from SBUF bf16 at 2x DVE rate
                    psq_bf = tmp.tile(
                        [128, 512], BF16, tag="psqbf", bufs=2
                    )
                    nc.vector.tensor_copy(psq_bf[:, :], psq[:, :])
                    state[(wtag, tb)] = psq_bf
                return emit

            def rope_unit(wtag, OUT, tb):
                def emit():
                    psq_bf = state.pop((wtag, tb))
                    # rope: out = psq*C + swap32(psq)*S
                    t2 = tmp.tile([128, 512], BF16, tag="t2")
                    cs = slice(512 * tb, 512 * (tb + 1))
                    # ropeS rows are pre-swapped host-side so each mul's two
                    # SBUF inputs share a base partition (walrus requirement)
                    for h2 in range(2):
                        b0 = 64 * h2
                        nc.vector.tensor_mul(
                            t2[b0 : b0 + 32, :],
                            psq_bf[b0 + 32 : b0 + 64, :],
                            ropeS[b0 + 32 : b0 + 64, cs],
                        )
                        nc.vector.tensor_mul(
                            t2[b0 + 32 : b0 + 64, :],
                            psq_bf[b0 : b0 + 32, :],
                            ropeS[b0 : b0 + 32, cs],
                        )
                    t1 = tmp.tile([128, 512], BF16, tag="t1")
                    nc.vector.tensor_mul(t1[:, :], psq_bf[:, :], ropeC[:, cs])
                    nc.vector.tensor_add(OUT[:, cs], t1[:, :], t2[:, :])
                return emit

            for w_d, outi, wtag in ((wq_d, 0, "wq"), (wk_d, 1, "wk")):
                units.append(dma_unit(w_d, wtag))
                for tb in range(NT):
                    units.append(mm_unit(wtag, tb))
                    units.append(
                        rope_unit(
                            wtag,
                            qt_tile if outi == 0 else kt_tile,
                            tb,
                        )
                    )
            return units

        # ---- o_proj units (aT in SBUF, bf16) ---------------------------
        def oproj_unit(tb, et):
            def emit():
                psy = pq.tile([128, 512], F32, tag="pq")
                for dd in range(4):
                    nc.tensor.matmul(
                        psy[:, :],
                        wo_sb[:, dd, 128 * et : 128 * (et + 1)],
                        aT[:, dd, tb, :],
                        start=(dd == 0),
                        stop=(dd == 3),
                    )
                y_t = ypool.tile([128, 512], F32, tag="y")
                nc.vector.tensor_copy(y_t[:, :], psy[:, :])
                nc.sync.dma_start(
                    out=yT_d[
                        128 * et : 128 * (et + 1),
                        512 * tb : 512 * (tb + 1),
                    ],
                    in_=y_t[:, :],
                )
            return emit

        # ---- phase 0: V projection + head-pair-0 projection ------------
        v_units = [v_unit(t) for t in range(NKT)]
        p0_units = proj_units(0)
        merged = list(v_units[:4])
        i = j = 0
        rest_v = v_units[4:]
        while i < len(p0_units) or j < len(rest_v):
            if j < len(rest_v):
                merged.append(rest_v[j])
                j += 1
            if i < len(p0_units):
                merged.append(p0_units[i])
                i += 1
        for u in merged:
            u()

        # ---- attention (o_proj interleaved into last head-pair) --------
        norm_a = deque()   # recip + bf16 convert (DVE), popped early
        norm_b = deque()   # broadcast matmuls + aT mul, popped later
        norm_state = {}

        def make_norm_a(key, pde_):
            def emit():
                # fast approx reciprocal of the softmax denominators
                # (rows 1-31,33+ of pde are unused garbage)
                den_f = tmp.tile([33, 512], F32, tag="denf")
                nc.vector.reciprocal_approx_fast(den_f[:, :], pde_[:, :])
                den = tmp.tile([33, 512], BF16, tag="den")
                nc.vector.tensor_copy(den[:, :], den_f[:, :])
                norm_state[key] = den
            return emit

        def make_norm_b(key, hp_, qb_, po_, pend_):
            def emit():
                den = norm_state.pop(key)
                psb = pq.tile([128, 512], F32, tag="pq")
                nc.tensor.matmul(
                    psb[0:64, :],
                    ones_row[0:1, :],
                    den[0:1, :],
                    start=True,
                    stop=True,
                    tile_position=(0, 0),
                    skip_group_check=True,
                )
                nc.tensor.matmul(
                    psb[64:128, :],
                    ones_row[32:33, :],
                    den[32:33, :],
                    start=True,
                    stop=True,
                    tile_position=(32, 64),
                    skip_group_check=True,
                )
                recbc = tmp.tile([128, 512], F32, tag="recbc")
                nc.vector.tensor_copy(recbc[:, :], psb[:, :])
                nc.vector.tensor_mul(
                    aT[:, hp_, qb_, :], po_[:, :], recbc[:, :]
                )
                if hp_ == NHP - 1:
                    for et in range(ND):
                        pend_.append(oproj_unit(qb_, et))
            return emit

        def emit_scores(hp, qb, kb):
            QT, KT = qk_tiles[hp]
            qslice0 = 512 * qb
            pss = ps.tile([128, 2, 512], F32, tag="ps")
            rr = kb - 4 * qb
            qq0 = 128 * rr if rr >= 0 else 0
            for h2 in range(2):
                b0 = 64 * h2
                nc.tensor.matmul(
                    pss[:, h2, qq0:512],
                    KT[b0 : b0 + 64, 128 * kb : 128 * (kb + 1)],
                    QT[b0 : b0 + 64, qslice0 + qq0 : qslice0 + 512],
                    start=True,
                    stop=True,
                    tile_position=(b0, 0),
                    skip_group_check=True,
                )
            if rr >= 0:
                # accumulate the causal -inf triangle onto the
                # diagonal 128x128 block of both heads on the PE
                for h2 in range(2):
                    nc.tensor.matmul(
                        pss[:, h2, qq0 : qq0 + 128],
                        maskT_sb[:, :],
                        ident_sb[:, :],
                        start=False,
                        stop=True,
                        skip_group_check=True,
                    )
            return pss

        iters = [
            (hp, qb, kb)
            for hp in range(NHP)
            for qb in range(NT)
            for kb in range(4 * qb + 4)
        ]
        pending = deque()
        every = 4
        slot = 0
        po = pde = None
        pss_next = emit_scores(*iters[0])
        for idx, (hp, qb, kb) in enumerate(iters):
            if kb == 0:
                if qb == 0:
                    pending = (
                        deque(proj_units(hp + 1))
                        if hp + 1 < NHP
                        else pending
                    )
                    every = 2 if hp + 1 < NHP else 1
                    slot = 0
                po = pv.tile([128, 512], F32, tag="pv")
                pde = pdb.tile([33, 512], F32, tag="pd")
            nkb = 4 * qb + 4
            pss = pss_next
            if idx + 1 < len(iters):
                pss_next = emit_scores(*iters[idx + 1])
            slot += 1
            if pending and slot % every == 0:
                pending.popleft()()
            r = kb - 4 * qb
            q0 = 128 * r if r >= 0 else 0
            es_t = es.tile([128, 2, 512], BF16, tag="es")
            nc.scalar.activation(
                es_t[:, :, q0:512],
                pss[:, :, q0:512],
                mybir.ActivationFunctionType.Exp,
            )
            # popped after the exp so the den-waiting psb matmuls never
            # sit ahead of the next scores pair in the in-order PE queue
            if norm_b and kb == 3:
                norm_b.popleft()()
            first = kb == 0
            last = kb == nkb - 1
            for h2 in range(2):
                b0 = 64 * h2
                h_global = 2 * hp + h2
                nc.tensor.matmul(
                    po[b0 : b0 + 64, q0:512],
                    V[:, kb, 64 * h_global : 64 * (h_global + 1)],
                    es_t[:, h2, q0:512],
                    start=first,
                    stop=last,
                    tile_position=(0, b0),
                    skip_group_check=True,
                )
            for h2 in range(2):
                nc.tensor.matmul(
                    pde[32 * h2 : 32 * h2 + 1, q0:512],
                    ones_col[:, :],
                    es_t[:, h2, q0:512],
                    start=first,
                    stop=last,
                    tile_position=(0, 32 * h2),
                    skip_group_check=True,
                )
            if last:
                # reciprocal emitted right away (frees the pd slot early);
                # the broadcast+aT-mul half is deferred into the next qb
                make_norm_a((hp, qb), pde)()
                norm_b.append(make_norm_b((hp, qb), hp, qb, po, pending))

        while pending:
            pending.popleft()()
        while norm_b:
            norm_b.popleft()()
        while pending:
            pending.popleft()()

    nc.compile()
    return nc


_PERM = np.concatenate([np.arange(0, DK, 2), np.arange(1, DK, 2)])
_IN_NP = ml_dtypes.bfloat16 if USE_BF16 else np.float32


def _prep_core_inputs(x, token_positions, w_qkv, w_o, core):
    b = core // 2
    h0 = HPC * (core % 2)

    xT = np.ascontiguousarray(x[b].T.astype(_IN_NP))

    w_q = w_qkv[0 * D : 1 * D]
    w_k = w_qkv[1 * D : 2 * D]
    w_v = w_qkv[2 * D : 3 * D]

    def gather(w, permute, scale):
        rows = []
        for j in range(HPC):
            g = h0 + j
            blk = w[DK * g : DK * (g + 1)]
            if permute:
                blk = blk[_PERM]
            rows.append(blk)
        out = np.concatenate(rows, axis=0).astype(np.float32) * scale
        return np.ascontiguousarray(out.T.astype(_IN_NP))  # [D, HPC*DK]

    wq = gather(w_q, True, 1.0 / math.sqrt(DK))
    wk = gather(w_k, True, 1.0)
    wv = gather(w_v, False, 1.0)

    # w_o: [e_out, d_in]; take the d rows of this core's heads -> [512, D]
    rows = []
    for j in range(HPC):
        g = h0 + j
        rows.append(w_o[:, DK * g : DK * (g + 1)].T)
    wo = np.ascontiguousarray(
        np.concatenate(rows, axis=0).astype(ml_dtypes.bfloat16)
    )

    pos = token_positions.astype(np.float32)
    inv = (10000.0 ** (-(np.arange(0, DK, 2, dtype=np.float32)) / DK)).astype(
        np.float32
    )
    ang = pos[:, None] * inv[None, :]  # [S, 32]
    c = np.cos(ang).T.astype(np.float32)  # [32, S]
    s = np.sin(ang).T.astype(np.float32)
    C64 = np.concatenate([c, c], axis=0)
    # rows pre-swapped: row block [0:32] holds +s (multiplies x1 into the
    # x2 output slot), [32:64] holds -s (multiplies x2 into the x1 slot)
    S64 = np.concatenate([s, -s], axis=0)
    ropeC = np.ascontiguousarray(
        np.concatenate([C64, C64], axis=0).astype(ml_dtypes.bfloat16)
    )
    ropeS = np.ascontiguousarray(
        np.concatenate([S64, S64], axis=0).astype(ml_dtypes.bfloat16)
    )

    ki = np.arange(128)[:, None]
    qi = np.arange(128)[None, :]
    mask = np.where(ki <= qi, 0.0, NEG).astype(np.float32)
    maskT = np.ascontiguousarray(mask.T.astype(ml_dtypes.bfloat16))
    ident = np.eye(128, dtype=np.float32).astype(ml_dtypes.bfloat16)

    return {
        "xT": xT,
        "wq": wq,
        "wk": wk,
        "wv": wv,
        "wo": wo,
        "ropeC": ropeC,
        "ropeS": ropeS,
        "maskT": maskT,
        "ident": ident,
    }


def kernel(x, token_positions, w_qkv, w_o):
    x = np.asarray(x, dtype=np.float32)
    token_positions = np.asarray(token_positions)
    w_qkv = np.asarray(w_qkv, dtype=np.float32)
    w_o = np.asarray(w_o, dtype=np.float32)

    if "nc" not in _CACHE:
        _CACHE["nc"] = _build()
    nc = _CACHE["nc"]

    in_maps = [
        _prep_core_inputs(x, token_positions, w_qkv, w_o, c)
        for c in range(NCORES)
    ]
    res = run_bass_kernel_spmd(nc, in_maps, core_ids=list(range(NCORES)))
    _CACHE["last_results"] = res

    out = np.empty((B, S, D), dtype=np.float32)
    for b in range(B):
        yT = res.results[2 * b]["yT"] + res.results[2 * b + 1]["yT"]
        out[b] = yT.T
    return out


# revision 35
# speedup vs baseline: 1.9106x; 1.0031x over previous
"""Multi-head self-attention (RoPE, causal) on 8 trn2 NeuronCores.

Sharding: batch (4) x head-group (2x8 heads) = 8 shards, one per core.
Each core: QKV projection for its 8 heads -> RoPE -> causal flash
attention (scores kept transposed [k, q] so PV needs no transposes;
softmax denominators accumulated on the PE via ones-column matmuls) ->
partial o_proj over its 512 head-dims. Host sums the two partial
o_proj outputs of each batch pair (the tensor-parallel all-reduce) and
concatenates batches.

v3: all matmuls bf16 (f32r streams at 1.5 cyc/row on HW); po/pde pairs
emitted pair-wise so the PE column-tiles run concurrently; softmax
normalization emission deferred two iterations so the in-order PE queue
never waits on the DVE reciprocal; startup DMAs spread across idle
engine queues; aT kept in SBUF; o_proj interleaved into the last
head-pair's attention loop.
"""
import sys
import math

sys.path.insert(0, "/opt/trn_rl_repo")

import numpy as np
import ml_dtypes
from contextlib import ExitStack
from collections import deque

import concourse.bacc as bacc
import concourse.tile as tile
from concourse import mybir
from concourse.bass_utils import run_bass_kernel_spmd

B, S, D, H, DK = 4, 2048, 1024, 16, 64
NCORES = 8
ND = D // 128          # 8 d-tiles of the model dim
NT = S // 512          # 4 token super-blocks
NKT = S // 128         # 16 key/token 128-blocks
HPC = H // 2           # heads per core = 8
NHP = HPC // 2         # head-pairs per core = 4
F32 = mybir.dt.float32
F32R = mybir.dt.float32r
BF16 = mybir.dt.bfloat16
NEG = -30000.0

USE_BF16 = True        # bf16 x/w/q/k (1 cyc/row on PE) vs f32r (1.5)

_CACHE = {}


def _build():
    nc = bacc.Bacc("TRN2", target_bir_lowering=False, num_devices=NCORES)

    IDT = BF16 if USE_BF16 else F32
    ILD = BF16 if USE_BF16 else F32R

    xT_d = nc.dram_tensor("xT", [D, S], IDT, kind="ExternalInput")
    wq_d = nc.dram_tensor("wq", [D, HPC * DK], IDT, kind="ExternalInput")
    wk_d = nc.dram_tensor("wk", [D, HPC * DK], IDT, kind="ExternalInput")
    wv_d = nc.dram_tensor("wv", [D, HPC * DK], IDT, kind="ExternalInput")
    wo_d = nc.dram_tensor("wo", [HPC * DK, D], BF16, kind="ExternalInput")
    ropeC_d = nc.dram_tensor("ropeC", [128, S], BF16, kind="ExternalInput")
    ropeS_d = nc.dram_tensor("ropeS", [128, S], BF16, kind="ExternalInput")
    maskT_d = nc.dram_tensor("maskT", [128, 128], BF16, kind="ExternalInput")
    ident_d = nc.dram_tensor("ident", [128, 128], BF16, kind="ExternalInput")
    yT_d = nc.dram_tensor("yT", [D, S], F32, kind="ExternalOutput")

    with ExitStack() as ctx:
        tc = ctx.enter_context(tile.TileContext(nc))

        const = ctx.enter_context(tc.tile_pool(name="const", bufs=1))
        ps = ctx.enter_context(tc.tile_pool(name="ps", bufs=2, space="PSUM"))
        pv = ctx.enter_context(tc.tile_pool(name="pv", bufs=2, space="PSUM"))
        pdb = ctx.enter_context(tc.tile_pool(name="pdb", bufs=1, space="PSUM"))
        pq = ctx.enter_context(tc.tile_pool(name="pq", bufs=1, space="PSUM"))
        xpool = ctx.enter_context(tc.tile_pool(name="x", bufs=1))
        vpool = ctx.enter_context(tc.tile_pool(name="v", bufs=1))
        wvpool = ctx.enter_context(tc.tile_pool(name="wv", bufs=1))
        qkpool = ctx.enter_context(tc.tile_pool(name="qk", bufs=2))
        wpool = ctx.enter_context(tc.tile_pool(name="w", bufs=2))
        atpool = ctx.enter_context(tc.tile_pool(name="at", bufs=1))
        wopool = ctx.enter_context(tc.tile_pool(name="wo", bufs=1))
        es = ctx.enter_context(tc.tile_pool(name="es", bufs=3))
        tmp = ctx.enter_context(tc.tile_pool(name="tmp", bufs=1))
        ypool = ctx.enter_context(tc.tile_pool(name="y", bufs=2))

        # ---- high-priority input DMAs, spread across idle engine queues ----
        wv_sb = wvpool.tile([128, ND, HPC * DK], ILD)
        xT = xpool.tile([128, ND, S], ILD)
        for d in range(ND):
            nc.sync.dma_start(
                out=wv_sb[:, d, :],
                in_=wv_d[128 * d : 128 * (d + 1), :],
            )
            nc.gpsimd.dma_start(
                out=xT[:, d, 0:512],
                in_=xT_d[128 * d : 128 * (d + 1), 0:512],
            )
        ropeC = const.tile([128, S], BF16)
        nc.scalar.dma_start(out=ropeC, in_=ropeC_d[:, :])
        ropeS = const.tile([128, S], BF16)
        nc.scalar.dma_start(out=ropeS, in_=ropeS_d[:, :])
        for tb in range(1, NT):
            for d in range(ND):
                eng = nc.sync if (d % 2 == 0) else nc.gpsimd
                eng.dma_start(
                    out=xT[:, d, 512 * tb : 512 * (tb + 1)],
                    in_=xT_d[
                        128 * d : 128 * (d + 1), 512 * tb : 512 * (tb + 1)
                    ],
                )
        maskT_sb = const.tile([128, 128], BF16)
        nc.scalar.dma_start(out=maskT_sb[:, :], in_=maskT_d[:, :])
        ident_sb = const.tile([128, 128], BF16)
        nc.scalar.dma_start(out=ident_sb[:, :], in_=ident_d[:, :])
        wo_sb = wopool.tile([128, 4, D], BF16)
        for dd in range(4):
            nc.sync.dma_start(
                out=wo_sb[:, dd, :],
                in_=wo_d[128 * dd : 128 * (dd + 1), :],
            )

        # ---- constants -------------------------------------------------
        ones_f = const.tile([128, 1], F32)
        nc.vector.memset(ones_f, 1.0)
        ones_col = const.tile([128, 1], BF16)
        nc.vector.tensor_copy(ones_col, ones_f)
        ones_row_f = const.tile([33, 64], F32)
        nc.vector.memset(ones_row_f, 1.0)
        ones_row = const.tile([33, 64], BF16)
        nc.vector.tensor_copy(ones_row, ones_row_f)
        # warm the ACT exp table set before any copies run on it
        warm = const.tile([128, 8], F32)
        nc.vector.memset(warm, 0.0)
        warm_out = const.tile([128, 8], BF16)
        nc.scalar.activation(
            warm_out, warm, mybir.ActivationFunctionType.Exp
        )
        # keep the PE busy through the startup DMA wait so the HAM clock
        # gate is at full rate (K=8/8) when the real matmuls arrive
        ones128 = const.tile([128, 64], BF16)
        nc.vector.memset(ones128, 1.0)
        pwarm = pq.tile([128, 512], F32, tag="pq", name="pwarm")
        for _ in range(64):
            nc.tensor.matmul(
                pwarm[0:64, 0:64],
                ones128[:, :],
                ones128[:, :],
                start=True,
                stop=True,
                skip_group_check=True,
            )

        V = vpool.tile([128, NKT, HPC * DK], BF16)
        aT = atpool.tile([128, NHP, NT, 512], BF16)

        # ---- V projection units (PSUM->SBUF cast on scalar engine) -----
        def v_unit(t):
            def emit():
                psv_t = ps.tile([128, 1024], F32, tag="ps")
                psv = psv_t[:, 0:512]
                for d in range(ND):
                    nc.tensor.matmul(
                        psv[:, :],
                        xT[:, d, 128 * t : 128 * (t + 1)],
                        wv_sb[:, d, :],
                        start=(d == 0),
                        stop=(d == ND - 1),
                    )
                nc.scalar.copy(V[:, t, :], psv[:, :])
            return emit

        # ---- per head-pair Q^T/K^T projection + rope units -------------
        qk_tiles = {}

        def proj_units(hp):
            qt_tile = qkpool.tile([128, S], IDT, tag="qt", name=f"qt{hp}")
            kt_tile = qkpool.tile([128, S], IDT, tag="kt", name=f"kt{hp}")
            qk_tiles[hp] = (qt_tile, kt_tile)
            units = []
            state = {}

            def dma_unit(w_d, wtag):
                def emit():
                    wt = wpool.tile(
                        [128, ND, 128], ILD, tag=wtag, name=f"{wtag}{hp}"
                    )
                    for d in range(ND):
                        nc.gpsimd.dma_start(
                            out=wt[:, d, :],
                            in_=w_d[
                                128 * d : 128 * (d + 1),
                                128 * hp : 128 * (hp + 1),
                            ],
                        )
                    state[wtag] = wt
                return emit

            def mm_unit(wtag, tb):
                def emit():
                    wt = state[wtag]
                    psq = pq.tile([128, 512], F32, tag="pq")
                    for d in range(ND):
                        nc.tensor.matmul(
                            psq[:, :],
                            wt[:, d, :],
                            xT[:, d, 512 * tb : 512 * (tb + 1)],
                            start=(d == 0),
                            stop=(d == ND - 1),
                        )
                    # single fast PSUM read frees the psum ring slot; the
                    # rope math then runs from SBUF bf16 at 2x DVE rate
                    psq_bf = tmp.tile(
                        [128, 512], BF16, tag="psqbf", bufs=2
                    )
                    nc.vector.tensor_copy(psq_bf[:, :], psq[:, :])
                    state[(wtag, tb)] = psq_bf
                return emit

            def rope_unit(wtag, OUT, tb):
                def emit():
                    psq_bf = state.pop((wtag, tb))
                    # rope: out = psq*C + swap32(psq)*S
                    t2 = tmp.tile([128, 512], BF16, tag="t2")
                    cs = slice(512 * tb, 512 * (tb + 1))
                    # ropeS rows are pre-swapped host-side so each mul's two
                    # SBUF inputs share a base partition (walrus requirement)
                    for h2 in range(2):
                        b0 = 64 * h2
                        nc.vector.tensor_mul(
                            t2[b0 : b0 + 32, :],
                            psq_bf[b0 + 32 : b0 + 64, :],
                            ropeS[b0 + 32 : b0 + 64, cs],
                        )
                        nc.vector.tensor_mul(
                            t2[b0 + 32 : b0 + 64, :],
                            psq_bf[b0 : b0 + 32, :],
                            ropeS[b0 : b0 + 32, cs],
                        )
                    t1 = tmp.tile([128, 512], BF16, tag="t1")
                    nc.vector.tensor_mul(t1[:, :], psq_bf[:, :], ropeC[:, cs])
                    nc.vector.tensor_add(OUT[:, cs], t1[:, :], t2[:, :])
                return emit

            for w_d, outi, wtag in ((wq_d, 0, "wq"), (wk_d, 1, "wk")):
                units.append(dma_unit(w_d, wtag))
                for tb in range(NT):
                    units.append(mm_unit(wtag, tb))
                    units.append(
                        rope_unit(
                            wtag,
                            qt_tile if outi == 0 else kt_tile,
                            tb,
                        )
                    )
            return units

        # ---- o_proj units (aT in SBUF, bf16) ---------------------------
        def oproj_unit(tb, et):
            def emit():
                psy = pq.tile([128, 512], F32, tag="pq")
                for dd in range(4):
                    nc.tensor.matmul(
                        psy[:, :],
                        wo_sb[:, dd, 128 * et : 128 * (et + 1)],
                        aT[:, dd, tb, :],
                        start=(dd == 0),
                        stop=(dd == 3),
                    )
                y_t = ypool.tile([128, 512], F32, tag="y")
                nc.vector.tensor_copy(y_t[:, :], psy[:, :])
                nc.sync.dma_start(
                    out=yT_d[
                        128 * et : 128 * (et + 1),
                        512 * tb : 512 * (tb + 1),
                    ],
                    in_=y_t[:, :],
                )
            return emit

        # ---- phase 0: V projection + head-pair-0 projection ------------
        v_units = [v_unit(t) for t in range(NKT)]
        p0_units = proj_units(0)
        merged = list(v_units[:4])
        i = j = 0
        rest_v = v_units[4:]
        while i < len(p0_units) or j < len(rest_v):
            if j < len(rest_v):
                merged.append(rest_v[j])
                j += 1
            if i < len(p0_units):
                merged.append(p0_units[i])
                i += 1
        for u in merged:
            u()

        # ---- attention (o_proj interleaved into last head-pair) --------
        norm_a = deque()   # recip + bf16 convert (DVE), popped early
        norm_b = deque()   # broadcast matmuls + aT mul, popped later
        norm_state = {}

        def make_norm_a(key, pde_):
            def emit():
                # fast approx reciprocal of the softmax denominators
                # (rows 1-31,33+ of pde are unused garbage)
                den_f = tmp.tile([33, 512], F32, tag="denf")
                nc.vector.reciprocal_approx_fast(den_f[:, :], pde_[:, :])
                den = tmp.tile([33, 512], BF16, tag="den")
                nc.vector.tensor_copy(den[:, :], den_f[:, :])
                norm_state[key] = den
            return emit

        def make_norm_b(key, hp_, qb_, po_, pend_):
            def emit():
                den = norm_state.pop(key)
                psb = pq.tile([128, 512], F32, tag="pq")
                nc.tensor.matmul(
                    psb[0:64, :],
                    ones_row[0:1, :],
                    den[0:1, :],
                    start=True,
                    stop=True,
                    tile_position=(0, 0),
                    skip_group_check=True,
                )
                nc.tensor.matmul(
                    psb[64:128, :],
                    ones_row[32:33, :],
                    den[32:33, :],
                    start=True,
                    stop=True,
                    tile_position=(32, 64),
                    skip_group_check=True,
                )
                recbc = tmp.tile([128, 512], F32, tag="recbc")
                nc.vector.tensor_copy(recbc[:, :], psb[:, :])
                nc.vector.tensor_mul(
                    aT[:, hp_, qb_, :], po_[:, :], recbc[:, :]
                )
                if hp_ == NHP - 1:
                    for et in range(ND):
                        pend_.append(oproj_unit(qb_, et))
            return emit

        def emit_scores(hp, qb, kb):
            QT, KT = qk_tiles[hp]
            qslice0 = 512 * qb
            pss = ps.tile([128, 2, 512], F32, tag="ps")
            rr = kb - 4 * qb
            qq0 = 128 * rr if rr >= 0 else 0
            for h2 in range(2):
                b0 = 64 * h2
                nc.tensor.matmul(
                    pss[:, h2, qq0:512],
                    KT[b0 : b0 + 64, 128 * kb : 128 * (kb + 1)],
                    QT[b0 : b0 + 64, qslice0 + qq0 : qslice0 + 512],
                    start=True,
                    stop=True,
                    tile_position=(b0, 0),
                    skip_group_check=True,
                )
            if rr >= 0:
                # accumulate the causal -inf triangle onto the
                # diagonal 128x128 block of both heads on the PE
                for h2 in range(2):
                    nc.tensor.matmul(
                        pss[:, h2, qq0 : qq0 + 128],
                        maskT_sb[:, :],
                        ident_sb[:, :],
                        start=False,
                        stop=True,
                        skip_group_check=True,
                    )
            return pss

        iters = [
            (hp, qb, kb)
            for hp in range(NHP)
            for qb in range(NT)
            for kb in range(4 * qb + 4)
        ]
        pending = deque()
        every = 4
        slot = 0
        po = pde = None
        pss_next = emit_scores(*iters[0])
        for idx, (hp, qb, kb) in enumerate(iters):
            if kb == 0:
                if qb == 0:
                    pending = (
                        deque(proj_units(hp + 1))
                        if hp + 1 < NHP
                        else pending
                    )
                    every = 2 if hp + 1 < NHP else 1
                    slot = 0
                po = pv.tile([128, 512], F32, tag="pv")
                pde = pdb.tile([33, 512], F32, tag="pd")
            nkb = 4 * qb + 4
            pss = pss_next
            if idx + 1 < len(iters):
                pss_next = emit_scores(*iters[idx + 1])
            slot += 1
            if pending and slot % every == 0:
                pending.popleft()()
            r = kb - 4 * qb
            q0 = 128 * r if r >= 0 else 0
            es_t = es.tile([128, 2, 512], BF16, tag="es")
            nc.scalar.activation(
                es_t[:, :, q0:512],
                pss[:, :, q0:512],
                mybir.ActivationFunctionType.Exp,
            )
            # popped after the exp so the den-waiting psb matmuls never
            # sit ahead of the next scores pair in the in-order PE queue
            if norm_b and kb == 3:
                norm_b.popleft()()
            first = kb == 0
            last = kb == nkb - 1
            for h2 in range(2):
                b0 = 64 * h2
                h_global = 2 * hp + h2
                nc.tensor.matmul(
                    po[b0 : b0 + 64, q0:512],
                    V[:, kb, 64 * h_global : 64 * (h_global + 1)],
                    es_t[:, h2, q0:512],
                    start=first,
                    stop=last,
                    tile_position=(0, b0),
                    skip_group_check=True,
                )
            for h2 in range(2):
                nc.tensor.matmul(
                    pde[32 * h2 : 32 * h2 + 1, q0:512],
                    ones_col[:, :],
                    es_t[:, h2, q0:512],
                    start=first,
                    stop=last,
                    tile_position=(0, 32 * h2),
                    skip_group_check=True,
                )
            if last:
                # reciprocal emitted right away (frees the pd slot early);
                # the broadcast+aT-mul half is deferred into the next qb
                make_norm_a((hp, qb), pde)()
                norm_b.append(make_norm_b((hp, qb), hp, qb, po, pending))

        while pending:
            pending.popleft()()
        while norm_b:
            norm_b.popleft()()
        while pending:
            pending.popleft()()

    nc.compile()
    return nc


_PERM = np.concatenate([np.arange(0, DK, 2), np.arange(1, DK, 2)])
_IN_NP = ml_dtypes.bfloat16 if USE_BF16 else np.float32


def _prep_core_inputs(x, token_positions, w_qkv, w_o, core):
    b = core // 2
    h0 = HPC * (core % 2)

    xT = np.ascontiguousarray(x[b].T.astype(_IN_NP))

    w_q = w_qkv[0 * D : 1 * D]
    w_k = w_qkv[1 * D : 2 * D]
    w_v = w_qkv[2 * D : 3 * D]

    def gather(w, permute, scale):
        rows = []
        for j in range(HPC):
            g = h0 + j
            blk = w[DK * g : DK * (g + 1)]
            if permute:
                blk = blk[_PERM]
            rows.append(blk)
        out = np.concatenate(rows, axis=0).astype(np.float32) * scale
        return np.ascontiguousarray(out.T.astype(_IN_NP))  # [D, HPC*DK]

    wq = gather(w_q, True, 1.0 / math.sqrt(DK))
    wk = gather(w_k, True, 1.0)
    wv = gather(w_v, False, 1.0)

    # w_o: [e_out, d_in]; take the d rows of this core's heads -> [512, D]
    rows = []
    for j in range(HPC):
        g = h0 + j
        rows.append(w_o[:, DK * g : DK * (g + 1)].T)
    wo = np.ascontiguousarray(
        np.concatenate(rows, axis=0).astype(ml_dtypes.bfloat16)
    )

    pos = token_positions.astype(np.float32)
    inv = (10000.0 ** (-(np.arange(0, DK, 2, dtype=np.float32)) / DK)).astype(
        np.float32
    )
    ang = pos[:, None] * inv[None, :]  # [S, 32]
    c = np.cos(ang).T.astype(np.float32)  # [32, S]
    s = np.sin(ang).T.astype(np.float32)
    C64 = np.concatenate([c, c], axis=0)
    # rows pre-swapped: row block [0:32] holds +s (multiplies x1 into the
    # x2 output slot), [32:64] holds -s (multiplies x2 into the x1 slot)
    S64 = np.concatenate([s, -s], axis=0)
    ropeC = np.ascontiguousarray(
        np.concatenate([C64, C64], axis=0).astype(ml_dtypes.bfloat16)
    )
    ropeS = np.ascontiguousarray(
        np.concatenate([S64, S64], axis=0).astype(ml_dtypes.bfloat16)
    )

    ki = np.arange(128)[:, None]
    qi = np.arange(128)[None, :]
    mask = np.where(ki <= qi, 0.0, NEG).astype(np.float32)
    maskT = np.ascontiguousarray(mask.T.astype(ml_dtypes.bfloat16))
    ident = np.eye(128, dtype=np.float32).astype(ml_dtypes.bfloat16)

    return {
        "xT": xT,
        "wq": wq,
        "wk": wk,
        "wv": wv,
        "wo": wo,
        "ropeC": ropeC,
        "ropeS": ropeS,
        "maskT": maskT,
        "ident": ident,
    }


def kernel(x, token_positions, w_qkv, w_o):
    x = np.asarray(x, dtype=np.float32)
    token_positions = np.asarray(token_positions)
    w_qkv = np.asarray(w_qkv, dtype=np.float32)
    w_o = np.asarray(w_o, dtype=np.float32)

    if "nc" not in _CACHE:
        _CACHE["nc"] = _build()
    nc = _CACHE["nc"]

    in_maps = [
        _prep_core_inputs(x, token_positions, w_qkv, w_o, c)
        for c in range(NCORES)
    ]
    res = run_bass_kernel_spmd(nc, in_maps, core_ids=list(range(NCORES)))
    _CACHE["last_results"] = res

    out = np.empty((B, S, D), dtype=np.float32)
    for b in range(B):
        yT = res.results[2 * b]["yT"] + res.results[2 * b + 1]["yT"]
        out[b] = yT.T
    return out


# revision 49
# speedup vs baseline: 1.9142x; 1.0019x over previous
"""Multi-head self-attention (RoPE, causal) on 8 trn2 NeuronCores.

Sharding: batch (4) x head-group (2x8 heads) = 8 shards, one per core.
Each core: QKV projection for its 8 heads -> RoPE -> causal flash
attention (scores kept transposed [k, q] so PV needs no transposes;
softmax denominators accumulated on the PE via ones-column matmuls) ->
partial o_proj over its 512 head-dims. Host sums the two partial
o_proj outputs of each batch pair (the tensor-parallel all-reduce) and
concatenates batches.

v3: all matmuls bf16 (f32r streams at 1.5 cyc/row on HW); po/pde pairs
emitted pair-wise so the PE column-tiles run concurrently; softmax
normalization emission deferred two iterations so the in-order PE queue
never waits on the DVE reciprocal; startup DMAs spread across idle
engine queues; aT kept in SBUF; o_proj interleaved into the last
head-pair's attention loop.
"""
import sys
import math

sys.path.insert(0, "/opt/trn_rl_repo")

import numpy as np
import ml_dtypes
from contextlib import ExitStack
from collections import deque

import concourse.bacc as bacc
import concourse.tile as tile
from concourse import mybir
from concourse.bass_utils import run_bass_kernel_spmd

B, S, D, H, DK = 4, 2048, 1024, 16, 64
NCORES = 8
ND = D // 128          # 8 d-tiles of the model dim
NT = S // 512          # 4 token super-blocks
NKT = S // 128         # 16 key/token 128-blocks
HPC = H // 2           # heads per core = 8
NHP = HPC // 2         # head-pairs per core = 4
F32 = mybir.dt.float32
F32R = mybir.dt.float32r
BF16 = mybir.dt.bfloat16
NEG = -30000.0

USE_BF16 = True        # bf16 x/w/q/k (1 cyc/row on PE) vs f32r (1.5)

_CACHE = {}


def _build():
    nc = bacc.Bacc("TRN2", target_bir_lowering=False, num_devices=NCORES)

    IDT = BF16 if USE_BF16 else F32
    ILD = BF16 if USE_BF16 else F32R

    xT_d = nc.dram_tensor("xT", [D, S], IDT, kind="ExternalInput")
    wq_d = nc.dram_tensor("wq", [D, HPC * DK], IDT, kind="ExternalInput")
    wk_d = nc.dram_tensor("wk", [D, HPC * DK], IDT, kind="ExternalInput")
    wv_d = nc.dram_tensor("wv", [D, HPC * DK], IDT, kind="ExternalInput")
    wo_d = nc.dram_tensor("wo", [HPC * DK, D], BF16, kind="ExternalInput")
    ropeC_d = nc.dram_tensor("ropeC", [128, S], BF16, kind="ExternalInput")
    ropeS_d = nc.dram_tensor("ropeS", [128, S], BF16, kind="ExternalInput")
    maskT_d = nc.dram_tensor("maskT", [128, 128], BF16, kind="ExternalInput")
    ident_d = nc.dram_tensor("ident", [128, 128], BF16, kind="ExternalInput")
    yT_d = nc.dram_tensor("yT", [D, S], F32, kind="ExternalOutput")

    with ExitStack() as ctx:
        tc = ctx.enter_context(tile.TileContext(nc))

        const = ctx.enter_context(tc.tile_pool(name="const", bufs=1))
        ps = ctx.enter_context(tc.tile_pool(name="ps", bufs=2, space="PSUM"))
        pv = ctx.enter_context(tc.tile_pool(name="pv", bufs=2, space="PSUM"))
        pdb = ctx.enter_context(tc.tile_pool(name="pdb", bufs=1, space="PSUM"))
        pq = ctx.enter_context(tc.tile_pool(name="pq", bufs=1, space="PSUM"))
        xpool = ctx.enter_context(tc.tile_pool(name="x", bufs=1))
        vpool = ctx.enter_context(tc.tile_pool(name="v", bufs=1))
        wvpool = ctx.enter_context(tc.tile_pool(name="wv", bufs=1))
        qkpool = ctx.enter_context(tc.tile_pool(name="qk", bufs=2))
        wpool = ctx.enter_context(tc.tile_pool(name="w", bufs=2))
        atpool = ctx.enter_context(tc.tile_pool(name="at", bufs=1))
        wopool = ctx.enter_context(tc.tile_pool(name="wo", bufs=1))
        es = ctx.enter_context(tc.tile_pool(name="es", bufs=4))
        tmp = ctx.enter_context(tc.tile_pool(name="tmp", bufs=1))
        ypool = ctx.enter_context(tc.tile_pool(name="y", bufs=2))

        # ---- high-priority input DMAs, spread across idle engine queues ----
        wv_sb = wvpool.tile([128, ND, HPC * DK], ILD)
        xT = xpool.tile([128, ND, S], ILD)
        for d in range(ND):
            nc.sync.dma_start(
                out=wv_sb[:, d, :],
                in_=wv_d[128 * d : 128 * (d + 1), :],
            )
            nc.gpsimd.dma_start(
                out=xT[:, d, 0:512],
                in_=xT_d[128 * d : 128 * (d + 1), 0:512],
            )
        ropeC = const.tile([128, S], BF16)
        nc.scalar.dma_start(out=ropeC, in_=ropeC_d[:, :])
        ropeS = const.tile([128, S], BF16)
        nc.scalar.dma_start(out=ropeS, in_=ropeS_d[:, :])
        for tb in range(1, NT):
            for d in range(ND):
                eng = nc.sync if (d % 2 == 0) else nc.gpsimd
                eng.dma_start(
                    out=xT[:, d, 512 * tb : 512 * (tb + 1)],
                    in_=xT_d[
                        128 * d : 128 * (d + 1), 512 * tb : 512 * (tb + 1)
                    ],
                )
        maskT_sb = const.tile([128, 128], BF16)
        nc.scalar.dma_start(out=maskT_sb[:, :], in_=maskT_d[:, :])
        ident_sb = const.tile([128, 128], BF16)
        nc.scalar.dma_start(out=ident_sb[:, :], in_=ident_d[:, :])
        wo_sb = wopool.tile([128, 4, D], BF16)
        for dd in range(4):
            nc.sync.dma_start(
                out=wo_sb[:, dd, :],
                in_=wo_d[128 * dd : 128 * (dd + 1), :],
            )

        # ---- constants -------------------------------------------------
        ones_f = const.tile([128, 1], F32)
        nc.vector.memset(ones_f, 1.0)
        ones_col = const.tile([128, 1], BF16)
        nc.vector.tensor_copy(ones_col, ones_f)
        ones_row_f = const.tile([33, 64], F32)
        nc.vector.memset(ones_row_f, 1.0)
        ones_row = const.tile([33, 64], BF16)
        nc.vector.tensor_copy(ones_row, ones_row_f)
        # warm the ACT exp table set before any copies run on it
        warm = const.tile([128, 8], F32)
        nc.vector.memset(warm, 0.0)
        warm_out = const.tile([128, 8], BF16)
        nc.scalar.activation(
            warm_out, warm, mybir.ActivationFunctionType.Exp
        )
        # keep the PE busy through the startup DMA wait so the HAM clock
        # gate is at full rate (K=8/8) when the real matmuls arrive
        ones128 = const.tile([128, 64], BF16)
        nc.vector.memset(ones128, 1.0)
        pwarm = pq.tile([128, 512], F32, tag="pq", name="pwarm")
        for _ in range(64):
            nc.tensor.matmul(
                pwarm[0:64, 0:64],
                ones128[:, :],
                ones128[:, :],
                start=True,
                stop=True,
                skip_group_check=True,
            )

        V = vpool.tile([128, NKT, HPC * DK], BF16)
        aT = atpool.tile([128, NHP, NT, 512], BF16)

        # ---- V projection units (PSUM->SBUF cast on scalar engine) -----
        def v_unit(t):
            def emit():
                psv_t = ps.tile([128, 1024], F32, tag="ps")
                psv = psv_t[:, 0:512]
                for d in range(ND):
                    nc.tensor.matmul(
                        psv[:, :],
                        xT[:, d, 128 * t : 128 * (t + 1)],
                        wv_sb[:, d, :],
                        start=(d == 0),
                        stop=(d == ND - 1),
                    )
                nc.scalar.copy(V[:, t, :], psv[:, :])
            return emit

        # ---- per head-pair Q^T/K^T projection + rope units -------------
        qk_tiles = {}

        def proj_units(hp):
            qt_tile = qkpool.tile([128, S], IDT, tag="qt", name=f"qt{hp}")
            kt_tile = qkpool.tile([128, S], IDT, tag="kt", name=f"kt{hp}")
            qk_tiles[hp] = (qt_tile, kt_tile)
            units = []
            state = {}

            def dma_unit(w_d, wtag):
                def emit():
                    wt = wpool.tile(
                        [128, ND, 128], ILD, tag=wtag, name=f"{wtag}{hp}"
                    )
                    for d in range(ND):
                        nc.gpsimd.dma_start(
                            out=wt[:, d, :],
                            in_=w_d[
                                128 * d : 128 * (d + 1),
                                128 * hp : 128 * (hp + 1),
                            ],
                        )
                    state[wtag] = wt
                return emit

            def mm_unit(wtag, tb):
                def emit():
                    wt = state[wtag]
                    psq = pq.tile([128, 512], F32, tag="pq")
                    for d in range(ND):
                        nc.tensor.matmul(
                            psq[:, :],
                            wt[:, d, :],
                            xT[:, d, 512 * tb : 512 * (tb + 1)],
                            start=(d == 0),
                            stop=(d == ND - 1),
                        )
                    # single fast PSUM read frees the psum ring slot; the
                    # rope math then runs from SBUF bf16 at 2x DVE rate
                    psq_bf = tmp.tile(
                        [128, 512], BF16, tag="psqbf", bufs=2
                    )
                    nc.vector.tensor_copy(psq_bf[:, :], psq[:, :])
                    state[(wtag, tb)] = psq_bf
                return emit

            def rope_unit(wtag, OUT, tb):
                def emit():
                    psq_bf = state.pop((wtag, tb))
                    # rope: out = psq*C + swap32(psq)*S
                    t2 = tmp.tile([128, 512], BF16, tag="t2")
                    cs = slice(512 * tb, 512 * (tb + 1))
                    # ropeS rows are pre-swapped host-side so each mul's two
                    # SBUF inputs share a base partition (walrus requirement)
                    for h2 in range(2):
                        b0 = 64 * h2
                        nc.vector.tensor_mul(
                            t2[b0 : b0 + 32, :],
                            psq_bf[b0 + 32 : b0 + 64, :],
                            ropeS[b0 + 32 : b0 + 64, cs],
                        )
                        nc.vector.tensor_mul(
                            t2[b0 + 32 : b0 + 64, :],
                            psq_bf[b0 : b0 + 32, :],
                            ropeS[b0 : b0 + 32, cs],
                        )
                    t1 = tmp.tile([128, 512], BF16, tag="t1")
                    nc.vector.tensor_mul(t1[:, :], psq_bf[:, :], ropeC[:, cs])
                    nc.vector.tensor_add(OUT[:, cs], t1[:, :], t2[:, :])
                return emit

            for w_d, outi, wtag in ((wq_d, 0, "wq"), (wk_d, 1, "wk")):
                units.append(dma_unit(w_d, wtag))
                for tb in range(NT):
                    units.append(mm_unit(wtag, tb))
                    units.append(
                        rope_unit(
                            wtag,
                            qt_tile if outi == 0 else kt_tile,
                            tb,
                        )
                    )
            return units

        # ---- o_proj units (aT in SBUF, bf16) ---------------------------
        def oproj_unit(tb, et):
            def emit():
                psy = pq.tile([128, 512], F32, tag="pq")
                for dd in range(4):
                    nc.tensor.matmul(
                        psy[:, :],
                        wo_sb[:, dd, 128 * et : 128 * (et + 1)],
                        aT[:, dd, tb, :],
                        start=(dd == 0),
                        stop=(dd == 3),
                    )
                y_t = ypool.tile([128, 512], F32, tag="y")
                nc.vector.tensor_copy(y_t[:, :], psy[:, :])
                nc.sync.dma_start(
                    out=yT_d[
                        128 * et : 128 * (et + 1),
                        512 * tb : 512 * (tb + 1),
                    ],
                    in_=y_t[:, :],
                )
            return emit

        # ---- phase 0: V projection + head-pair-0 projection ------------
        v_units = [v_unit(t) for t in range(NKT)]
        p0_units = proj_units(0)
        merged = list(v_units[:4])
        i = j = 0
        rest_v = v_units[4:]
        while i < len(p0_units) or j < len(rest_v):
            if j < len(rest_v):
                merged.append(rest_v[j])
                j += 1
            if i < len(p0_units):
                merged.append(p0_units[i])
                i += 1
        for u in merged:
            u()

        # ---- attention (o_proj interleaved into last head-pair) --------
        norm_a = deque()   # recip + bf16 convert (DVE), popped early
        norm_b = deque()   # broadcast matmuls + aT mul, popped later
        norm_state = {}

        def make_norm_a(key, pde_):
            def emit():
                # fast approx reciprocal of the softmax denominators
                # (rows 1-31,33+ of pde are unused garbage)
                den_f = tmp.tile([33, 512], F32, tag="denf")
                nc.vector.reciprocal_approx_fast(den_f[:, :], pde_[:, :])
                den = tmp.tile([33, 512], BF16, tag="den")
                nc.vector.tensor_copy(den[:, :], den_f[:, :])
                norm_state[key] = den
            return emit

        def make_norm_b(key, hp_, qb_, po_, pend_):
            def emit():
                den = norm_state.pop(key)
                psb = pq.tile([128, 512], F32, tag="pq")
                nc.tensor.matmul(
                    psb[0:64, :],
                    ones_row[0:1, :],
                    den[0:1, :],
                    start=True,
                    stop=True,
                    tile_position=(0, 0),
                    skip_group_check=True,
                )
                nc.tensor.matmul(
                    psb[64:128, :],
                    ones_row[32:33, :],
                    den[32:33, :],
                    start=True,
                    stop=True,
                    tile_position=(32, 64),
                    skip_group_check=True,
                )
                recbc = tmp.tile([128, 512], F32, tag="recbc")
                nc.vector.tensor_copy(recbc[:, :], psb[:, :])
                nc.vector.tensor_mul(
                    aT[:, hp_, qb_, :], po_[:, :], recbc[:, :]
                )
                if hp_ == NHP - 1:
                    for et in range(ND):
                        pend_.append(oproj_unit(qb_, et))
            return emit

        def emit_scores(hp, qb, kb):
            QT, KT = qk_tiles[hp]
            qslice0 = 512 * qb
            pss = ps.tile([128, 2, 512], F32, tag="ps")
            rr = kb - 4 * qb
            qq0 = 128 * rr if rr >= 0 else 0
            for h2 in range(2):
                b0 = 64 * h2
                nc.tensor.matmul(
                    pss[:, h2, qq0:512],
                    KT[b0 : b0 + 64, 128 * kb : 128 * (kb + 1)],
                    QT[b0 : b0 + 64, qslice0 + qq0 : qslice0 + 512],
                    start=True,
                    stop=True,
                    tile_position=(b0, 0),
                    skip_group_check=True,
                )
            if rr >= 0:
                # accumulate the causal -inf triangle onto the
                # diagonal 128x128 block of both heads on the PE
                for h2 in range(2):
                    nc.tensor.matmul(
                        pss[:, h2, qq0 : qq0 + 128],
                        maskT_sb[:, :],
                        ident_sb[:, :],
                        start=False,
                        stop=True,
                        skip_group_check=True,
                    )
            return pss

        iters = [
            (hp, qb, kb)
            for hp in range(NHP)
            for qb in range(NT)
            for kb in range(4 * qb + 4)
        ]
        pending = deque()
        every = 4
        slot = 0
        po = pde = None
        pss_next = emit_scores(*iters[0])
        for idx, (hp, qb, kb) in enumerate(iters):
            if kb == 0:
                if qb == 0:
                    pending = (
                        deque(proj_units(hp + 1))
                        if hp + 1 < NHP
                        else pending
                    )
                    every = 2 if hp + 1 < NHP else 1
                    slot = 0
                po = pv.tile([128, 512], F32, tag="pv")
                pde = pdb.tile([33, 512], F32, tag="pd")
            nkb = 4 * qb + 4
            pss = pss_next
            if idx + 1 < len(iters):
                pss_next = emit_scores(*iters[idx + 1])
            slot += 1
            if pending and slot % every == 0:
                pending.popleft()()
            r = kb - 4 * qb
            q0 = 128 * r if r >= 0 else 0
            es_t = es.tile([128, 2, 512], BF16, tag="es")
            nc.scalar.activation(
                es_t[:, :, q0:512],
                pss[:, :, q0:512],
                mybir.ActivationFunctionType.Exp,
            )
            # popped after the exp so the den-waiting psb matmuls never
            # sit ahead of the next scores pair in the in-order PE queue
            if norm_b and kb == 3:
                norm_b.popleft()()
            first = kb == 0
            last = kb == nkb - 1
            for h2 in range(2):
                b0 = 64 * h2
                h_global = 2 * hp + h2
                nc.tensor.matmul(
                    po[b0 : b0 + 64, q0:512],
                    V[:, kb, 64 * h_global : 64 * (h_global + 1)],
                    es_t[:, h2, q0:512],
                    start=first,
                    stop=last,
                    tile_position=(0, b0),
                    skip_group_check=True,
                )
            for h2 in range(2):
                nc.tensor.matmul(
                    pde[32 * h2 : 32 * h2 + 1, q0:512],
                    ones_col[:, :],
                    es_t[:, h2, q0:512],
                    start=first,
                    stop=last,
                    tile_position=(0, 32 * h2),
                    skip_group_check=True,
                )
            if last:
                # reciprocal emitted right away (frees the pd slot early);
                # the broadcast+aT-mul half is deferred into the next qb
                make_norm_a((hp, qb), pde)()
                norm_b.append(make_norm_b((hp, qb), hp, qb, po, pending))

        while pending:
            pending.popleft()()
        while norm_b:
            norm_b.popleft()()
        while pending:
            pending.popleft()()

    nc.compile()
    return nc


_PERM = np.concatenate([np.arange(0, DK, 2), np.arange(1, DK, 2)])
_IN_NP = ml_dtypes.bfloat16 if USE_BF16 else np.float32


def _prep_core_inputs(x, token_positions, w_qkv, w_o, core):
    b = core // 2
    h0 = HPC * (core % 2)

    xT = np.ascontiguousarray(x[b].T.astype(_IN_NP))

    w_q = w_qkv[0 * D : 1 * D]
    w_k = w_qkv[1 * D : 2 * D]
    w_v = w_qkv[2 * D : 3 * D]

    def gather(w, permute, scale):
        rows = []
        for j in range(HPC):
            g = h0 + j
            blk = w[DK * g : DK * (g + 1)]
            if permute:
                blk = blk[_PERM]
            rows.append(blk)
        out = np.concatenate(rows, axis=0).astype(np.float32) * scale
        return np.ascontiguousarray(out.T.astype(_IN_NP))  # [D, HPC*DK]

    wq = gather(w_q, True, 1.0 / math.sqrt(DK))
    wk = gather(w_k, True, 1.0)
    wv = gather(w_v, False, 1.0)

    # w_o: [e_out, d_in]; take the d rows of this core's heads -> [512, D]
    rows = []
    for j in range(HPC):
        g = h0 + j
        rows.append(w_o[:, DK * g : DK * (g + 1)].T)
    wo = np.ascontiguousarray(
        np.concatenate(rows, axis=0).astype(ml_dtypes.bfloat16)
    )

    pos = token_positions.astype(np.float32)
    inv = (10000.0 ** (-(np.arange(0, DK, 2, dtype=np.float32)) / DK)).astype(
        np.float32
    )
    ang = pos[:, None] * inv[None, :]  # [S, 32]
    c = np.cos(ang).T.astype(np.float32)  # [32, S]
    s = np.sin(ang).T.astype(np.float32)
    C64 = np.concatenate([c, c], axis=0)
    # rows pre-swapped: row block [0:32] holds +s (multiplies x1 into the
    # x2 output slot), [32:64] holds -s (multiplies x2 into the x1 slot)
    S64 = np.concatenate([s, -s], axis=0)
    ropeC = np.ascontiguousarray(
        np.concatenate([C64, C64], axis=0).astype(ml_dtypes.bfloat16)
    )
    ropeS = np.ascontiguousarray(
        np.concatenate([S64, S64], axis=0).astype(ml_dtypes.bfloat16)
    )

    ki = np.arange(128)[:, None]
    qi = np.arange(128)[None, :]
    mask = np.where(ki <= qi, 0.0, NEG).astype(np.float32)
    maskT = np.ascontiguousarray(mask.T.astype(ml_dtypes.bfloat16))
    ident = np.eye(128, dtype=np.float32).astype(ml_dtypes.bfloat16)

    return {
        "xT": xT,
        "wq": wq,
        "wk": wk,
        "wv": wv,
        "wo": wo,
        "ropeC": ropeC,
        "ropeS": ropeS,
        "maskT": maskT,
        "ident": ident,
    }


def kernel(x, token_positions, w_qkv, w_o):
    x = np.asarray(x, dtype=np.float32)
    token_positions = np.asarray(token_positions)
    w_qkv = np.asarray(w_qkv, dtype=np.float32)
    w_o = np.asarray(w_o, dtype=np.float32)

    if "nc" not in _CACHE:
        _CACHE["nc"] = _build()
    nc = _CACHE["nc"]

    in_maps = [
        _prep_core_inputs(x, token_positions, w_qkv, w_o, c)
        for c in range(NCORES)
    ]
    res = run_bass_kernel_spmd(nc, in_maps, core_ids=list(range(NCORES)))
    _CACHE["last_results"] = res

    out = np.empty((B, S, D), dtype=np.float32)
    for b in range(B):
        yT = res.results[2 * b]["yT"] + res.results[2 * b + 1]["yT"]
        out[b] = yT.T
    return out


# revision 50
# speedup vs baseline: 1.9153x; 1.0006x over previous
"""Multi-head self-attention (RoPE, causal) on 8 trn2 NeuronCores.

Sharding: batch (4) x head-group (2x8 heads) = 8 shards, one per core.
Each core: QKV projection for its 8 heads -> RoPE -> causal flash
attention (scores kept transposed [k, q] so PV needs no transposes;
softmax denominators accumulated on the PE via ones-column matmuls) ->
partial o_proj over its 512 head-dims. Host sums the two partial
o_proj outputs of each batch pair (the tensor-parallel all-reduce) and
concatenates batches.

v3: all matmuls bf16 (f32r streams at 1.5 cyc/row on HW); po/pde pairs
emitted pair-wise so the PE column-tiles run concurrently; softmax
normalization emission deferred two iterations so the in-order PE queue
never waits on the DVE reciprocal; startup DMAs spread across idle
engine queues; aT kept in SBUF; o_proj interleaved into the last
head-pair's attention loop.
"""
import sys
import math

sys.path.insert(0, "/opt/trn_rl_repo")

import numpy as np
import ml_dtypes
from contextlib import ExitStack
from collections import deque

import concourse.bacc as bacc
import concourse.tile as tile
from concourse import mybir
from concourse.bass_utils import run_bass_kernel_spmd

B, S, D, H, DK = 4, 2048, 1024, 16, 64
NCORES = 8
ND = D // 128          # 8 d-tiles of the model dim
NT = S // 512          # 4 token super-blocks
NKT = S // 128         # 16 key/token 128-blocks
HPC = H // 2           # heads per core = 8
NHP = HPC // 2         # head-pairs per core = 4
F32 = mybir.dt.float32
F32R = mybir.dt.float32r
BF16 = mybir.dt.bfloat16
NEG = -30000.0

USE_BF16 = True        # bf16 x/w/q/k (1 cyc/row on PE) vs f32r (1.5)

_CACHE = {}


def _build():
    nc = bacc.Bacc("TRN2", target_bir_lowering=False, num_devices=NCORES)

    IDT = BF16 if USE_BF16 else F32
    ILD = BF16 if USE_BF16 else F32R

    xT_d = nc.dram_tensor("xT", [D, S], IDT, kind="ExternalInput")
    wq_d = nc.dram_tensor("wq", [D, HPC * DK], IDT, kind="ExternalInput")
    wk_d = nc.dram_tensor("wk", [D, HPC * DK], IDT, kind="ExternalInput")
    wv_d = nc.dram_tensor("wv", [D, HPC * DK], IDT, kind="ExternalInput")
    wo_d = nc.dram_tensor("wo", [HPC * DK, D], BF16, kind="ExternalInput")
    ropeC_d = nc.dram_tensor("ropeC", [128, S], BF16, kind="ExternalInput")
    ropeS_d = nc.dram_tensor("ropeS", [128, S], BF16, kind="ExternalInput")
    maskT_d = nc.dram_tensor("maskT", [128, 128], BF16, kind="ExternalInput")
    ident_d = nc.dram_tensor("ident", [128, 128], BF16, kind="ExternalInput")
    yT_d = nc.dram_tensor("yT", [D, S], F32, kind="ExternalOutput")

    with ExitStack() as ctx:
        tc = ctx.enter_context(tile.TileContext(nc))

        const = ctx.enter_context(tc.tile_pool(name="const", bufs=1))
        ps = ctx.enter_context(tc.tile_pool(name="ps", bufs=2, space="PSUM"))
        pv = ctx.enter_context(tc.tile_pool(name="pv", bufs=2, space="PSUM"))
        pdb = ctx.enter_context(tc.tile_pool(name="pdb", bufs=1, space="PSUM"))
        pq = ctx.enter_context(tc.tile_pool(name="pq", bufs=1, space="PSUM"))
        xpool = ctx.enter_context(tc.tile_pool(name="x", bufs=1))
        vpool = ctx.enter_context(tc.tile_pool(name="v", bufs=1))
        wvpool = ctx.enter_context(tc.tile_pool(name="wv", bufs=1))
        qkpool = ctx.enter_context(tc.tile_pool(name="qk", bufs=2))
        wpool = ctx.enter_context(tc.tile_pool(name="w", bufs=2))
        atpool = ctx.enter_context(tc.tile_pool(name="at", bufs=1))
        wopool = ctx.enter_context(tc.tile_pool(name="wo", bufs=1))
        es = ctx.enter_context(tc.tile_pool(name="es", bufs=4))
        tmp = ctx.enter_context(tc.tile_pool(name="tmp", bufs=1))
        ypool = ctx.enter_context(tc.tile_pool(name="y", bufs=2))

        # ---- high-priority input DMAs, spread across idle engine queues ----
        wv_sb = wvpool.tile([128, ND, HPC * DK], ILD)
        xT = xpool.tile([128, ND, S], ILD)
        for d in range(ND):
            nc.sync.dma_start(
                out=wv_sb[:, d, :],
                in_=wv_d[128 * d : 128 * (d + 1), :],
            )
            nc.gpsimd.dma_start(
                out=xT[:, d, 0:512],
                in_=xT_d[128 * d : 128 * (d + 1), 0:512],
            )
        ropeC = const.tile([128, S], BF16)
        nc.scalar.dma_start(out=ropeC, in_=ropeC_d[:, :])
        ropeS = const.tile([128, S], BF16)
        nc.scalar.dma_start(out=ropeS, in_=ropeS_d[:, :])
        for tb in range(1, NT):
            for d in range(ND):
                eng = nc.sync if (d % 2 == 0) else nc.gpsimd
                eng.dma_start(
                    out=xT[:, d, 512 * tb : 512 * (tb + 1)],
                    in_=xT_d[
                        128 * d : 128 * (d + 1), 512 * tb : 512 * (tb + 1)
                    ],
                )
        maskT_sb = const.tile([128, 128], BF16)
        nc.scalar.dma_start(out=maskT_sb[:, :], in_=maskT_d[:, :])
        ident_sb = const.tile([128, 128], BF16)
        nc.scalar.dma_start(out=ident_sb[:, :], in_=ident_d[:, :])
        wo_sb = wopool.tile([128, 4, D], BF16)
        for dd in range(4):
            nc.sync.dma_start(
                out=wo_sb[:, dd, :],
                in_=wo_d[128 * dd : 128 * (dd + 1), :],
            )

        # ---- constants -------------------------------------------------
        ones_f = const.tile([128, 1], F32)
        nc.vector.memset(ones_f, 1.0)
        ones_col = const.tile([128, 1], BF16)
        nc.vector.tensor_copy(ones_col, ones_f)
        ones_row_f = const.tile([33, 64], F32)
        nc.vector.memset(ones_row_f, 1.0)
        ones_row = const.tile([33, 64], BF16)
        nc.vector.tensor_copy(ones_row, ones_row_f)
        # warm the ACT exp table set before any copies run on it
        warm = const.tile([128, 8], F32)
        nc.vector.memset(warm, 0.0)
        warm_out = const.tile([128, 8], BF16)
        nc.scalar.activation(
            warm_out, warm, mybir.ActivationFunctionType.Exp
        )
        # keep the PE busy through the startup DMA wait so the HAM clock
        # gate is at full rate (K=8/8) when the real matmuls arrive
        ones128 = const.tile([128, 64], BF16)
        nc.vector.memset(ones128, 1.0)
        pwarm = pq.tile([128, 512], F32, tag="pq", name="pwarm")
        for _ in range(64):
            nc.tensor.matmul(
                pwarm[0:64, 0:64],
                ones128[:, :],
                ones128[:, :],
                start=True,
                stop=True,
                skip_group_check=True,
            )

        V = vpool.tile([128, NKT, HPC * DK], BF16)
        aT = atpool.tile([128, NHP, NT, 512], BF16)

        # ---- V projection units (PSUM->SBUF cast on scalar engine) -----
        def v_unit(t):
            def emit():
                psv_t = ps.tile([128, 1024], F32, tag="ps")
                psv = psv_t[:, 0:512]
                for d in range(ND):
                    nc.tensor.matmul(
                        psv[:, :],
                        xT[:, d, 128 * t : 128 * (t + 1)],
                        wv_sb[:, d, :],
                        start=(d == 0),
                        stop=(d == ND - 1),
                    )
                nc.scalar.copy(V[:, t, :], psv[:, :])
            return emit

        # ---- per head-pair Q^T/K^T projection + rope units -------------
        qk_tiles = {}

        def proj_units(hp):
            qt_tile = qkpool.tile([128, S], IDT, tag="qt", name=f"qt{hp}")
            kt_tile = qkpool.tile([128, S], IDT, tag="kt", name=f"kt{hp}")
            qk_tiles[hp] = (qt_tile, kt_tile)
            units = []
            state = {}

            def dma_unit(w_d, wtag):
                def emit():
                    wt = wpool.tile(
                        [128, ND, 128], ILD, tag=wtag, name=f"{wtag}{hp}"
                    )
                    for d in range(ND):
                        nc.gpsimd.dma_start(
                            out=wt[:, d, :],
                            in_=w_d[
                                128 * d : 128 * (d + 1),
                                128 * hp : 128 * (hp + 1),
                            ],
                        )
                    state[wtag] = wt
                return emit

            def mm_unit(wtag, tb):
                def emit():
                    wt = state[wtag]
                    psq = pq.tile([128, 512], F32, tag="pq")
                    for d in range(ND):
                        nc.tensor.matmul(
                            psq[:, :],
                            wt[:, d, :],
                            xT[:, d, 512 * tb : 512 * (tb + 1)],
                            start=(d == 0),
                            stop=(d == ND - 1),
                        )
                    # single fast PSUM read frees the psum ring slot; the
                    # rope math then runs from SBUF bf16 at 2x DVE rate
                    psq_bf = tmp.tile(
                        [128, 512], BF16, tag="psqbf", bufs=2
                    )
                    nc.vector.tensor_copy(psq_bf[:, :], psq[:, :])
                    state[(wtag, tb)] = psq_bf
                return emit

            def rope_unit(wtag, OUT, tb):
                def emit():
                    psq_bf = state.pop((wtag, tb))
                    # rope: out = psq*C + swap32(psq)*S
                    t2 = tmp.tile([128, 512], BF16, tag="t2")
                    cs = slice(512 * tb, 512 * (tb + 1))
                    # ropeS rows are pre-swapped host-side so each mul's two
                    # SBUF inputs share a base partition (walrus requirement)
                    for h2 in range(2):
                        b0 = 64 * h2
                        nc.vector.tensor_mul(
                            t2[b0 : b0 + 32, :],
                            psq_bf[b0 + 32 : b0 + 64, :],
                            ropeS[b0 + 32 : b0 + 64, cs],
                        )
                        nc.vector.tensor_mul(
                            t2[b0 + 32 : b0 + 64, :],
                            psq_bf[b0 : b0 + 32, :],
                            ropeS[b0 : b0 + 32, cs],
                        )
                    t1 = tmp.tile([128, 512], BF16, tag="t1")
                    nc.vector.tensor_mul(t1[:, :], psq_bf[:, :], ropeC[:, cs])
                    nc.vector.tensor_add(OUT[:, cs], t1[:, :], t2[:, :])
                return emit

            # both weight DMAs pop first: wk then has ~20 iterations of
            # lead over its first consuming matmuls instead of ~2, so the
            # in-order PE queue never stalls on the wk transfer
            units.append(dma_unit(wq_d, "wq"))
            units.append(dma_unit(wk_d, "wk"))
            for wtag, outt in (("wq", qt_tile), ("wk", kt_tile)):
                for tb in range(NT):
                    units.append(mm_unit(wtag, tb))
                    units.append(rope_unit(wtag, outt, tb))
            return units

        # ---- o_proj units (aT in SBUF, bf16) ---------------------------
        def oproj_unit(tb, et):
            def emit():
                psy = pq.tile([128, 512], F32, tag="pq")
                for dd in range(4):
                    nc.tensor.matmul(
                        psy[:, :],
                        wo_sb[:, dd, 128 * et : 128 * (et + 1)],
                        aT[:, dd, tb, :],
                        start=(dd == 0),
                        stop=(dd == 3),
                    )
                y_t = ypool.tile([128, 512], F32, tag="y")
                nc.vector.tensor_copy(y_t[:, :], psy[:, :])
                nc.sync.dma_start(
                    out=yT_d[
                        128 * et : 128 * (et + 1),
                        512 * tb : 512 * (tb + 1),
                    ],
                    in_=y_t[:, :],
                )
            return emit

        # ---- phase 0: V projection + head-pair-0 projection ------------
        v_units = [v_unit(t) for t in range(NKT)]
        p0_units = proj_units(0)
        merged = list(v_units[:4])
        i = j = 0
        rest_v = v_units[4:]
        while i < len(p0_units) or j < len(rest_v):
            if j < len(rest_v):
                merged.append(rest_v[j])
                j += 1
            if i < len(p0_units):
                merged.append(p0_units[i])
                i += 1
        for u in merged:
            u()

        # ---- attention (o_proj interleaved into last head-pair) --------
        norm_a = deque()   # recip + bf16 convert (DVE), popped early
        norm_b = deque()   # broadcast matmuls + aT mul, popped later
        norm_state = {}

        def make_norm_a(key, pde_):
            def emit():
                # fast approx reciprocal of the softmax denominators
                # (rows 1-31,33+ of pde are unused garbage)
                den_f = tmp.tile([33, 512], F32, tag="denf")
                nc.vector.reciprocal_approx_fast(den_f[:, :], pde_[:, :])
                den = tmp.tile([33, 512], BF16, tag="den")
                nc.vector.tensor_copy(den[:, :], den_f[:, :])
                norm_state[key] = den
            return emit

        def make_norm_b(key, hp_, qb_, po_, pend_):
            def emit():
                den = norm_state.pop(key)
                psb = pq.tile([128, 512], F32, tag="pq")
                nc.tensor.matmul(
                    psb[0:64, :],
                    ones_row[0:1, :],
                    den[0:1, :],
                    start=True,
                    stop=True,
                    tile_position=(0, 0),
                    skip_group_check=True,
                )
                nc.tensor.matmul(
                    psb[64:128, :],
                    ones_row[32:33, :],
                    den[32:33, :],
                    start=True,
                    stop=True,
                    tile_position=(32, 64),
                    skip_group_check=True,
                )
                recbc = tmp.tile([128, 512], F32, tag="recbc")
                nc.vector.tensor_copy(recbc[:, :], psb[:, :])
                nc.vector.tensor_mul(
                    aT[:, hp_, qb_, :], po_[:, :], recbc[:, :]
                )
                if hp_ == NHP - 1:
                    for et in range(ND):
                        pend_.append(oproj_unit(qb_, et))
            return emit

        def emit_scores(hp, qb, kb):
            QT, KT = qk_tiles[hp]
            qslice0 = 512 * qb
            pss = ps.tile([128, 2, 512], F32, tag="ps")
            rr = kb - 4 * qb
            qq0 = 128 * rr if rr >= 0 else 0
            for h2 in range(2):
                b0 = 64 * h2
                nc.tensor.matmul(
                    pss[:, h2, qq0:512],
                    KT[b0 : b0 + 64, 128 * kb : 128 * (kb + 1)],
                    QT[b0 : b0 + 64, qslice0 + qq0 : qslice0 + 512],
                    start=True,
                    stop=True,
                    tile_position=(b0, 0),
                    skip_group_check=True,
                )
            if rr >= 0:
                # accumulate the causal -inf triangle onto the
                # diagonal 128x128 block of both heads on the PE
                for h2 in range(2):
                    nc.tensor.matmul(
                        pss[:, h2, qq0 : qq0 + 128],
                        maskT_sb[:, :],
                        ident_sb[:, :],
                        start=False,
                        stop=True,
                        skip_group_check=True,
                    )
            return pss

        iters = [
            (hp, qb, kb)
            for hp in range(NHP)
            for qb in range(NT)
            for kb in range(4 * qb + 4)
        ]
        pending = deque()
        every = 4
        slot = 0
        po = pde = None
        pss_next = emit_scores(*iters[0])
        for idx, (hp, qb, kb) in enumerate(iters):
            if kb == 0:
                if qb == 0:
                    pending = (
                        deque(proj_units(hp + 1))
                        if hp + 1 < NHP
                        else pending
                    )
                    every = 2 if hp + 1 < NHP else 1
                    slot = 0
                po = pv.tile([128, 512], F32, tag="pv")
                pde = pdb.tile([33, 512], F32, tag="pd")
            nkb = 4 * qb + 4
            pss = pss_next
            if idx + 1 < len(iters):
                pss_next = emit_scores(*iters[idx + 1])
            slot += 1
            if pending and slot % every == 0:
                pending.popleft()()
            r = kb - 4 * qb
            q0 = 128 * r if r >= 0 else 0
            es_t = es.tile([128, 2, 512], BF16, tag="es")
            nc.scalar.activation(
                es_t[:, :, q0:512],
                pss[:, :, q0:512],
                mybir.ActivationFunctionType.Exp,
            )
            # popped after the exp so the den-waiting psb matmuls never
            # sit ahead of the next scores pair in the in-order PE queue
            if norm_b and kb == 3:
                norm_b.popleft()()
            first = kb == 0
            last = kb == nkb - 1
            for h2 in range(2):
                b0 = 64 * h2
                h_global = 2 * hp + h2
                nc.tensor.matmul(
                    po[b0 : b0 + 64, q0:512],
                    V[:, kb, 64 * h_global : 64 * (h_global + 1)],
                    es_t[:, h2, q0:512],
                    start=first,
                    stop=last,
                    tile_position=(0, b0),
                    skip_group_check=True,
                )
            for h2 in range(2):
                nc.tensor.matmul(
                    pde[32 * h2 : 32 * h2 + 1, q0:512],
                    ones_col[:, :],
                    es_t[:, h2, q0:512],
                    start=first,
                    stop=last,
                    tile_position=(0, 32 * h2),
                    skip_group_check=True,
                )
            if last:
                # reciprocal emitted right away (frees the pd slot early);
                # the broadcast+aT-mul half is deferred into the next qb
                make_norm_a((hp, qb), pde)()
                norm_b.append(make_norm_b((hp, qb), hp, qb, po, pending))

        while pending:
            pending.popleft()()
        while norm_b:
            norm_b.popleft()()
        while pending:
            pending.popleft()()

    nc.compile()
    return nc


_PERM = np.concatenate([np.arange(0, DK, 2), np.arange(1, DK, 2)])
_IN_NP = ml_dtypes.bfloat16 if USE_BF16 else np.float32


def _prep_core_inputs(x, token_positions, w_qkv, w_o, core):
    b = core // 2
    h0 = HPC * (core % 2)

    xT = np.ascontiguousarray(x[b].T.astype(_IN_NP))

    w_q = w_qkv[0 * D : 1 * D]
    w_k = w_qkv[1 * D : 2 * D]
    w_v = w_qkv[2 * D : 3 * D]

    def gather(w, permute, scale):
        rows = []
        for j in range(HPC):
            g = h0 + j
            blk = w[DK * g : DK * (g + 1)]
            if permute:
                blk = blk[_PERM]
            rows.append(blk)
        out = np.concatenate(rows, axis=0).astype(np.float32) * scale
        return np.ascontiguousarray(out.T.astype(_IN_NP))  # [D, HPC*DK]

    wq = gather(w_q, True, 1.0 / math.sqrt(DK))
    wk = gather(w_k, True, 1.0)
    wv = gather(w_v, False, 1.0)

    # w_o: [e_out, d_in]; take the d rows of this core's heads -> [512, D]
    rows = []
    for j in range(HPC):
        g = h0 + j
        rows.append(w_o[:, DK * g : DK * (g + 1)].T)
    wo = np.ascontiguousarray(
        np.concatenate(rows, axis=0).astype(ml_dtypes.bfloat16)
    )

    pos = token_positions.astype(np.float32)
    inv = (10000.0 ** (-(np.arange(0, DK, 2, dtype=np.float32)) / DK)).astype(
        np.float32
    )
    ang = pos[:, None] * inv[None, :]  # [S, 32]
    c = np.cos(ang).T.astype(np.float32)  # [32, S]
    s = np.sin(ang).T.astype(np.float32)
    C64 = np.concatenate([c, c], axis=0)
    # rows pre-swapped: row block [0:32] holds +s (multiplies x1 into the
    # x2 output slot), [32:64] holds -s (multiplies x2 into the x1 slot)
    S64 = np.concatenate([s, -s], axis=0)
    ropeC = np.ascontiguousarray(
        np.concatenate([C64, C64], axis=0).astype(ml_dtypes.bfloat16)
    )
    ropeS = np.ascontiguousarray(
        np.concatenate([S64, S64], axis=0).astype(ml_dtypes.bfloat16)
    )

    ki = np.arange(128)[:, None]
    qi = np.arange(128)[None, :]
    mask = np.where(ki <= qi, 0.0, NEG).astype(np.float32)
    maskT = np.ascontiguousarray(mask.T.astype(ml_dtypes.bfloat16))
    ident = np.eye(128, dtype=np.float32).astype(ml_dtypes.bfloat16)

    return {
        "xT": xT,
        "wq": wq,
        "wk": wk,
        "wv": wv,
        "wo": wo,
        "ropeC": ropeC,
        "ropeS": ropeS,
        "maskT": maskT,
        "ident": ident,
    }


def kernel(x, token_positions, w_qkv, w_o):
    x = np.asarray(x, dtype=np.float32)
    token_positions = np.asarray(token_positions)
    w_qkv = np.asarray(w_qkv, dtype=np.float32)
    w_o = np.asarray(w_o, dtype=np.float32)

    if "nc" not in _CACHE:
        _CACHE["nc"] = _build()
    nc = _CACHE["nc"]

    in_maps = [
        _prep_core_inputs(x, token_positions, w_qkv, w_o, c)
        for c in range(NCORES)
    ]
    res = run_bass_kernel_spmd(nc, in_maps, core_ids=list(range(NCORES)))
    _CACHE["last_results"] = res

    out = np.empty((B, S, D), dtype=np.float32)
    for b in range(B):
        yT = res.results[2 * b]["yT"] + res.results[2 * b + 1]["yT"]
        out[b] = yT.T
    return out
